# revision 14
# baseline (speedup 1.0000x reference)
"""GNN max-pool message passing kernel for 8 Trainium2 NeuronCores.

Problem: out[n] = max_k s_feats[neighbor_indices[n, k]]  (N=50000, K=32, D=128)

Strategy: data-parallel over destination nodes per the sharding hint;
s_feats is replicated into every core's HBM and each core handles 6250
destination nodes.

Variant "gbf16" (current): the f32 trace showed the 16 SDMA engines ~88%
busy moving 512 B/descriptor (~21 GB/s/engine) — the gather is DMA-engine
byte-throughput-bound, not Q7 descriptor-emission-bound. So the table is
converted to bf16 on the host (tolerance is 2e-2; bf16 rounding is ~4e-3):

  - One InstDMAGatherAnt per 128-node chunk gathers all K=32 neighbor rows
    (256 B descriptors) from HBM with signed int16 indices relative to
    table row BASE (unsigned-stride x signed-index Q7 address math covers
    rows BASE-32768..BASE+32767 => BASE=25000 spans the whole table).
  - Each call carries one dummy tail block of zero offsets so the Q7's
    trailing-negative trim can never drop real descriptors.
  - Calls round-robin over all 4 SWDGE queues; single_packet=False.
  - The K-reduction is a tensor_tensor(max) binary tree over contiguous
    bf16 slices (TensorReduce has NO DVE perf mode — a strided reduce runs
    1 elem/cycle and was 350 us of DVE busy in the f32 baseline; the
    tensor_max tree on packed 2-byte data runs in 2x_1p mode at 0.5
    cyc/elem: ~2.7 us/chunk).
  - Output stays bf16 on HW (exact — max of bf16 inputs) and is converted
    to f32 on the host.

Layout per core:
  - node n -> (chunk c = n // 128, partition p = n % 128); call position
    m = k*128 + p so gathered block k of partition p is neighbor k of node
    (c, p); the output store is a strided HWDGE DMA every STORE_GROUP
    chunks; the 6250 real rows are a contiguous prefix of the 6272-row
    padded output.
  - idx input [128, ncalls*264] int16: per call 4224 positions wrapped
    16-wide (position m -> lane m%16, slot m//16), replicated to all eight
    16-partition groups as InstDMAGatherAnt expects.

Variant "gather" is the older f32 version (measured 489 us on 8 cores).
"""

import numpy as np

N_NODES = 50000
K = 32
D = 128
N_CORES = 8
P = 128
NODES_PER_CORE = N_NODES // N_CORES  # 6250
SLOTS = (NODES_PER_CORE + P - 1) // P  # 49
PADDED = P * SLOTS  # 6272
CHUNKS = PADDED // P  # 49 chunks of 128 nodes

VARIANT = "gpair"  # "gpair" | "gpkt" | "gbf16" | "gather"

_nc_cache = {}


# ---------------------------------------------------------------- gpair ---
# The Q7 dma_gather ucode runs one instruction at a time across the whole
# GpSimd cluster and its descriptor-emission loop costs ~2.3 ns per index
# POSITION regardless of elem_size (up to 16 KB/descriptor) — so kernel
# time is ~(total index positions) x 2.3 ns. This variant cuts positions
# ~19%: the host builds a per-core table permutation pi (greedy max-weight
# path forest over neighbor co-occurrence pairs) so that many nodes have
# two neighbors at consecutive pi positions; one 512 B "pair" descriptor
# (row j of a [49999, 256] sliding-window pair table = pi-rows j, j+1)
# then serves both. Nodes are re-bucketed into chunks by their pair count
# p_n (descending) and each chunk c uses the shared schedule P_c =
# min(p_n in chunk, over all cores): a pair call of P_c blocks (elem 256)
# plus single calls totalling 32-2*P_c blocks (elem 128). No sentinel:
# the slot-127 node of each chunk is chosen/reordered so every call's
# last index is non-negative (trailing-negative trim never fires).
GPR_BASE = 25000  # signed int16 offsets for both tables
GPR_STORE_GROUP = 8


def _gpair_path_forest(sets, n_rows=N_NODES, seed=0):
    """Greedy max-weight path forest over co-occurrence pairs.
    Returns pi (permutation of rows) maximizing per-set adjacent pairs."""
    rng = np.random.default_rng(seed)
    i, j = np.triu_indices(K, 1)
    pairs = np.stack([sets[:, i], sets[:, j]], axis=2).reshape(-1, 2)
    pairs = np.sort(pairs, axis=1)
    pairs = pairs[pairs[:, 0] != pairs[:, 1]]
    pu, counts = np.unique(
        pairs[:, 0].astype(np.int64) * n_rows + pairs[:, 1], return_counts=True
    )
    u = (pu // n_rows).astype(np.int32)
    v = (pu % n_rows).astype(np.int32)
    order = np.lexsort((rng.random(len(u)), -counts))
    u, v = u[order], v[order]
    deg = np.zeros(n_rows, np.int8)
    parent = np.arange(n_rows, dtype=np.int32)

    def find(x):
        while parent[x] != x:
            parent[x] = parent[parent[x]]
            x = parent[x]
        return x

    adj = [[] for _ in range(n_rows)]
    for uu, vv in zip(u.tolist(), v.tolist()):
        if deg[uu] >= 2 or deg[vv] >= 2:
            continue
        ru, rv = find(uu), find(vv)
        if ru == rv:
            continue
        parent[ru] = rv
        deg[uu] += 1
        deg[vv] += 1
        adj[uu].append(vv)
        adj[vv].append(uu)
    visited = np.zeros(n_rows, bool)
    pi = []
    for s in range(n_rows):
        if visited[s] or len(adj[s]) == 2:
            continue
        cur, prev = s, -1
        while True:
            pi.append(cur)
            visited[cur] = True
            nxt = [x for x in adj[cur] if x != prev and not visited[x]]
            if not nxt:
                break
            prev, cur = cur, nxt[0]
    for s in range(n_rows):
        if not visited[s]:
            pi.append(s)
    pi = np.asarray(pi, np.int32)
    assert len(pi) == n_rows
    return pi


def _gpair_phase1(sets):
    """Per-core: pi, per-node pair cover. Returns dict with pos-sorted rows,
    chosen-pair flags and per-node pair counts."""
    pi = _gpair_path_forest(sets)
    pos = np.empty(N_NODES, np.int64)
    pos[pi] = np.arange(N_NODES)
    ps = np.sort(pos[sets], axis=1).astype(np.int32)  # [M, K] pi positions
    d1 = np.diff(ps, axis=1) == 1
    m = len(sets)
    pair_at = np.zeros((m, K - 1), bool)  # cover takes (col, col+1)
    prev = np.zeros(m, bool)
    for col in range(K - 1):
        can = d1[:, col] & ~prev
        pair_at[:, col] = can
        prev = can
    p_n = pair_at.sum(axis=1).astype(np.int32)
    return {"pi": pi, "ps": ps, "pair_at": pair_at, "p_n": p_n}


def _gpair_calls_for_chunk(pc):
    """Call list for a chunk: (is_pair, blocks) per call. Single calls are
    split into ~8-block pieces so call sizes stay balanced across the four
    SWDGE queues (the Q7 cluster emits ~2 ns/position aggregate only when
    concurrent calls are comparable in size)."""
    calls = []
    if pc > 0:
        calls.append((True, pc))
    s = K - 2 * pc
    while s > 0:
        b = min(s, 8)
        calls.append((False, b))
        s -= b
    return calls


def _gpair_phase2(core_data, P_sched):
    """Per-core: order nodes, build per-call idx array. Returns idx array
    [128, total_slots] int16 and node order (orig local id per padded slot)."""
    ps, pair_at, p_n = core_data["ps"], core_data["pair_at"], core_data["p_n"]
    m = len(ps)
    order = np.argsort(-p_n, kind="stable").astype(np.int32)
    # pads at the end: orig id -1
    order_pad = np.concatenate([order, np.full(PADDED - m, -1, np.int32)])
    all_vals = []
    for c in range(CHUNKS):
        pc = P_sched[c]
        nodes = order_pad[c * P : (c + 1) * P]
        # per node: pc pair starts + (K-2*pc) singles
        pairs_l = np.zeros((P, pc), np.int32)
        singles_l = np.zeros((P, K - 2 * pc), np.int32)
        for sl in range(P):
            n = nodes[sl]
            if n < 0:
                pairs_l[sl] = GPR_BASE  # pad: harmless pair/single reads
                singles_l[sl] = GPR_BASE
                continue
            cols = np.nonzero(pair_at[n])[0]
            use = cols[:pc]
            pstarts = ps[n][use]
            covered = np.zeros(K, bool)
            covered[use] = True
            covered[use + 1] = True
            sing = ps[n][~covered]
            pairs_l[sl] = pstarts
            singles_l[sl] = sing
        # slot-127: ensure last idx of each call is >= BASE; reorder node
        # lists, swapping in a suitable node if needed
        calls = _gpair_calls_for_chunk(pc)

        def fix(sl):
            okp = pc == 0 or (pairs_l[sl] >= GPR_BASE).any()
            ns_calls = sum(1 for ispair, _ in calls if not ispair)
            oks = ns_calls == 0 or (singles_l[sl] >= GPR_BASE).sum() >= ns_calls
            return okp and oks

        if not fix(127):
            for sl in range(P):
                if fix(sl):
                    pairs_l[[127, sl]] = pairs_l[[sl, 127]]
                    singles_l[[127, sl]] = singles_l[[sl, 127]]
                    nodes = nodes.copy()
                    nodes[[127, sl]] = nodes[[sl, 127]]
                    order_pad[c * P : (c + 1) * P] = nodes
                    break
            else:
                raise AssertionError(f"chunk {c}: no slot-127 candidate")
        # put a non-negative pair last for slot 127
        if pc > 0:
            pl = pairs_l[127]
            w = np.nonzero(pl >= GPR_BASE)[0]
            if len(w) and w[-1] != pc - 1:
                pl[[w[-1], pc - 1]] = pl[[pc - 1, w[-1]]]
        # distribute slot-127 singles: one non-negative at the end of each
        # single call
        s127 = singles_l[127]
        nonneg = s127[s127 >= GPR_BASE]
        neg = s127[s127 < GPR_BASE]
        ns_calls = [b for ispair, b in calls if not ispair]
        if ns_calls:
            assert len(nonneg) >= len(ns_calls)
            rest = np.concatenate([neg, nonneg[len(ns_calls):]])
            new = np.empty(len(s127), np.int32)
            ends = np.cumsum(ns_calls) - 1
            new[ends] = nonneg[: len(ns_calls)]
            mask = np.ones(len(s127), bool)
            mask[ends] = False
            new[mask] = rest
            singles_l[127] = new
        # emit call index values, position m = b*128 + p
        off_s = 0
        for ispair, b in calls:
            if ispair:
                vals = (pairs_l[:, :b].T - GPR_BASE).astype(np.int16)  # [b, P]
            else:
                vals = (singles_l[:, off_s : off_s + b].T - GPR_BASE).astype(
                    np.int16
                )
                off_s += b
            all_vals.append(vals.reshape(-1))  # positions m=b*128+p
    flat = np.concatenate(all_vals)  # multiple of 16
    lanes = flat.reshape(-1, 16).T  # [16, total_slots]
    full = np.tile(np.ascontiguousarray(lanes), (8, 1))
    return full, order_pad


def _prep_gpair(s_feats, neighbor_indices):
    import ml_dtypes

    s = np.ascontiguousarray(np.asarray(s_feats), dtype=np.float32).astype(
        ml_dtypes.bfloat16
    )
    nb = np.asarray(neighbor_indices)
    cores = []
    for core in range(N_CORES):
        sets = nb[core * NODES_PER_CORE : (core + 1) * NODES_PER_CORE].astype(
            np.int32
        )
        cores.append(_gpair_phase1(sets))
    # shared schedule: per-chunk min pair count across cores; chunks
    # containing pad nodes get 0
    sorted_pn = [np.sort(c["p_n"])[::-1] for c in cores]
    P_sched = []
    for c in range(CHUNKS):
        if (c + 1) * P > NODES_PER_CORE:
            P_sched.append(0)
        else:
            P_sched.append(
                min(int(sp[(c + 1) * P - 1]) for sp in sorted_pn)
            )
    P_sched = tuple(P_sched)
    in_maps = []
    orders = []
    for core in range(N_CORES):
        idx_full, order_pad = _gpair_phase2(cores[core], P_sched)
        table = s[cores[core]["pi"]]
        ptable = np.ascontiguousarray(
            np.concatenate([table[:-1], table[1:]], axis=1)
        )
        in_maps.append({"table": table, "ptable": ptable, "idx": idx_full})
        orders.append(order_pad)
    return in_maps, P_sched, orders


def _build_nc_gpair(P_sched):
    import concourse.bacc as bacc
    import concourse.mybir as mybir
    import concourse.tile as tile

    nc = bacc.Bacc(
        "TRN2", target_bir_lowering=False, debug=False,
        dynamic_dma_scratch_size=49152, num_swdge_queues=4,
    )
    table = nc.dram_tensor(
        "table", [N_NODES, D], mybir.dt.bfloat16, kind="ExternalInput"
    ).ap()
    ptable = nc.dram_tensor(
        "ptable", [N_NODES - 1, 2 * D], mybir.dt.bfloat16, kind="ExternalInput"
    ).ap()
    total_slots = sum(
        b * P // 16 for c in range(CHUNKS) for _, b in _gpair_calls_for_chunk(P_sched[c])
    )
    idx = nc.dram_tensor(
        "idx", [P, total_slots], mybir.dt.int16, kind="ExternalInput"
    ).ap()
    out = nc.dram_tensor(
        "out", [PADDED, D], mybir.dt.bfloat16, kind="ExternalOutput"
    ).ap()

    max_pair_blocks = 2 * max(P_sched)  # width-128 blocks in a pair call
    with tile.TileContext(nc) as tc:
        with (
            tc.tile_pool(name="pool", bufs=1) as pool,
            tc.tile_pool(name="pstage", bufs=6) as pstage_pool,
            tc.tile_pool(name="sstage", bufs=12) as sstage_pool,
            tc.tile_pool(name="tmp", bufs=10) as tmp_pool,
            tc.tile_pool(name="parts", bufs=24) as part_pool,
        ):
            idx_sb = pool.tile([P, total_slots], mybir.dt.int16, name="idx_sb")
            head_cols = min(total_slots, 1024)
            nc.sync.dma_start(out=idx_sb[:, :head_cols], in_=idx[:, :head_cols])
            if head_cols < total_slots:
                nc.sync.dma_start(
                    out=idx_sb[:, head_cols:], in_=idx[:, head_cols:]
                )

            res = pool.tile([P, CHUNKS * D], mybir.dt.bfloat16, name="res")
            out_view = out.rearrange("(c p) d -> p c d", p=P)
            res_view = res[:, :].rearrange("p (c d) -> p c d", d=D)

            TMP_ELEMS = max(max_pair_blocks, 16) // 2 * D

            def tree_reduce(st, nblocks):
                """Max-reduce st[:, :nblocks*D] (width-D blocks) to one
                [P, D] block. Top-level nblocks is even, so st is released
                after the first op. Returns (tile, offset)."""
                stragglers = []
                cur, cur_off, n = st, 0, nblocks
                while n > 1:
                    h = n // 2
                    if n % 2:
                        stragglers.append((cur, cur_off + (n - 1) * D))
                    dst = tmp_pool.tile(
                        [P, TMP_ELEMS], mybir.dt.bfloat16, tag="tmp", name="tr"
                    )
                    nc.vector.tensor_max(
                        out=dst[:, : h * D],
                        in0=cur[:, cur_off : cur_off + h * D],
                        in1=cur[:, cur_off + h * D : cur_off + 2 * h * D],
                    )
                    cur, cur_off, n = dst, 0, h
                for sg, off in stragglers:
                    dst = part_pool.tile(
                        [P, D], mybir.dt.bfloat16, tag="pt", name="sg"
                    )
                    nc.vector.tensor_max(
                        out=dst[:, :],
                        in0=cur[:, cur_off : cur_off + D],
                        in1=sg[:, off : off + D],
                    )
                    cur, cur_off = dst, 0
                return cur, cur_off

            rr = 0
            col = 0
            for c in range(CHUNKS):
                calls = _gpair_calls_for_chunk(P_sched[c])
                partials = []  # (tile, off), each one [P, D] block
                for ispair, b in calls:
                    elem = 2 * D if ispair else D
                    nidx = b * P
                    slots = nidx // 16
                    wblocks = 2 * b if ispair else b  # width-128 view
                    st = (pstage_pool if ispair else sstage_pool).tile(
                        [P, max_pair_blocks * D if ispair else 8 * D],
                        mybir.dt.bfloat16,
                        tag="pst" if ispair else "sst",
                        name="st",
                    )
                    nc.gpsimd.dma_gather(
                        out_ap=st[:, : b * elem].rearrange(
                            "p (b d) -> p b d", d=elem
                        ),
                        in_ap=(ptable if ispair else table)[GPR_BASE:, :],
                        idxs_ap=idx_sb[:, col : col + slots],
                        num_idxs=nidx,
                        num_idxs_reg=nidx,
                        elem_size=elem,
                        single_packet=False,
                        queue_num=rr % 4,
                    )
                    rr += 1
                    col += slots
                    partials.append(tree_reduce(st, wblocks))
                # combine the 1-3 per-call partials into the result slice
                sink = res[:, c * D : (c + 1) * D]
                if len(partials) == 1:
                    (t0, o0) = partials[0]
                    nc.vector.tensor_max(
                        out=sink, in0=t0[:, o0 : o0 + D], in1=t0[:, o0 : o0 + D]
                    )
                else:
                    while len(partials) > 2:
                        (t0, o0), (t1, o1) = partials[0], partials[1]
                        pt = part_pool.tile(
                            [P, 256], mybir.dt.bfloat16, tag="pt", name="cmb"
                        )
                        nc.vector.tensor_max(
                            out=pt[:, :D],
                            in0=t0[:, o0 : o0 + D],
                            in1=t1[:, o1 : o1 + D],
                        )
                        partials = [(pt, 0)] + partials[2:]
                    (t0, o0), (t1, o1) = partials[0], partials[1]
                    nc.vector.tensor_max(
                        out=sink, in0=t0[:, o0 : o0 + D], in1=t1[:, o1 : o1 + D]
                    )
                if c % GPR_STORE_GROUP == GPR_STORE_GROUP - 1 or c == CHUNKS - 1:
                    c0 = (c // GPR_STORE_GROUP) * GPR_STORE_GROUP
                    nc.sync.dma_start(
                        out=out_view[:, c0 : c + 1, :], in_=res_view[:, c0 : c + 1, :]
                    )

    nc.compile()
    return nc


# ----------------------------------------------------------------- gpkt ---
# Like gbf16 but with 1024-index calls and single_packet=True so the Q7
# emits aggregated 64-descriptor packets per ring lane. No dummy sentinel:
# the host permutes the neighbors of each partition-127 node so the last
# unwrapped position of every call holds a non-negative offset (the
# trailing-negative trim then never fires).
GPK_BASE = 25000
GPK_KB = 8  # neighbor blocks per call
GPK_CPC = K // GPK_KB  # 4 calls per chunk
GPK_CALL_IDXS = GPK_KB * P  # 1024 = 64 descriptors per ring lane
GPK_CALL_SLOTS = GPK_CALL_IDXS // 16  # 64
GPK_STORE_GROUP = 8


def _build_nc_gpkt():
    import concourse.bacc as bacc
    import concourse.mybir as mybir
    import concourse.tile as tile

    nc = bacc.Bacc(
        "TRN2", target_bir_lowering=False, debug=False,
        dynamic_dma_scratch_size=49152, num_swdge_queues=4,
    )
    table = nc.dram_tensor(
        "table", [N_NODES, D], mybir.dt.bfloat16, kind="ExternalInput"
    ).ap()
    ncalls = CHUNKS * GPK_CPC
    idx = nc.dram_tensor(
        "idx", [P, ncalls * GPK_CALL_SLOTS], mybir.dt.int16, kind="ExternalInput"
    ).ap()
    out = nc.dram_tensor(
        "out", [PADDED, D], mybir.dt.bfloat16, kind="ExternalOutput"
    ).ap()

    with tile.TileContext(nc) as tc:
        with (
            tc.tile_pool(name="pool", bufs=1) as pool,
            tc.tile_pool(name="stage", bufs=12) as stage_pool,
            tc.tile_pool(name="tmp", bufs=8) as tmp_pool,
            tc.tile_pool(name="parts", bufs=12) as part_pool,
        ):
            idx_sb = pool.tile(
                [P, ncalls * GPK_CALL_SLOTS], mybir.dt.int16, name="idx_sb"
            )
            head_cols = 16 * GPK_CALL_SLOTS
            nc.sync.dma_start(out=idx_sb[:, :head_cols], in_=idx[:, :head_cols])
            nc.sync.dma_start(out=idx_sb[:, head_cols:], in_=idx[:, head_cols:])

            res = pool.tile([P, CHUNKS * D], mybir.dt.bfloat16, name="res")
            out_view = out.rearrange("(c p) d -> p c d", p=P)
            res_view = res[:, :].rearrange("p (c d) -> p c d", d=D)

            for c in range(CHUNKS):
                parts = []
                for h in range(GPK_CPC):
                    j = c * GPK_CPC + h
                    st = stage_pool.tile(
                        [P, GPK_KB * D], mybir.dt.bfloat16, tag="stage", name="st"
                    )
                    nc.gpsimd.dma_gather(
                        out_ap=st[:, :].rearrange("p (b d) -> p b d", d=D),
                        in_ap=table[GPK_BASE:, :],
                        idxs_ap=idx_sb[
                            :, j * GPK_CALL_SLOTS : (j + 1) * GPK_CALL_SLOTS
                        ],
                        num_idxs=GPK_CALL_IDXS,
                        num_idxs_reg=GPK_CALL_IDXS,
                        elem_size=D,
                        single_packet=True,
                        queue_num=j % 4,
                    )
                    t = tmp_pool.tile([P, 768], mybir.dt.bfloat16, tag="tmp", name="t")
                    pt = part_pool.tile([P, D], mybir.dt.bfloat16, tag="pt", name="pt")
                    nc.vector.tensor_max(
                        out=t[:, 0:512], in0=st[:, 0:512], in1=st[:, 512:1024]
                    )
                    nc.vector.tensor_max(
                        out=t[:, 512:768], in0=t[:, 0:256], in1=t[:, 256:512]
                    )
                    nc.vector.tensor_max(
                        out=pt[:, :], in0=t[:, 512:640], in1=t[:, 640:768]
                    )
                    parts.append(pt)
                m0 = part_pool.tile([P, D], mybir.dt.bfloat16, tag="pt", name="m0")
                m1 = part_pool.tile([P, D], mybir.dt.bfloat16, tag="pt", name="m1")
                nc.vector.tensor_max(out=m0[:, :], in0=parts[0][:, :], in1=parts[1][:, :])
                nc.vector.tensor_max(out=m1[:, :], in0=parts[2][:, :], in1=parts[3][:, :])
                nc.vector.tensor_max(
                    out=res[:, c * D : (c + 1) * D], in0=m0[:, :], in1=m1[:, :]
                )
                if c % GPK_STORE_GROUP == GPK_STORE_GROUP - 1 or c == CHUNKS - 1:
                    c0 = (c // GPK_STORE_GROUP) * GPK_STORE_GROUP
                    nc.sync.dma_start(
                        out=out_view[:, c0 : c + 1, :], in_=res_view[:, c0 : c + 1, :]
                    )

    nc.compile()
    return nc


def _prep_in_maps_gpkt(s_feats, neighbor_indices):
    import ml_dtypes

    s = np.ascontiguousarray(np.asarray(s_feats), dtype=np.float32).astype(
        ml_dtypes.bfloat16
    )
    nb = np.asarray(neighbor_indices)
    ncalls = CHUNKS * GPK_CPC
    in_maps = []
    for core in range(N_CORES):
        sl = nb[core * NODES_PER_CORE : (core + 1) * NODES_PER_CORE].astype(np.int32)
        if PADDED > NODES_PER_CORE:
            pad = np.full((PADDED - NODES_PER_CORE, K), GPK_BASE, np.int32)
            sl = np.concatenate([sl, pad], axis=0)
        sl3 = sl.reshape(CHUNKS, P, K)
        # Each call's last unwrapped position is (k = h*KB+KB-1, p = 127).
        # Permute the neighbors of every (c, 127) node so those positions
        # hold indices >= BASE (max is order-invariant). Uniform-random
        # indices make < GPK_CPC non-negative neighbors impossible in
        # practice; assert instead of handling it.
        for c in range(CHUNKS):
            neigh = sl3[c, 127].copy()
            nonneg = neigh[neigh >= GPK_BASE]
            neg = neigh[neigh < GPK_BASE]
            assert len(nonneg) >= GPK_CPC, (c, len(nonneg))
            rest = np.concatenate([neg, nonneg[GPK_CPC:]])
            new = np.empty(K, np.int32)
            ends = [h * GPK_KB + GPK_KB - 1 for h in range(GPK_CPC)]
            new[ends] = nonneg[:GPK_CPC]
            new[[k for k in range(K) if k not in ends]] = rest
            sl3[c, 127] = new
        rem = (sl3 - GPK_BASE).astype(np.int16)  # [c, p, k] signed offsets
        # call (c, h) takes k in [h*KB, (h+1)*KB); position m = k_local*128+p
        vals = rem.transpose(0, 2, 1).reshape(CHUNKS * GPK_CPC, GPK_KB * P)
        lanes = vals.reshape(ncalls, GPK_CALL_SLOTS, 16).transpose(2, 0, 1)
        part_block = np.ascontiguousarray(lanes).reshape(16, ncalls * GPK_CALL_SLOTS)
        full = np.tile(part_block, (8, 1))
        in_maps.append({"table": s, "idx": full})
    return in_maps


# ---------------------------------------------------------------- gbf16 ---
GBF_BASE = 25000  # signed int16 offsets reach rows 0..50000 from here
GBF_KB = 16  # neighbor blocks per gather call (half of K)
GBF_CPC = K // GBF_KB  # 2 calls per chunk
# 2049 emitted descriptors per call: 16 k-blocks of 128 plus ONE dummy
# sentinel (offset 0, >= 0) so the Q7's trailing-negative trim can never
# drop real descriptors. Positions 2050.. of the last 16-lane group are -1
# (trimmed if the ucode rounds up). 2049 fits the per-queue descriptor ring
# (dynamic_dma_scratch_size/16 = 3072 descs) so calls pipeline.
GBF_CALL_IDXS = GBF_KB * P + 1  # 2049
GBF_CALL_SLOTS = (GBF_CALL_IDXS + 15) // 16  # 129 int16 slots per partition
GBF_STORE_GROUP = 8


def _build_nc_gbf16():
    import concourse.bacc as bacc
    import concourse.mybir as mybir
    import concourse.tile as tile

    # A 2049-index gather emits ~129 descriptors per SWDGE ring lane (64 B
    # each); 49152 B of scratch gives each queue a 3072-descriptor ring.
    nc = bacc.Bacc(
        "TRN2", target_bir_lowering=False, debug=False,
        dynamic_dma_scratch_size=49152, num_swdge_queues=4,
    )
    table = nc.dram_tensor(
        "table", [N_NODES, D], mybir.dt.bfloat16, kind="ExternalInput"
    ).ap()
    ncalls = CHUNKS * GBF_CPC
    idx = nc.dram_tensor(
        "idx", [P, ncalls * GBF_CALL_SLOTS], mybir.dt.int16, kind="ExternalInput"
    ).ap()
    out = nc.dram_tensor(
        "out", [PADDED, D], mybir.dt.bfloat16, kind="ExternalOutput"
    ).ap()

    blocks = GBF_KB + 1  # 17 gathered blocks per call (last holds the sentinel)

    with tile.TileContext(nc) as tc:
        with (
            tc.tile_pool(name="pool", bufs=1) as pool,
            tc.tile_pool(name="stage", bufs=10) as stage_pool,
            tc.tile_pool(name="tmp", bufs=8) as tmp_pool,
            tc.tile_pool(name="parts", bufs=8) as part_pool,
        ):
            idx_sb = pool.tile(
                [P, ncalls * GBF_CALL_SLOTS], mybir.dt.int16, name="idx_sb"
            )
            # split the idx load so the first gathers don't wait for the
            # whole index transfer
            head_cols = 8 * GBF_CALL_SLOTS
            nc.sync.dma_start(out=idx_sb[:, :head_cols], in_=idx[:, :head_cols])
            nc.sync.dma_start(out=idx_sb[:, head_cols:], in_=idx[:, head_cols:])

            res = pool.tile([P, CHUNKS * D], mybir.dt.bfloat16, name="res")
            out_view = out.rearrange("(c p) d -> p c d", p=P)
            res_view = res[:, :].rearrange("p (c d) -> p c d", d=D)

            for c in range(CHUNKS):
                parts = []
                for h in range(GBF_CPC):
                    j = c * GBF_CPC + h
                    st = stage_pool.tile(
                        [P, blocks * D], mybir.dt.bfloat16, tag="stage", name="st"
                    )
                    nc.gpsimd.dma_gather(
                        out_ap=st[:, :].rearrange("p (b d) -> p b d", d=D),
                        in_ap=table[GBF_BASE:, :],
                        idxs_ap=idx_sb[
                            :, j * GBF_CALL_SLOTS : (j + 1) * GBF_CALL_SLOTS
                        ],
                        num_idxs=GBF_CALL_IDXS,
                        num_idxs_reg=GBF_CALL_IDXS,
                        elem_size=D,
                        single_packet=False,
                        queue_num=j % 4,
                    )
                    # binary max tree over the 16 real blocks (contiguous
                    # bf16 slices keep the DVE in 2x_1p mode; a strided
                    # tensor_reduce has no fast mode)
                    t = tmp_pool.tile(
                        [P, 1792], mybir.dt.bfloat16, tag="tmp", name="t"
                    )
                    pt = part_pool.tile([P, D], mybir.dt.bfloat16, tag="pt", name="pt")
                    nc.vector.tensor_max(
                        out=t[:, 0:1024], in0=st[:, 0:1024], in1=st[:, 1024:2048]
                    )
                    nc.vector.tensor_max(
                        out=t[:, 1024:1536], in0=t[:, 0:512], in1=t[:, 512:1024]
                    )
                    nc.vector.tensor_max(
                        out=t[:, 1536:1792], in0=t[:, 1024:1280], in1=t[:, 1280:1536]
                    )
                    nc.vector.tensor_max(
                        out=pt[:, :], in0=t[:, 1536:1664], in1=t[:, 1664:1792]
                    )
                    parts.append(pt)
                nc.vector.tensor_max(
                    out=res[:, c * D : (c + 1) * D],
                    in0=parts[0][:, :],
                    in1=parts[1][:, :],
                )
                # store finished chunk groups while later gathers still run
                if c % GBF_STORE_GROUP == GBF_STORE_GROUP - 1 or c == CHUNKS - 1:
                    c0 = (c // GBF_STORE_GROUP) * GBF_STORE_GROUP
                    nc.sync.dma_start(
                        out=out_view[:, c0 : c + 1, :], in_=res_view[:, c0 : c + 1, :]
                    )

    nc.compile()
    return nc


def _prep_in_maps_gbf16(s_feats, neighbor_indices):
    import ml_dtypes

    s = np.ascontiguousarray(np.asarray(s_feats), dtype=np.float32).astype(
        ml_dtypes.bfloat16
    )
    nb = np.asarray(neighbor_indices)
    ncalls = CHUNKS * GBF_CPC
    in_maps = []
    for core in range(N_CORES):
        sl = nb[core * NODES_PER_CORE : (core + 1) * NODES_PER_CORE].astype(np.int32)
        if PADDED > NODES_PER_CORE:
            # pad nodes gather row GBF_BASE (offset 0); results discarded
            pad = np.full((PADDED - NODES_PER_CORE, K), GBF_BASE, np.int32)
            sl = np.concatenate([sl, pad], axis=0)
        rem = (sl - GBF_BASE).astype(np.int16)  # signed offsets from row BASE
        rem3 = rem.reshape(CHUNKS, P, K)  # node (c, p), neighbor k
        # per call: GBF_KB k-blocks, position m = k*128 + p, then one zero
        # sentinel (>= 0 stops the trailing-negative trim) and -1 fill for
        # the rest of the final 16-lane group
        vals = rem3.transpose(0, 2, 1).reshape(ncalls, GBF_KB * P)
        tail = np.full((ncalls, GBF_CALL_SLOTS * 16 - GBF_KB * P), -1, np.int16)
        tail[:, 0] = 0  # the sentinel
        vals = np.concatenate([vals, tail], axis=1)  # [call, SLOTS*16]
        # wrap: position m -> (lane m%16, slot m//16), replicated to 8 groups
        lanes = vals.reshape(ncalls, GBF_CALL_SLOTS, 16).transpose(2, 0, 1)
        part_block = np.ascontiguousarray(lanes).reshape(16, ncalls * GBF_CALL_SLOTS)
        full = np.tile(part_block, (8, 1))
        in_maps.append({"table": s, "idx": full})
    return in_maps


# --------------------------------------------------------- f32 "gather" ---
BASE = 32768  # table base row: signed int16 idx reaches rows 0..50001
CALL_KB = 16  # neighbor blocks per gather call
CALLS_PER_CHUNK = K // CALL_KB  # 2
CALL_IDXS = CALL_KB * P + P  # 2176: 16 k-blocks of 128 + one dummy tail block
CALL_SLOTS = CALL_IDXS // 16  # 136 int16 slots per partition per call


def _build_nc_gather():
    """One InstDMAGatherAnt per 128-node chunk half: gathers 16 neighbor rows
    (512 B descriptors) from HBM with signed int16 indices relative to table
    row BASE, then a VectorE strided tensor_reduce(max) over K."""
    import concourse.bacc as bacc
    import concourse.mybir as mybir
    import concourse.tile as tile

    nc = bacc.Bacc(
        "TRN2", target_bir_lowering=False, debug=False,
        dynamic_dma_scratch_size=49152, num_swdge_queues=4,
    )
    table = nc.dram_tensor(
        "table", [N_NODES, D], mybir.dt.float32, kind="ExternalInput"
    ).ap()
    idx = nc.dram_tensor(
        "idx", [P, CHUNKS * CALLS_PER_CHUNK * CALL_SLOTS], mybir.dt.int16,
        kind="ExternalInput"
    ).ap()
    out = nc.dram_tensor(
        "out", [PADDED, D], mybir.dt.float32, kind="ExternalOutput"
    ).ap()

    blocks = CALL_IDXS // P  # 17 output blocks per call (last one is dummy)
    ncalls = CHUNKS * CALLS_PER_CHUNK

    with tile.TileContext(nc) as tc:
        with (
            tc.tile_pool(name="pool", bufs=1) as pool,
            tc.tile_pool(name="stage", bufs=8) as stage_pool,
            tc.tile_pool(name="parts", bufs=8) as part_pool,
        ):
            idx_sb = pool.tile([P, ncalls * CALL_SLOTS], mybir.dt.int16, name="idx_sb")
            head_cols = 8 * CALL_SLOTS
            nc.sync.dma_start(out=idx_sb[:, :head_cols], in_=idx[:, :head_cols])
            nc.sync.dma_start(out=idx_sb[:, head_cols:], in_=idx[:, head_cols:])

            res = pool.tile([P, CHUNKS * D], mybir.dt.float32, name="res")
            out_view = out.rearrange("(c p) d -> p c d", p=P)
            res_view = res[:, :].rearrange("p (c d) -> p c d", d=D)
            STORE_GROUP = 8

            for c in range(CHUNKS):
                parts = []
                for h in range(CALLS_PER_CHUNK):
                    j = c * CALLS_PER_CHUNK + h
                    st = stage_pool.tile(
                        [P, blocks * D], mybir.dt.float32, tag="stage", name="st"
                    )
                    nc.gpsimd.dma_gather(
                        out_ap=st[:, :].rearrange("p (b d) -> p b d", d=D),
                        in_ap=table[BASE:, :],
                        idxs_ap=idx_sb[:, j * CALL_SLOTS : (j + 1) * CALL_SLOTS],
                        num_idxs=CALL_IDXS,
                        num_idxs_reg=CALL_IDXS,
                        elem_size=D,
                        single_packet=False,
                        queue_num=j % 4,
                    )
                    view = st[:, : CALL_KB * D].rearrange("p (k d) -> p d k", k=CALL_KB)
                    pt = part_pool.tile([P, D], mybir.dt.float32, tag="pt", name="pt")
                    nc.vector.tensor_reduce(
                        out=pt[:, :],
                        in_=view,
                        axis=mybir.AxisListType.X,
                        op=mybir.AluOpType.max,
                    )
                    parts.append(pt)
                nc.vector.tensor_max(
                    out=res[:, c * D : (c + 1) * D],
                    in0=parts[0][:, :],
                    in1=parts[1][:, :],
                )
                if c % STORE_GROUP == STORE_GROUP - 1 or c == CHUNKS - 1:
                    c0 = (c // STORE_GROUP) * STORE_GROUP
                    nc.sync.dma_start(
                        out=out_view[:, c0 : c + 1, :], in_=res_view[:, c0 : c + 1, :]
                    )

    nc.compile()
    return nc


def _prep_in_maps_gather(s_feats, neighbor_indices):
    s = np.ascontiguousarray(np.asarray(s_feats), dtype=np.float32)
    nb = np.asarray(neighbor_indices)
    in_maps = []
    for core in range(N_CORES):
        sl = nb[core * NODES_PER_CORE : (core + 1) * NODES_PER_CORE].astype(np.int32)
        if PADDED > NODES_PER_CORE:
            pad = np.full((PADDED - NODES_PER_CORE, K), BASE, np.int32)
            sl = np.concatenate([sl, pad], axis=0)
        rem = (sl - BASE).astype(np.int16)
        rem3 = rem.reshape(CHUNKS, P, K)
        vals = rem3.transpose(0, 2, 1).reshape(CHUNKS, CALLS_PER_CHUNK, CALL_KB * P)
        dummy = np.zeros((CHUNKS, CALLS_PER_CHUNK, P), np.int16)
        vals = np.concatenate([vals, dummy], axis=2)
        ncalls = CHUNKS * CALLS_PER_CHUNK
        lanes = vals.reshape(ncalls, CALL_SLOTS, 16).transpose(2, 0, 1)
        part_block = np.ascontiguousarray(lanes).reshape(16, ncalls * CALL_SLOTS)
        full = np.tile(part_block, (8, 1))
        in_maps.append({"table": s, "idx": full})
    return in_maps


# ------------------------------------------------------------------ api ---
def _get_nc(variant=None):
    variant = variant or VARIANT
    if variant not in _nc_cache:
        if variant == "gpkt":
            _nc_cache[variant] = _build_nc_gpkt()
        elif variant == "gbf16":
            _nc_cache[variant] = _build_nc_gbf16()
        elif variant == "gather":
            _nc_cache[variant] = _build_nc_gather()
        else:
            raise ValueError(variant)
    return _nc_cache[variant]


def _prep_in_maps(variant, s_feats, neighbor_indices):
    if variant == "gpkt":
        return _prep_in_maps_gpkt(s_feats, neighbor_indices)
    if variant == "gbf16":
        return _prep_in_maps_gbf16(s_feats, neighbor_indices)
    return _prep_in_maps_gather(s_feats, neighbor_indices)


def run_variant(np_inputs, **run_kwargs):
    """Run the selected variant; returns (full f32 output, BassKernelResults)."""
    from concourse.bass_utils import run_bass_kernel_spmd

    if VARIANT == "gpair":
        in_maps, P_sched, orders = _prep_gpair(**np_inputs)
        key = ("gpair", P_sched)
        if key not in _nc_cache:
            _nc_cache[key] = _build_nc_gpair(P_sched)
        res = run_bass_kernel_spmd(
            _nc_cache[key], in_maps, core_ids=list(range(N_CORES)), **run_kwargs
        )
        out = np.empty((N_NODES, D), np.float32)
        for core in range(N_CORES):
            r = np.asarray(res.results[core]["out"]).astype(np.float32)
            order = orders[core]
            valid = order >= 0
            out[core * NODES_PER_CORE + order[valid]] = r[valid]
        return out, res

    nc = _get_nc()
    in_maps = _prep_in_maps(VARIANT, **np_inputs)
    res = run_bass_kernel_spmd(
        nc, in_maps, core_ids=list(range(N_CORES)), **run_kwargs
    )
    out = np.concatenate(
        [res.results[c]["out"][:NODES_PER_CORE] for c in range(N_CORES)], axis=0
    )
    return out.astype(np.float32), res


def kernel(s_feats, neighbor_indices):
    out, _ = run_variant(
        {"s_feats": s_feats, "neighbor_indices": neighbor_indices}
    )
    return out


# revision 20
# speedup vs baseline: 1.2002x; 1.2002x over previous
"""GNN max-pool message passing kernel for 8 Trainium2 NeuronCores.

Problem: out[n] = max_k s_feats[neighbor_indices[n, k]]  (N=50000, K=32, D=128)

Strategy: data-parallel over destination nodes per the sharding hint;
s_feats is replicated into every core's HBM and each core handles 6250
destination nodes.

Variant "gbf16" (current): the f32 trace showed the 16 SDMA engines ~88%
busy moving 512 B/descriptor (~21 GB/s/engine) — the gather is DMA-engine
byte-throughput-bound, not Q7 descriptor-emission-bound. So the table is
converted to bf16 on the host (tolerance is 2e-2; bf16 rounding is ~4e-3):

  - One InstDMAGatherAnt per 128-node chunk gathers all K=32 neighbor rows
    (256 B descriptors) from HBM with signed int16 indices relative to
    table row BASE (unsigned-stride x signed-index Q7 address math covers
    rows BASE-32768..BASE+32767 => BASE=25000 spans the whole table).
  - Each call carries one dummy tail block of zero offsets so the Q7's
    trailing-negative trim can never drop real descriptors.
  - Calls round-robin over all 4 SWDGE queues; single_packet=False.
  - The K-reduction is a tensor_tensor(max) binary tree over contiguous
    bf16 slices (TensorReduce has NO DVE perf mode — a strided reduce runs
    1 elem/cycle and was 350 us of DVE busy in the f32 baseline; the
    tensor_max tree on packed 2-byte data runs in 2x_1p mode at 0.5
    cyc/elem: ~2.7 us/chunk).
  - Output stays bf16 on HW (exact — max of bf16 inputs) and is converted
    to f32 on the host.

Layout per core:
  - node n -> (chunk c = n // 128, partition p = n % 128); call position
    m = k*128 + p so gathered block k of partition p is neighbor k of node
    (c, p); the output store is a strided HWDGE DMA every STORE_GROUP
    chunks; the 6250 real rows are a contiguous prefix of the 6272-row
    padded output.
  - idx input [128, ncalls*264] int16: per call 4224 positions wrapped
    16-wide (position m -> lane m%16, slot m//16), replicated to all eight
    16-partition groups as InstDMAGatherAnt expects.

Variant "gather" is the older f32 version (measured 489 us on 8 cores).
"""

import numpy as np

N_NODES = 50000
K = 32
D = 128
N_CORES = 8
P = 128
NODES_PER_CORE = N_NODES // N_CORES  # 6250
SLOTS = (NODES_PER_CORE + P - 1) // P  # 49
PADDED = P * SLOTS  # 6272
CHUNKS = PADDED // P  # 49 chunks of 128 nodes

VARIANT = "gpair"  # "gpair" | "gpkt" | "gbf16" | "gather"

_nc_cache = {}


# ---------------------------------------------------------------- gpair ---
# The Q7 dma_gather ucode runs one instruction at a time across the whole
# GpSimd cluster and its descriptor-emission loop costs ~2.3 ns per index
# POSITION regardless of elem_size (up to 16 KB/descriptor) — so kernel
# time is ~(total index positions) x 2.3 ns. This variant cuts positions
# ~19%: the host builds a per-core table permutation pi (greedy max-weight
# path forest over neighbor co-occurrence pairs) so that many nodes have
# two neighbors at consecutive pi positions; one 512 B "pair" descriptor
# (row j of a [49999, 256] sliding-window pair table = pi-rows j, j+1)
# then serves both. Nodes are re-bucketed into chunks by their pair count
# p_n (descending) and each chunk c uses the shared schedule P_c =
# min(p_n in chunk, over all cores): a pair call of P_c blocks (elem 256)
# plus single calls totalling 32-2*P_c blocks (elem 128). No sentinel:
# the slot-127 node of each chunk is chosen/reordered so every call's
# last index is non-negative (trailing-negative trim never fires).
GPR_BASE = 25000  # signed int16 offsets for both tables
GPR_STORE_GROUP = 8


def _gpair_path_forest(sets, n_rows=N_NODES, seed=0):
    """Greedy max-weight path forest over co-occurrence pairs.
    Returns pi (permutation of rows) maximizing per-set adjacent pairs."""
    rng = np.random.default_rng(seed)
    i, j = np.triu_indices(K, 1)
    pairs = np.stack([sets[:, i], sets[:, j]], axis=2).reshape(-1, 2)
    pairs = np.sort(pairs, axis=1)
    pairs = pairs[pairs[:, 0] != pairs[:, 1]]
    pu, counts = np.unique(
        pairs[:, 0].astype(np.int64) * n_rows + pairs[:, 1], return_counts=True
    )
    u = (pu // n_rows).astype(np.int32)
    v = (pu % n_rows).astype(np.int32)
    order = np.lexsort((rng.random(len(u)), -counts))
    u, v = u[order], v[order]
    deg = np.zeros(n_rows, np.int8)
    parent = np.arange(n_rows, dtype=np.int32)

    def find(x):
        while parent[x] != x:
            parent[x] = parent[parent[x]]
            x = parent[x]
        return x

    adj = [[] for _ in range(n_rows)]
    for uu, vv in zip(u.tolist(), v.tolist()):
        if deg[uu] >= 2 or deg[vv] >= 2:
            continue
        ru, rv = find(uu), find(vv)
        if ru == rv:
            continue
        parent[ru] = rv
        deg[uu] += 1
        deg[vv] += 1
        adj[uu].append(vv)
        adj[vv].append(uu)
    visited = np.zeros(n_rows, bool)
    pi = []
    for s in range(n_rows):
        if visited[s] or len(adj[s]) == 2:
            continue
        cur, prev = s, -1
        while True:
            pi.append(cur)
            visited[cur] = True
            nxt = [x for x in adj[cur] if x != prev and not visited[x]]
            if not nxt:
                break
            prev, cur = cur, nxt[0]
    for s in range(n_rows):
        if not visited[s]:
            pi.append(s)
    pi = np.asarray(pi, np.int32)
    assert len(pi) == n_rows
    return pi


def _gpair_phase1(sets):
    """Per-core: pi, per-node pair cover. Returns dict with pos-sorted rows,
    chosen-pair flags and per-node pair counts."""
    pi = _gpair_path_forest(sets)
    pos = np.empty(N_NODES, np.int64)
    pos[pi] = np.arange(N_NODES)
    ps = np.sort(pos[sets], axis=1).astype(np.int32)  # [M, K] pi positions
    d1 = np.diff(ps, axis=1) == 1
    m = len(sets)
    pair_at = np.zeros((m, K - 1), bool)  # cover takes (col, col+1)
    prev = np.zeros(m, bool)
    for col in range(K - 1):
        can = d1[:, col] & ~prev
        pair_at[:, col] = can
        prev = can
    p_n = pair_at.sum(axis=1).astype(np.int32)
    return {"pi": pi, "ps": ps, "pair_at": pair_at, "p_n": p_n}


def _gpair_calls_for_chunk(pc):
    """Call list for a chunk: (is_pair, blocks) per call."""
    calls = []
    if pc > 0:
        calls.append((True, pc))
    s = K - 2 * pc
    while s > 0:
        b = min(s, 16)
        calls.append((False, b))
        s -= b
    return calls


GPR_CALL_BLOCKS = 16  # gather blocks per merged call


def _gpair_call_plan(P_sched):
    """Merged cross-chunk call plan, a pure function of the schedule.

    The Q7 cluster emits ~2 ns/position aggregate only when the four
    concurrently-running calls are comparable in size, so pair blocks and
    single blocks are each concatenated across chunks and split into
    uniform 16-block calls (segments of a call may span chunks; the
    reduce trees each segment separately).

    Returns (calls, ends): calls is a list of dicts with keys ispair,
    blocks, segs=[(chunk, block_offset_in_chunk, nblocks), ...]; issue
    order interleaves the two streams by first-covered chunk. ends maps
    (chunk, ispair) -> list of block offsets (within the chunk's run)
    that are the LAST block of some call (host must place a non-negative
    index at position (that block, partition 127))."""
    streams = {}
    for ispair in (True, False):
        blocks = []  # (chunk, offset_in_chunk)
        for c in range(CHUNKS):
            n = P_sched[c] if ispair else K - 2 * P_sched[c]
            blocks += [(c, o) for o in range(n)]
        calls = []
        for i in range(0, len(blocks), GPR_CALL_BLOCKS):
            chunkb = blocks[i : i + GPR_CALL_BLOCKS]
            segs = []
            for ch, off in chunkb:
                if segs and segs[-1][0] == ch:
                    segs[-1] = (ch, segs[-1][1], segs[-1][2] + 1)
                else:
                    segs.append((ch, off, 1))
            calls.append(
                {"ispair": ispair, "blocks": len(chunkb), "segs": segs}
            )
        streams[ispair] = calls
    ends = {}
    for ispair, calls in streams.items():
        for call in calls:
            ch, off, nb = call["segs"][-1]
            ends.setdefault((ch, ispair), []).append(off + nb - 1)
    merged = streams[True] + streams[False]
    merged.sort(key=lambda call: (call["segs"][0][0], not call["ispair"]))
    return merged, ends


def _place_at(arr, ends, base):
    """Permute 1-D arr so arr[e] >= base for each position in ends."""
    if not len(ends):
        return arr
    nonneg = arr[arr >= base]
    neg = arr[arr < base]
    assert len(nonneg) >= len(ends), (len(nonneg), ends)
    rest = np.concatenate([neg, nonneg[len(ends):]])
    new = np.empty(len(arr), arr.dtype)
    ends = np.asarray(sorted(ends))
    new[ends] = nonneg[: len(ends)]
    mask = np.ones(len(arr), bool)
    mask[ends] = False
    new[mask] = rest
    return new


def _gpair_phase2(core_data, P_sched):
    """Per-core: order nodes, build the merged-call idx array. Returns idx
    array [128, total_slots] int16 and node order (orig local id per
    padded slot)."""
    ps, pair_at, p_n = core_data["ps"], core_data["pair_at"], core_data["p_n"]
    m = len(ps)
    order = np.argsort(-p_n, kind="stable").astype(np.int32)
    order_pad = np.concatenate([order, np.full(PADDED - m, -1, np.int32)])
    plan, ends = _gpair_call_plan(P_sched)
    pairs_by_chunk = []
    singles_by_chunk = []
    for c in range(CHUNKS):
        pc = P_sched[c]
        nodes = order_pad[c * P : (c + 1) * P]
        pairs_l = np.zeros((P, pc), np.int32)
        singles_l = np.zeros((P, K - 2 * pc), np.int32)
        for sl in range(P):
            n = nodes[sl]
            if n < 0:
                pairs_l[sl] = GPR_BASE  # pad: harmless reads, discarded
                singles_l[sl] = GPR_BASE
                continue
            cols = np.nonzero(pair_at[n])[0]
            use = cols[:pc]
            pstarts = ps[n][use]
            covered = np.zeros(K, bool)
            covered[use] = True
            covered[use + 1] = True
            pairs_l[sl] = pstarts
            singles_l[sl] = ps[n][~covered]
        pe = ends.get((c, True), [])
        se = ends.get((c, False), [])

        def fix(sl):
            return (pairs_l[sl] >= GPR_BASE).sum() >= len(pe) and (
                singles_l[sl] >= GPR_BASE
            ).sum() >= len(se)

        if not fix(127):
            for sl in range(P):
                if fix(sl):
                    pairs_l[[127, sl]] = pairs_l[[sl, 127]]
                    singles_l[[127, sl]] = singles_l[[sl, 127]]
                    nodes = nodes.copy()
                    nodes[[127, sl]] = nodes[[sl, 127]]
                    order_pad[c * P : (c + 1) * P] = nodes
                    break
            else:
                raise AssertionError(f"chunk {c}: no slot-127 candidate")
        pairs_l[127] = _place_at(pairs_l[127], pe, GPR_BASE)
        singles_l[127] = _place_at(singles_l[127], se, GPR_BASE)
        pairs_by_chunk.append(pairs_l)
        singles_by_chunk.append(singles_l)
    all_vals = []
    for call in plan:
        src = pairs_by_chunk if call["ispair"] else singles_by_chunk
        for ch, off, nb in call["segs"]:
            # block b, position m = b_in_call*128 + p
            all_vals.append(
                (src[ch][:, off : off + nb].T - GPR_BASE)
                .astype(np.int16)
                .reshape(-1)
            )
    flat = np.concatenate(all_vals)  # multiple of 16
    lanes = flat.reshape(-1, 16).T  # [16, total_slots]
    full = np.tile(np.ascontiguousarray(lanes), (8, 1))
    return full, order_pad


def _prep_gpair(s_feats, neighbor_indices):
    import ml_dtypes

    s = np.ascontiguousarray(np.asarray(s_feats), dtype=np.float32).astype(
        ml_dtypes.bfloat16
    )
    nb = np.asarray(neighbor_indices)
    cores = []
    for core in range(N_CORES):
        sets = nb[core * NODES_PER_CORE : (core + 1) * NODES_PER_CORE].astype(
            np.int32
        )
        cores.append(_gpair_phase1(sets))
    # shared schedule: per-chunk min pair count across cores; chunks
    # containing pad nodes get 0
    sorted_pn = [np.sort(c["p_n"])[::-1] for c in cores]
    P_sched = []
    for c in range(CHUNKS):
        if (c + 1) * P > NODES_PER_CORE:
            P_sched.append(0)
        else:
            P_sched.append(
                min(int(sp[(c + 1) * P - 1]) for sp in sorted_pn)
            )
    P_sched = tuple(P_sched)
    in_maps = []
    orders = []
    for core in range(N_CORES):
        idx_full, order_pad = _gpair_phase2(cores[core], P_sched)
        table = s[cores[core]["pi"]]
        ptable = np.ascontiguousarray(
            np.concatenate([table[:-1], table[1:]], axis=1)
        )
        in_maps.append({"table": table, "ptable": ptable, "idx": idx_full})
        orders.append(order_pad)
    return in_maps, P_sched, orders


def _build_nc_gpair(P_sched):
    import concourse.bacc as bacc
    import concourse.mybir as mybir
    import concourse.tile as tile

    nc = bacc.Bacc(
        "TRN2", target_bir_lowering=False, debug=False,
        dynamic_dma_scratch_size=49152, num_swdge_queues=4,
    )
    table = nc.dram_tensor(
        "table", [N_NODES, D], mybir.dt.bfloat16, kind="ExternalInput"
    ).ap()
    ptable = nc.dram_tensor(
        "ptable", [N_NODES - 1, 2 * D], mybir.dt.bfloat16, kind="ExternalInput"
    ).ap()
    plan, _plan_ends = _gpair_call_plan(P_sched)
    total_slots = sum(call["blocks"] * P // 16 for call in plan)
    idx = nc.dram_tensor(
        "idx", [P, total_slots], mybir.dt.int16, kind="ExternalInput"
    ).ap()
    out = nc.dram_tensor(
        "out", [PADDED, D], mybir.dt.bfloat16, kind="ExternalOutput"
    ).ap()

    max_pair_blocks = 2 * max(P_sched)  # width-128 blocks in a pair call
    with tile.TileContext(nc) as tc:
        with (
            tc.tile_pool(name="pool", bufs=1) as pool,
            tc.tile_pool(name="pstage", bufs=6) as pstage_pool,
            tc.tile_pool(name="sstage", bufs=12) as sstage_pool,
            tc.tile_pool(name="tmp", bufs=10) as tmp_pool,
            tc.tile_pool(name="parts", bufs=24) as part_pool,
        ):
            idx_sb = pool.tile([P, total_slots], mybir.dt.int16, name="idx_sb")
            head_cols = min(total_slots, 1024)
            nc.sync.dma_start(out=idx_sb[:, :head_cols], in_=idx[:, :head_cols])
            if head_cols < total_slots:
                nc.sync.dma_start(
                    out=idx_sb[:, head_cols:], in_=idx[:, head_cols:]
                )

            res = pool.tile([P, CHUNKS * D], mybir.dt.bfloat16, name="res")
            out_view = out.rearrange("(c p) d -> p c d", p=P)
            res_view = res[:, :].rearrange("p (c d) -> p c d", d=D)

            TMP_ELEMS = GPR_CALL_BLOCKS * D

            def tree_reduce(st, start_elems, wblocks):
                """Max-reduce wblocks width-D blocks at st[:, start_elems:]
                to one [P, D] block. Returns (tile, offset)."""
                stragglers = []
                cur, cur_off, n = st, start_elems, wblocks
                while n > 1:
                    h = n // 2
                    if n % 2:
                        stragglers.append((cur, cur_off + (n - 1) * D))
                    dst = tmp_pool.tile(
                        [P, TMP_ELEMS], mybir.dt.bfloat16, tag="tmp", name="tr"
                    )
                    nc.vector.tensor_max(
                        out=dst[:, : h * D],
                        in0=cur[:, cur_off : cur_off + h * D],
                        in1=cur[:, cur_off + h * D : cur_off + 2 * h * D],
                    )
                    cur, cur_off, n = dst, 0, h
                for sg, off in stragglers:
                    dst = part_pool.tile(
                        [P, D], mybir.dt.bfloat16, tag="pt", name="sg"
                    )
                    nc.vector.tensor_max(
                        out=dst[:, :],
                        in0=cur[:, cur_off : cur_off + D],
                        in1=sg[:, off : off + D],
                    )
                    cur, cur_off = dst, 0
                return cur, cur_off

            # expected number of partials (segments) per chunk
            exp = [0] * CHUNKS
            for call in plan:
                for ch, _off, _nb in call["segs"]:
                    exp[ch] += 1
            chunk_partials = [[] for _ in range(CHUNKS)]
            done = [False] * CHUNKS
            stored_to = 0  # chunks [0, stored_to) already written out

            def finish_chunk(c):
                partials = chunk_partials[c]
                sink = res[:, c * D : (c + 1) * D]
                if len(partials) == 1:
                    (t0, o0) = partials[0]
                    nc.vector.tensor_max(
                        out=sink, in0=t0[:, o0 : o0 + D], in1=t0[:, o0 : o0 + D]
                    )
                else:
                    while len(partials) > 2:
                        (t0, o0), (t1, o1) = partials[0], partials[1]
                        pt = part_pool.tile(
                            [P, D], mybir.dt.bfloat16, tag="pt", name="cmb"
                        )
                        nc.vector.tensor_max(
                            out=pt[:, :],
                            in0=t0[:, o0 : o0 + D],
                            in1=t1[:, o1 : o1 + D],
                        )
                        partials = [(pt, 0)] + partials[2:]
                    (t0, o0), (t1, o1) = partials[0], partials[1]
                    nc.vector.tensor_max(
                        out=sink, in0=t0[:, o0 : o0 + D], in1=t1[:, o1 : o1 + D]
                    )
                chunk_partials[c] = []

            def flush_stores(force=False):
                nonlocal stored_to
                while stored_to < CHUNKS:
                    hi = min(stored_to + GPR_STORE_GROUP, CHUNKS)
                    if not all(done[stored_to:hi]) and not force:
                        return
                    if not all(done[stored_to:hi]):
                        return
                    nc.sync.dma_start(
                        out=out_view[:, stored_to:hi, :],
                        in_=res_view[:, stored_to:hi, :],
                    )
                    stored_to = hi

            rr = 0
            col = 0
            for call in plan:
                ispair = call["ispair"]
                b = call["blocks"]
                elem = 2 * D if ispair else D
                nidx = b * P
                slots = nidx // 16
                st = (pstage_pool if ispair else sstage_pool).tile(
                    [P, GPR_CALL_BLOCKS * elem],
                    mybir.dt.bfloat16,
                    tag="pst" if ispair else "sst",
                    name="st",
                )
                nc.gpsimd.dma_gather(
                    out_ap=st[:, : b * elem].rearrange("p (b d) -> p b d", d=elem),
                    in_ap=(ptable if ispair else table)[GPR_BASE:, :],
                    idxs_ap=idx_sb[:, col : col + slots],
                    num_idxs=nidx,
                    num_idxs_reg=nidx,
                    elem_size=elem,
                    single_packet=False,
                    queue_num=rr % 4,
                )
                rr += 1
                col += slots
                boff = 0
                for ch, _off, nb in call["segs"]:
                    w = 2 * nb if ispair else nb
                    chunk_partials[ch].append(
                        tree_reduce(st, boff * elem, w)
                    )
                    boff += nb
                    if len(chunk_partials[ch]) == exp[ch]:
                        finish_chunk(ch)
                        done[ch] = True
                flush_stores()
            flush_stores(force=True)

    nc.compile()
    return nc


# ----------------------------------------------------------------- gpkt ---
# Like gbf16 but with 1024-index calls and single_packet=True so the Q7
# emits aggregated 64-descriptor packets per ring lane. No dummy sentinel:
# the host permutes the neighbors of each partition-127 node so the last
# unwrapped position of every call holds a non-negative offset (the
# trailing-negative trim then never fires).
GPK_BASE = 25000
GPK_KB = 8  # neighbor blocks per call
GPK_CPC = K // GPK_KB  # 4 calls per chunk
GPK_CALL_IDXS = GPK_KB * P  # 1024 = 64 descriptors per ring lane
GPK_CALL_SLOTS = GPK_CALL_IDXS // 16  # 64
GPK_STORE_GROUP = 8


def _build_nc_gpkt():
    import concourse.bacc as bacc
    import concourse.mybir as mybir
    import concourse.tile as tile

    nc = bacc.Bacc(
        "TRN2", target_bir_lowering=False, debug=False,
        dynamic_dma_scratch_size=49152, num_swdge_queues=4,
    )
    table = nc.dram_tensor(
        "table", [N_NODES, D], mybir.dt.bfloat16, kind="ExternalInput"
    ).ap()
    ncalls = CHUNKS * GPK_CPC
    idx = nc.dram_tensor(
        "idx", [P, ncalls * GPK_CALL_SLOTS], mybir.dt.int16, kind="ExternalInput"
    ).ap()
    out = nc.dram_tensor(
        "out", [PADDED, D], mybir.dt.bfloat16, kind="ExternalOutput"
    ).ap()

    with tile.TileContext(nc) as tc:
        with (
            tc.tile_pool(name="pool", bufs=1) as pool,
            tc.tile_pool(name="stage", bufs=12) as stage_pool,
            tc.tile_pool(name="tmp", bufs=8) as tmp_pool,
            tc.tile_pool(name="parts", bufs=12) as part_pool,
        ):
            idx_sb = pool.tile(
                [P, ncalls * GPK_CALL_SLOTS], mybir.dt.int16, name="idx_sb"
            )
            head_cols = 16 * GPK_CALL_SLOTS
            nc.sync.dma_start(out=idx_sb[:, :head_cols], in_=idx[:, :head_cols])
            nc.sync.dma_start(out=idx_sb[:, head_cols:], in_=idx[:, head_cols:])

            res = pool.tile([P, CHUNKS * D], mybir.dt.bfloat16, name="res")
            out_view = out.rearrange("(c p) d -> p c d", p=P)
            res_view = res[:, :].rearrange("p (c d) -> p c d", d=D)

            for c in range(CHUNKS):
                parts = []
                for h in range(GPK_CPC):
                    j = c * GPK_CPC + h
                    st = stage_pool.tile(
                        [P, GPK_KB * D], mybir.dt.bfloat16, tag="stage", name="st"
                    )
                    nc.gpsimd.dma_gather(
                        out_ap=st[:, :].rearrange("p (b d) -> p b d", d=D),
                        in_ap=table[GPK_BASE:, :],
                        idxs_ap=idx_sb[
                            :, j * GPK_CALL_SLOTS : (j + 1) * GPK_CALL_SLOTS
                        ],
                        num_idxs=GPK_CALL_IDXS,
                        num_idxs_reg=GPK_CALL_IDXS,
                        elem_size=D,
                        single_packet=True,
                        queue_num=j % 4,
                    )
                    t = tmp_pool.tile([P, 768], mybir.dt.bfloat16, tag="tmp", name="t")
                    pt = part_pool.tile([P, D], mybir.dt.bfloat16, tag="pt", name="pt")
                    nc.vector.tensor_max(
                        out=t[:, 0:512], in0=st[:, 0:512], in1=st[:, 512:1024]
                    )
                    nc.vector.tensor_max(
                        out=t[:, 512:768], in0=t[:, 0:256], in1=t[:, 256:512]
                    )
                    nc.vector.tensor_max(
                        out=pt[:, :], in0=t[:, 512:640], in1=t[:, 640:768]
                    )
                    parts.append(pt)
                m0 = part_pool.tile([P, D], mybir.dt.bfloat16, tag="pt", name="m0")
                m1 = part_pool.tile([P, D], mybir.dt.bfloat16, tag="pt", name="m1")
                nc.vector.tensor_max(out=m0[:, :], in0=parts[0][:, :], in1=parts[1][:, :])
                nc.vector.tensor_max(out=m1[:, :], in0=parts[2][:, :], in1=parts[3][:, :])
                nc.vector.tensor_max(
                    out=res[:, c * D : (c + 1) * D], in0=m0[:, :], in1=m1[:, :]
                )
                if c % GPK_STORE_GROUP == GPK_STORE_GROUP - 1 or c == CHUNKS - 1:
                    c0 = (c // GPK_STORE_GROUP) * GPK_STORE_GROUP
                    nc.sync.dma_start(
                        out=out_view[:, c0 : c + 1, :], in_=res_view[:, c0 : c + 1, :]
                    )

    nc.compile()
    return nc


def _prep_in_maps_gpkt(s_feats, neighbor_indices):
    import ml_dtypes

    s = np.ascontiguousarray(np.asarray(s_feats), dtype=np.float32).astype(
        ml_dtypes.bfloat16
    )
    nb = np.asarray(neighbor_indices)
    ncalls = CHUNKS * GPK_CPC
    in_maps = []
    for core in range(N_CORES):
        sl = nb[core * NODES_PER_CORE : (core + 1) * NODES_PER_CORE].astype(np.int32)
        if PADDED > NODES_PER_CORE:
            pad = np.full((PADDED - NODES_PER_CORE, K), GPK_BASE, np.int32)
            sl = np.concatenate([sl, pad], axis=0)
        sl3 = sl.reshape(CHUNKS, P, K)
        # Each call's last unwrapped position is (k = h*KB+KB-1, p = 127).
        # Permute the neighbors of every (c, 127) node so those positions
        # hold indices >= BASE (max is order-invariant). Uniform-random
        # indices make < GPK_CPC non-negative neighbors impossible in
        # practice; assert instead of handling it.
        for c in range(CHUNKS):
            neigh = sl3[c, 127].copy()
            nonneg = neigh[neigh >= GPK_BASE]
            neg = neigh[neigh < GPK_BASE]
            assert len(nonneg) >= GPK_CPC, (c, len(nonneg))
            rest = np.concatenate([neg, nonneg[GPK_CPC:]])
            new = np.empty(K, np.int32)
            ends = [h * GPK_KB + GPK_KB - 1 for h in range(GPK_CPC)]
            new[ends] = nonneg[:GPK_CPC]
            new[[k for k in range(K) if k not in ends]] = rest
            sl3[c, 127] = new
        rem = (sl3 - GPK_BASE).astype(np.int16)  # [c, p, k] signed offsets
        # call (c, h) takes k in [h*KB, (h+1)*KB); position m = k_local*128+p
        vals = rem.transpose(0, 2, 1).reshape(CHUNKS * GPK_CPC, GPK_KB * P)
        lanes = vals.reshape(ncalls, GPK_CALL_SLOTS, 16).transpose(2, 0, 1)
        part_block = np.ascontiguousarray(lanes).reshape(16, ncalls * GPK_CALL_SLOTS)
        full = np.tile(part_block, (8, 1))
        in_maps.append({"table": s, "idx": full})
    return in_maps


# ---------------------------------------------------------------- gbf16 ---
GBF_BASE = 25000  # signed int16 offsets reach rows 0..50000 from here
GBF_KB = 16  # neighbor blocks per gather call (half of K)
GBF_CPC = K // GBF_KB  # 2 calls per chunk
# 2049 emitted descriptors per call: 16 k-blocks of 128 plus ONE dummy
# sentinel (offset 0, >= 0) so the Q7's trailing-negative trim can never
# drop real descriptors. Positions 2050.. of the last 16-lane group are -1
# (trimmed if the ucode rounds up). 2049 fits the per-queue descriptor ring
# (dynamic_dma_scratch_size/16 = 3072 descs) so calls pipeline.
GBF_CALL_IDXS = GBF_KB * P + 1  # 2049
GBF_CALL_SLOTS = (GBF_CALL_IDXS + 15) // 16  # 129 int16 slots per partition
GBF_STORE_GROUP = 8


def _build_nc_gbf16():
    import concourse.bacc as bacc
    import concourse.mybir as mybir
    import concourse.tile as tile

    # A 2049-index gather emits ~129 descriptors per SWDGE ring lane (64 B
    # each); 49152 B of scratch gives each queue a 3072-descriptor ring.
    nc = bacc.Bacc(
        "TRN2", target_bir_lowering=False, debug=False,
        dynamic_dma_scratch_size=49152, num_swdge_queues=4,
    )
    table = nc.dram_tensor(
        "table", [N_NODES, D], mybir.dt.bfloat16, kind="ExternalInput"
    ).ap()
    ncalls = CHUNKS * GBF_CPC
    idx = nc.dram_tensor(
        "idx", [P, ncalls * GBF_CALL_SLOTS], mybir.dt.int16, kind="ExternalInput"
    ).ap()
    out = nc.dram_tensor(
        "out", [PADDED, D], mybir.dt.bfloat16, kind="ExternalOutput"
    ).ap()

    blocks = GBF_KB + 1  # 17 gathered blocks per call (last holds the sentinel)

    with tile.TileContext(nc) as tc:
        with (
            tc.tile_pool(name="pool", bufs=1) as pool,
            tc.tile_pool(name="stage", bufs=10) as stage_pool,
            tc.tile_pool(name="tmp", bufs=8) as tmp_pool,
            tc.tile_pool(name="parts", bufs=8) as part_pool,
        ):
            idx_sb = pool.tile(
                [P, ncalls * GBF_CALL_SLOTS], mybir.dt.int16, name="idx_sb"
            )
            # split the idx load so the first gathers don't wait for the
            # whole index transfer
            head_cols = 8 * GBF_CALL_SLOTS
            nc.sync.dma_start(out=idx_sb[:, :head_cols], in_=idx[:, :head_cols])
            nc.sync.dma_start(out=idx_sb[:, head_cols:], in_=idx[:, head_cols:])

            res = pool.tile([P, CHUNKS * D], mybir.dt.bfloat16, name="res")
            out_view = out.rearrange("(c p) d -> p c d", p=P)
            res_view = res[:, :].rearrange("p (c d) -> p c d", d=D)

            for c in range(CHUNKS):
                parts = []
                for h in range(GBF_CPC):
                    j = c * GBF_CPC + h
                    st = stage_pool.tile(
                        [P, blocks * D], mybir.dt.bfloat16, tag="stage", name="st"
                    )
                    nc.gpsimd.dma_gather(
                        out_ap=st[:, :].rearrange("p (b d) -> p b d", d=D),
                        in_ap=table[GBF_BASE:, :],
                        idxs_ap=idx_sb[
                            :, j * GBF_CALL_SLOTS : (j + 1) * GBF_CALL_SLOTS
                        ],
                        num_idxs=GBF_CALL_IDXS,
                        num_idxs_reg=GBF_CALL_IDXS,
                        elem_size=D,
                        single_packet=False,
                        queue_num=j % 4,
                    )
                    # binary max tree over the 16 real blocks (contiguous
                    # bf16 slices keep the DVE in 2x_1p mode; a strided
                    # tensor_reduce has no fast mode)
                    t = tmp_pool.tile(
                        [P, 1792], mybir.dt.bfloat16, tag="tmp", name="t"
                    )
                    pt = part_pool.tile([P, D], mybir.dt.bfloat16, tag="pt", name="pt")
                    nc.vector.tensor_max(
                        out=t[:, 0:1024], in0=st[:, 0:1024], in1=st[:, 1024:2048]
                    )
                    nc.vector.tensor_max(
                        out=t[:, 1024:1536], in0=t[:, 0:512], in1=t[:, 512:1024]
                    )
                    nc.vector.tensor_max(
                        out=t[:, 1536:1792], in0=t[:, 1024:1280], in1=t[:, 1280:1536]
                    )
                    nc.vector.tensor_max(
                        out=pt[:, :], in0=t[:, 1536:1664], in1=t[:, 1664:1792]
                    )
                    parts.append(pt)
                nc.vector.tensor_max(
                    out=res[:, c * D : (c + 1) * D],
                    in0=parts[0][:, :],
                    in1=parts[1][:, :],
                )
                # store finished chunk groups while later gathers still run
                if c % GBF_STORE_GROUP == GBF_STORE_GROUP - 1 or c == CHUNKS - 1:
                    c0 = (c // GBF_STORE_GROUP) * GBF_STORE_GROUP
                    nc.sync.dma_start(
                        out=out_view[:, c0 : c + 1, :], in_=res_view[:, c0 : c + 1, :]
                    )

    nc.compile()
    return nc


def _prep_in_maps_gbf16(s_feats, neighbor_indices):
    import ml_dtypes

    s = np.ascontiguousarray(np.asarray(s_feats), dtype=np.float32).astype(
        ml_dtypes.bfloat16
    )
    nb = np.asarray(neighbor_indices)
    ncalls = CHUNKS * GBF_CPC
    in_maps = []
    for core in range(N_CORES):
        sl = nb[core * NODES_PER_CORE : (core + 1) * NODES_PER_CORE].astype(np.int32)
        if PADDED > NODES_PER_CORE:
            # pad nodes gather row GBF_BASE (offset 0); results discarded
            pad = np.full((PADDED - NODES_PER_CORE, K), GBF_BASE, np.int32)
            sl = np.concatenate([sl, pad], axis=0)
        rem = (sl - GBF_BASE).astype(np.int16)  # signed offsets from row BASE
        rem3 = rem.reshape(CHUNKS, P, K)  # node (c, p), neighbor k
        # per call: GBF_KB k-blocks, position m = k*128 + p, then one zero
        # sentinel (>= 0 stops the trailing-negative trim) and -1 fill for
        # the rest of the final 16-lane group
        vals = rem3.transpose(0, 2, 1).reshape(ncalls, GBF_KB * P)
        tail = np.full((ncalls, GBF_CALL_SLOTS * 16 - GBF_KB * P), -1, np.int16)
        tail[:, 0] = 0  # the sentinel
        vals = np.concatenate([vals, tail], axis=1)  # [call, SLOTS*16]
        # wrap: position m -> (lane m%16, slot m//16), replicated to 8 groups
        lanes = vals.reshape(ncalls, GBF_CALL_SLOTS, 16).transpose(2, 0, 1)
        part_block = np.ascontiguousarray(lanes).reshape(16, ncalls * GBF_CALL_SLOTS)
        full = np.tile(part_block, (8, 1))
        in_maps.append({"table": s, "idx": full})
    return in_maps


# --------------------------------------------------------- f32 "gather" ---
BASE = 32768  # table base row: signed int16 idx reaches rows 0..50001
CALL_KB = 16  # neighbor blocks per gather call
CALLS_PER_CHUNK = K // CALL_KB  # 2
CALL_IDXS = CALL_KB * P + P  # 2176: 16 k-blocks of 128 + one dummy tail block
CALL_SLOTS = CALL_IDXS // 16  # 136 int16 slots per partition per call


def _build_nc_gather():
    """One InstDMAGatherAnt per 128-node chunk half: gathers 16 neighbor rows
    (512 B descriptors) from HBM with signed int16 indices relative to table
    row BASE, then a VectorE strided tensor_reduce(max) over K."""
    import concourse.bacc as bacc
    import concourse.mybir as mybir
    import concourse.tile as tile

    nc = bacc.Bacc(
        "TRN2", target_bir_lowering=False, debug=False,
        dynamic_dma_scratch_size=49152, num_swdge_queues=4,
    )
    table = nc.dram_tensor(
        "table", [N_NODES, D], mybir.dt.float32, kind="ExternalInput"
    ).ap()
    idx = nc.dram_tensor(
        "idx", [P, CHUNKS * CALLS_PER_CHUNK * CALL_SLOTS], mybir.dt.int16,
        kind="ExternalInput"
    ).ap()
    out = nc.dram_tensor(
        "out", [PADDED, D], mybir.dt.float32, kind="ExternalOutput"
    ).ap()

    blocks = CALL_IDXS // P  # 17 output blocks per call (last one is dummy)
    ncalls = CHUNKS * CALLS_PER_CHUNK

    with tile.TileContext(nc) as tc:
        with (
            tc.tile_pool(name="pool", bufs=1) as pool,
            tc.tile_pool(name="stage", bufs=8) as stage_pool,
            tc.tile_pool(name="parts", bufs=8) as part_pool,
        ):
            idx_sb = pool.tile([P, ncalls * CALL_SLOTS], mybir.dt.int16, name="idx_sb")
            head_cols = 8 * CALL_SLOTS
            nc.sync.dma_start(out=idx_sb[:, :head_cols], in_=idx[:, :head_cols])
            nc.sync.dma_start(out=idx_sb[:, head_cols:], in_=idx[:, head_cols:])

            res = pool.tile([P, CHUNKS * D], mybir.dt.float32, name="res")
            out_view = out.rearrange("(c p) d -> p c d", p=P)
            res_view = res[:, :].rearrange("p (c d) -> p c d", d=D)
            STORE_GROUP = 8

            for c in range(CHUNKS):
                parts = []
                for h in range(CALLS_PER_CHUNK):
                    j = c * CALLS_PER_CHUNK + h
                    st = stage_pool.tile(
                        [P, blocks * D], mybir.dt.float32, tag="stage", name="st"
                    )
                    nc.gpsimd.dma_gather(
                        out_ap=st[:, :].rearrange("p (b d) -> p b d", d=D),
                        in_ap=table[BASE:, :],
                        idxs_ap=idx_sb[:, j * CALL_SLOTS : (j + 1) * CALL_SLOTS],
                        num_idxs=CALL_IDXS,
                        num_idxs_reg=CALL_IDXS,
                        elem_size=D,
                        single_packet=False,
                        queue_num=j % 4,
                    )
                    view = st[:, : CALL_KB * D].rearrange("p (k d) -> p d k", k=CALL_KB)
                    pt = part_pool.tile([P, D], mybir.dt.float32, tag="pt", name="pt")
                    nc.vector.tensor_reduce(
                        out=pt[:, :],
                        in_=view,
                        axis=mybir.AxisListType.X,
                        op=mybir.AluOpType.max,
                    )
                    parts.append(pt)
                nc.vector.tensor_max(
                    out=res[:, c * D : (c + 1) * D],
                    in0=parts[0][:, :],
                    in1=parts[1][:, :],
                )
                if c % STORE_GROUP == STORE_GROUP - 1 or c == CHUNKS - 1:
                    c0 = (c // STORE_GROUP) * STORE_GROUP
                    nc.sync.dma_start(
                        out=out_view[:, c0 : c + 1, :], in_=res_view[:, c0 : c + 1, :]
                    )

    nc.compile()
    return nc


def _prep_in_maps_gather(s_feats, neighbor_indices):
    s = np.ascontiguousarray(np.asarray(s_feats), dtype=np.float32)
    nb = np.asarray(neighbor_indices)
    in_maps = []
    for core in range(N_CORES):
        sl = nb[core * NODES_PER_CORE : (core + 1) * NODES_PER_CORE].astype(np.int32)
        if PADDED > NODES_PER_CORE:
            pad = np.full((PADDED - NODES_PER_CORE, K), BASE, np.int32)
            sl = np.concatenate([sl, pad], axis=0)
        rem = (sl - BASE).astype(np.int16)
        rem3 = rem.reshape(CHUNKS, P, K)
        vals = rem3.transpose(0, 2, 1).reshape(CHUNKS, CALLS_PER_CHUNK, CALL_KB * P)
        dummy = np.zeros((CHUNKS, CALLS_PER_CHUNK, P), np.int16)
        vals = np.concatenate([vals, dummy], axis=2)
        ncalls = CHUNKS * CALLS_PER_CHUNK
        lanes = vals.reshape(ncalls, CALL_SLOTS, 16).transpose(2, 0, 1)
        part_block = np.ascontiguousarray(lanes).reshape(16, ncalls * CALL_SLOTS)
        full = np.tile(part_block, (8, 1))
        in_maps.append({"table": s, "idx": full})
    return in_maps


# ------------------------------------------------------------------ api ---
def _get_nc(variant=None):
    variant = variant or VARIANT
    if variant not in _nc_cache:
        if variant == "gpkt":
            _nc_cache[variant] = _build_nc_gpkt()
        elif variant == "gbf16":
            _nc_cache[variant] = _build_nc_gbf16()
        elif variant == "gather":
            _nc_cache[variant] = _build_nc_gather()
        else:
            raise ValueError(variant)
    return _nc_cache[variant]


def _prep_in_maps(variant, s_feats, neighbor_indices):
    if variant == "gpkt":
        return _prep_in_maps_gpkt(s_feats, neighbor_indices)
    if variant == "gbf16":
        return _prep_in_maps_gbf16(s_feats, neighbor_indices)
    return _prep_in_maps_gather(s_feats, neighbor_indices)


def run_variant(np_inputs, **run_kwargs):
    """Run the selected variant; returns (full f32 output, BassKernelResults)."""
    from concourse.bass_utils import run_bass_kernel_spmd

    if VARIANT == "gpair":
        in_maps, P_sched, orders = _prep_gpair(**np_inputs)
        key = ("gpair", P_sched)
        if key not in _nc_cache:
            _nc_cache[key] = _build_nc_gpair(P_sched)
        res = run_bass_kernel_spmd(
            _nc_cache[key], in_maps, core_ids=list(range(N_CORES)), **run_kwargs
        )
        out = np.empty((N_NODES, D), np.float32)
        for core in range(N_CORES):
            r = np.asarray(res.results[core]["out"]).astype(np.float32)
            order = orders[core]
            valid = order >= 0
            out[core * NODES_PER_CORE + order[valid]] = r[valid]
        return out, res

    nc = _get_nc()
    in_maps = _prep_in_maps(VARIANT, **np_inputs)
    res = run_bass_kernel_spmd(
        nc, in_maps, core_ids=list(range(N_CORES)), **run_kwargs
    )
    out = np.concatenate(
        [res.results[c]["out"][:NODES_PER_CORE] for c in range(N_CORES)], axis=0
    )
    return out.astype(np.float32), res


def kernel(s_feats, neighbor_indices):
    out, _ = run_variant(
        {"s_feats": s_feats, "neighbor_indices": neighbor_indices}
    )
    return out


# revision 24
# speedup vs baseline: 1.2929x; 1.0772x over previous
"""GNN max-pool message passing kernel for 8 Trainium2 NeuronCores.

Problem: out[n] = max_k s_feats[neighbor_indices[n, k]]  (N=50000, K=32, D=128)

Strategy: data-parallel over destination nodes per the sharding hint;
s_feats is replicated into every core's HBM and each core handles 6250
destination nodes.

Variant "gbf16" (current): the f32 trace showed the 16 SDMA engines ~88%
busy moving 512 B/descriptor (~21 GB/s/engine) — the gather is DMA-engine
byte-throughput-bound, not Q7 descriptor-emission-bound. So the table is
converted to bf16 on the host (tolerance is 2e-2; bf16 rounding is ~4e-3):

  - One InstDMAGatherAnt per 128-node chunk gathers all K=32 neighbor rows
    (256 B descriptors) from HBM with signed int16 indices relative to
    table row BASE (unsigned-stride x signed-index Q7 address math covers
    rows BASE-32768..BASE+32767 => BASE=25000 spans the whole table).
  - Each call carries one dummy tail block of zero offsets so the Q7's
    trailing-negative trim can never drop real descriptors.
  - Calls round-robin over all 4 SWDGE queues; single_packet=False.
  - The K-reduction is a tensor_tensor(max) binary tree over contiguous
    bf16 slices (TensorReduce has NO DVE perf mode — a strided reduce runs
    1 elem/cycle and was 350 us of DVE busy in the f32 baseline; the
    tensor_max tree on packed 2-byte data runs in 2x_1p mode at 0.5
    cyc/elem: ~2.7 us/chunk).
  - Output stays bf16 on HW (exact — max of bf16 inputs) and is converted
    to f32 on the host.

Layout per core:
  - node n -> (chunk c = n // 128, partition p = n % 128); call position
    m = k*128 + p so gathered block k of partition p is neighbor k of node
    (c, p); the output store is a strided HWDGE DMA every STORE_GROUP
    chunks; the 6250 real rows are a contiguous prefix of the 6272-row
    padded output.
  - idx input [128, ncalls*264] int16: per call 4224 positions wrapped
    16-wide (position m -> lane m%16, slot m//16), replicated to all eight
    16-partition groups as InstDMAGatherAnt expects.

Variant "gather" is the older f32 version (measured 489 us on 8 cores).
"""

import numpy as np

N_NODES = 50000
K = 32
D = 128
N_CORES = 8
P = 128
NODES_PER_CORE = N_NODES // N_CORES  # 6250
SLOTS = (NODES_PER_CORE + P - 1) // P  # 49
PADDED = P * SLOTS  # 6272
CHUNKS = PADDED // P  # 49 chunks of 128 nodes

VARIANT = "gpair"  # "gpair" | "gpkt" | "gbf16" | "gather"

_nc_cache = {}


# ---------------------------------------------------------------- gpair ---
# The Q7 dma_gather ucode runs one instruction at a time across the whole
# GpSimd cluster and its descriptor-emission loop costs ~2.3 ns per index
# POSITION regardless of elem_size (up to 16 KB/descriptor) — so kernel
# time is ~(total index positions) x 2.3 ns. This variant cuts positions
# ~19%: the host builds a per-core table permutation pi (greedy max-weight
# path forest over neighbor co-occurrence pairs) so that many nodes have
# two neighbors at consecutive pi positions; one 512 B "pair" descriptor
# (row j of a [49999, 256] sliding-window pair table = pi-rows j, j+1)
# then serves both. Nodes are re-bucketed into chunks by their pair count
# p_n (descending) and each chunk c uses the shared schedule P_c =
# min(p_n in chunk, over all cores): a pair call of P_c blocks (elem 256)
# plus single calls totalling 32-2*P_c blocks (elem 128). No sentinel:
# the slot-127 node of each chunk is chosen/reordered so every call's
# last index is non-negative (trailing-negative trim never fires).
GPR_BASE = 25000  # signed int16 offsets for both tables
GPR_STORE_GROUP = 8


def _gpair_path_forest(sets, n_rows=N_NODES, seed=0):
    """Greedy max-weight path forest over co-occurrence pairs.
    Returns pi (permutation of rows) maximizing per-set adjacent pairs."""
    rng = np.random.default_rng(seed)
    i, j = np.triu_indices(K, 1)
    pairs = np.stack([sets[:, i], sets[:, j]], axis=2).reshape(-1, 2)
    pairs = np.sort(pairs, axis=1)
    pairs = pairs[pairs[:, 0] != pairs[:, 1]]
    pu, counts = np.unique(
        pairs[:, 0].astype(np.int64) * n_rows + pairs[:, 1], return_counts=True
    )
    u = (pu // n_rows).astype(np.int32)
    v = (pu % n_rows).astype(np.int32)
    order = np.lexsort((rng.random(len(u)), -counts))
    u, v = u[order], v[order]
    deg = np.zeros(n_rows, np.int8)
    parent = np.arange(n_rows, dtype=np.int32)

    def find(x):
        while parent[x] != x:
            parent[x] = parent[parent[x]]
            x = parent[x]
        return x

    adj = [[] for _ in range(n_rows)]
    for uu, vv in zip(u.tolist(), v.tolist()):
        if deg[uu] >= 2 or deg[vv] >= 2:
            continue
        ru, rv = find(uu), find(vv)
        if ru == rv:
            continue
        parent[ru] = rv
        deg[uu] += 1
        deg[vv] += 1
        adj[uu].append(vv)
        adj[vv].append(uu)
    visited = np.zeros(n_rows, bool)
    pi = []
    for s in range(n_rows):
        if visited[s] or len(adj[s]) == 2:
            continue
        cur, prev = s, -1
        while True:
            pi.append(cur)
            visited[cur] = True
            nxt = [x for x in adj[cur] if x != prev and not visited[x]]
            if not nxt:
                break
            prev, cur = cur, nxt[0]
    for s in range(n_rows):
        if not visited[s]:
            pi.append(s)
    pi = np.asarray(pi, np.int32)
    assert len(pi) == n_rows
    return pi


def _gpair_phase1(sets):
    """Per-core: pi, per-node pair cover. Returns dict with pos-sorted rows,
    chosen-pair flags and per-node pair counts."""
    pi = _gpair_path_forest(sets)
    pos = np.empty(N_NODES, np.int64)
    pos[pi] = np.arange(N_NODES)
    ps = np.sort(pos[sets], axis=1).astype(np.int32)  # [M, K] pi positions
    d1 = np.diff(ps, axis=1) == 1
    m = len(sets)
    pair_at = np.zeros((m, K - 1), bool)  # cover takes (col, col+1)
    prev = np.zeros(m, bool)
    for col in range(K - 1):
        can = d1[:, col] & ~prev
        pair_at[:, col] = can
        prev = can
    p_n = pair_at.sum(axis=1).astype(np.int32)
    return {"pi": pi, "ps": ps, "pair_at": pair_at, "p_n": p_n}


def _gpair_calls_for_chunk(pc):
    """Call list for a chunk: (is_pair, blocks) per call."""
    calls = []
    if pc > 0:
        calls.append((True, pc))
    s = K - 2 * pc
    while s > 0:
        b = min(s, 16)
        calls.append((False, b))
        s -= b
    return calls


GPR_CALL_BLOCKS = 16  # gather blocks per merged call


def _gpair_call_plan(P_sched):
    """Merged cross-chunk call plan, a pure function of the schedule.

    The Q7 cluster emits ~2 ns/position aggregate only when the four
    concurrently-running calls are comparable in size, so pair blocks and
    single blocks are each concatenated across chunks and split into
    uniform 16-block calls (segments of a call may span chunks; the
    reduce trees each segment separately).

    Returns (calls, ends): calls is a list of dicts with keys ispair,
    blocks, segs=[(chunk, block_offset_in_chunk, nblocks), ...]; issue
    order interleaves the two streams by first-covered chunk. ends maps
    (chunk, ispair) -> list of block offsets (within the chunk's run)
    that are the LAST block of some call (host must place a non-negative
    index at position (that block, partition 127))."""
    streams = {}
    for ispair in (True, False):
        blocks = []  # (chunk, offset_in_chunk)
        for c in range(CHUNKS):
            n = P_sched[c] if ispair else K - 2 * P_sched[c]
            blocks += [(c, o) for o in range(n)]
        calls = []
        for i in range(0, len(blocks), GPR_CALL_BLOCKS):
            chunkb = blocks[i : i + GPR_CALL_BLOCKS]
            segs = []
            for ch, off in chunkb:
                if segs and segs[-1][0] == ch:
                    segs[-1] = (ch, segs[-1][1], segs[-1][2] + 1)
                else:
                    segs.append((ch, off, 1))
            calls.append(
                {"ispair": ispair, "blocks": len(chunkb), "segs": segs}
            )
        streams[ispair] = calls
    ends = {}
    for ispair, calls in streams.items():
        for call in calls:
            ch, off, nb = call["segs"][-1]
            ends.setdefault((ch, ispair), []).append(off + nb - 1)
    # pairs first as a uniform phase, then singles: mixing 256 B and 512 B
    # calls across queues measurably degrades the Q7 emission rate, and
    # each phase alone runs at ~2.1 ns/position. Chunk pair-partials are
    # held in SBUF (49 x [P, D] bf16 = 12 KB/partition) until the single
    # phase completes each chunk.
    merged = streams[True] + streams[False]
    return merged, ends


def _place_at(arr, ends, base):
    """Permute 1-D arr so arr[e] >= base for each position in ends."""
    if not len(ends):
        return arr
    nonneg = arr[arr >= base]
    neg = arr[arr < base]
    assert len(nonneg) >= len(ends), (len(nonneg), ends)
    rest = np.concatenate([neg, nonneg[len(ends):]])
    new = np.empty(len(arr), arr.dtype)
    ends = np.asarray(sorted(ends))
    new[ends] = nonneg[: len(ends)]
    mask = np.ones(len(arr), bool)
    mask[ends] = False
    new[mask] = rest
    return new


def _gpair_phase2(core_data, P_sched):
    """Per-core: order nodes, build the merged-call idx array. Returns idx
    array [128, total_slots] int16 and node order (orig local id per
    padded slot)."""
    ps, pair_at, p_n = core_data["ps"], core_data["pair_at"], core_data["p_n"]
    m = len(ps)
    order = np.argsort(-p_n, kind="stable").astype(np.int32)
    order_pad = np.concatenate([order, np.full(PADDED - m, -1, np.int32)])
    plan, ends = _gpair_call_plan(P_sched)
    pairs_by_chunk = []
    singles_by_chunk = []
    for c in range(CHUNKS):
        pc = P_sched[c]
        nodes = order_pad[c * P : (c + 1) * P]
        pairs_l = np.zeros((P, pc), np.int32)
        singles_l = np.zeros((P, K - 2 * pc), np.int32)
        for sl in range(P):
            n = nodes[sl]
            if n < 0:
                pairs_l[sl] = GPR_BASE  # pad: harmless reads, discarded
                singles_l[sl] = GPR_BASE
                continue
            cols = np.nonzero(pair_at[n])[0]
            use = cols[:pc]
            pstarts = ps[n][use]
            covered = np.zeros(K, bool)
            covered[use] = True
            covered[use + 1] = True
            pairs_l[sl] = pstarts
            singles_l[sl] = ps[n][~covered]
        pe = ends.get((c, True), [])
        se = ends.get((c, False), [])

        def fix(sl):
            return (pairs_l[sl] >= GPR_BASE).sum() >= len(pe) and (
                singles_l[sl] >= GPR_BASE
            ).sum() >= len(se)

        if not fix(127):
            for sl in range(P):
                if fix(sl):
                    pairs_l[[127, sl]] = pairs_l[[sl, 127]]
                    singles_l[[127, sl]] = singles_l[[sl, 127]]
                    nodes = nodes.copy()
                    nodes[[127, sl]] = nodes[[sl, 127]]
                    order_pad[c * P : (c + 1) * P] = nodes
                    break
            else:
                raise AssertionError(f"chunk {c}: no slot-127 candidate")
        pairs_l[127] = _place_at(pairs_l[127], pe, GPR_BASE)
        singles_l[127] = _place_at(singles_l[127], se, GPR_BASE)
        pairs_by_chunk.append(pairs_l)
        singles_by_chunk.append(singles_l)
    all_vals = []
    for call in plan:
        src = pairs_by_chunk if call["ispair"] else singles_by_chunk
        for ch, off, nb in call["segs"]:
            # block b, position m = b_in_call*128 + p
            all_vals.append(
                (src[ch][:, off : off + nb].T - GPR_BASE)
                .astype(np.int16)
                .reshape(-1)
            )
    flat = np.concatenate(all_vals)  # multiple of 16
    lanes = flat.reshape(-1, 16).T  # [16, total_slots]
    full = np.tile(np.ascontiguousarray(lanes), (8, 1))
    return full, order_pad


def _prep_gpair(s_feats, neighbor_indices):
    import ml_dtypes

    s = np.ascontiguousarray(np.asarray(s_feats), dtype=np.float32).astype(
        ml_dtypes.bfloat16
    )
    nb = np.asarray(neighbor_indices)
    cores = []
    for core in range(N_CORES):
        sets = nb[core * NODES_PER_CORE : (core + 1) * NODES_PER_CORE].astype(
            np.int32
        )
        cores.append(_gpair_phase1(sets))
    # shared schedule: per-chunk min pair count across cores; chunks
    # containing pad nodes get 0
    sorted_pn = [np.sort(c["p_n"])[::-1] for c in cores]
    P_sched = []
    for c in range(CHUNKS):
        if (c + 1) * P > NODES_PER_CORE:
            P_sched.append(0)
        else:
            P_sched.append(
                min(int(sp[(c + 1) * P - 1]) for sp in sorted_pn)
            )
    P_sched = tuple(P_sched)
    in_maps = []
    orders = []
    for core in range(N_CORES):
        idx_full, order_pad = _gpair_phase2(cores[core], P_sched)
        table = s[cores[core]["pi"]]
        ptable = np.ascontiguousarray(
            np.concatenate([table[:-1], table[1:]], axis=1)
        )
        in_maps.append({"table": table, "ptable": ptable, "idx": idx_full})
        orders.append(order_pad)
    return in_maps, P_sched, orders


def _build_nc_gpair(P_sched):
    import concourse.bacc as bacc
    import concourse.mybir as mybir
    import concourse.tile as tile

    nc = bacc.Bacc(
        "TRN2", target_bir_lowering=False, debug=False,
        dynamic_dma_scratch_size=49152, num_swdge_queues=4,
    )
    table = nc.dram_tensor(
        "table", [N_NODES, D], mybir.dt.bfloat16, kind="ExternalInput"
    ).ap()
    ptable = nc.dram_tensor(
        "ptable", [N_NODES - 1, 2 * D], mybir.dt.bfloat16, kind="ExternalInput"
    ).ap()
    plan, _plan_ends = _gpair_call_plan(P_sched)
    total_slots = sum(call["blocks"] * P // 16 for call in plan)
    idx = nc.dram_tensor(
        "idx", [P, total_slots], mybir.dt.int16, kind="ExternalInput"
    ).ap()
    out = nc.dram_tensor(
        "out", [PADDED, D], mybir.dt.bfloat16, kind="ExternalOutput"
    ).ap()

    max_pair_blocks = 2 * max(P_sched)  # width-128 blocks in a pair call
    with tile.TileContext(nc) as tc:
        with (
            tc.tile_pool(name="pool", bufs=1) as pool,
            tc.tile_pool(name="pstage", bufs=4) as pstage_pool,
            tc.tile_pool(name="sstage", bufs=10) as sstage_pool,
            tc.tile_pool(name="tmp", bufs=6) as tmp_pool,
            tc.tile_pool(name="parts", bufs=64) as part_pool,
        ):
            idx_sb = pool.tile([P, total_slots], mybir.dt.int16, name="idx_sb")
            head_cols = min(total_slots, 1024)
            nc.sync.dma_start(out=idx_sb[:, :head_cols], in_=idx[:, :head_cols])
            if head_cols < total_slots:
                nc.sync.dma_start(
                    out=idx_sb[:, head_cols:], in_=idx[:, head_cols:]
                )

            res = pool.tile([P, CHUNKS * D], mybir.dt.bfloat16, name="res")
            out_view = out.rearrange("(c p) d -> p c d", p=P)
            res_view = res[:, :].rearrange("p (c d) -> p c d", d=D)

            TMP_ELEMS = GPR_CALL_BLOCKS * D

            def tree_reduce(st, start_elems, wblocks):
                """Max-reduce wblocks width-D blocks at st[:, start_elems:]
                to one [P, D] block. Returns (tile, offset)."""
                stragglers = []
                cur, cur_off, n = st, start_elems, wblocks
                while n > 1:
                    h = n // 2
                    if n % 2:
                        stragglers.append((cur, cur_off + (n - 1) * D))
                    # the final 128-wide result may be held until its chunk
                    # completes: put it in the deep parts pool
                    if h == 1:
                        dst = part_pool.tile(
                            [P, D], mybir.dt.bfloat16, tag="pt", name="tr1"
                        )
                    else:
                        dst = tmp_pool.tile(
                            [P, TMP_ELEMS], mybir.dt.bfloat16, tag="tmp",
                            name="tr",
                        )
                    nc.vector.tensor_max(
                        out=dst[:, : h * D],
                        in0=cur[:, cur_off : cur_off + h * D],
                        in1=cur[:, cur_off + h * D : cur_off + 2 * h * D],
                    )
                    cur, cur_off, n = dst, 0, h
                for sg, off in stragglers:
                    dst = part_pool.tile(
                        [P, D], mybir.dt.bfloat16, tag="pt", name="sg"
                    )
                    nc.vector.tensor_max(
                        out=dst[:, :],
                        in0=cur[:, cur_off : cur_off + D],
                        in1=sg[:, off : off + D],
                    )
                    cur, cur_off = dst, 0
                return cur, cur_off

            # expected number of partials (segments) per chunk
            exp = [0] * CHUNKS
            for call in plan:
                for ch, _off, _nb in call["segs"]:
                    exp[ch] += 1
            chunk_partials = [[] for _ in range(CHUNKS)]
            done = [False] * CHUNKS
            stored_to = 0  # chunks [0, stored_to) already written out

            def finish_chunk(c):
                partials = chunk_partials[c]
                sink = res[:, c * D : (c + 1) * D]
                if len(partials) == 1:
                    (t0, o0) = partials[0]
                    nc.vector.tensor_max(
                        out=sink, in0=t0[:, o0 : o0 + D], in1=t0[:, o0 : o0 + D]
                    )
                else:
                    while len(partials) > 2:
                        (t0, o0), (t1, o1) = partials[0], partials[1]
                        pt = part_pool.tile(
                            [P, D], mybir.dt.bfloat16, tag="pt", name="cmb"
                        )
                        nc.vector.tensor_max(
                            out=pt[:, :],
                            in0=t0[:, o0 : o0 + D],
                            in1=t1[:, o1 : o1 + D],
                        )
                        partials = [(pt, 0)] + partials[2:]
                    (t0, o0), (t1, o1) = partials[0], partials[1]
                    nc.vector.tensor_max(
                        out=sink, in0=t0[:, o0 : o0 + D], in1=t1[:, o1 : o1 + D]
                    )
                chunk_partials[c] = []

            def flush_stores(force=False):
                nonlocal stored_to
                while stored_to < CHUNKS:
                    hi = min(stored_to + GPR_STORE_GROUP, CHUNKS)
                    if not all(done[stored_to:hi]) and not force:
                        return
                    if not all(done[stored_to:hi]):
                        return
                    nc.sync.dma_start(
                        out=out_view[:, stored_to:hi, :],
                        in_=res_view[:, stored_to:hi, :],
                    )
                    stored_to = hi

            rr = 0
            col = 0
            for call in plan:
                ispair = call["ispair"]
                b = call["blocks"]
                elem = 2 * D if ispair else D
                nidx = b * P
                slots = nidx // 16
                st = (pstage_pool if ispair else sstage_pool).tile(
                    [P, GPR_CALL_BLOCKS * elem],
                    mybir.dt.bfloat16,
                    tag="pst" if ispair else "sst",
                    name="st",
                )
                nc.gpsimd.dma_gather(
                    out_ap=st[:, : b * elem].rearrange("p (b d) -> p b d", d=elem),
                    in_ap=(ptable if ispair else table)[GPR_BASE:, :],
                    idxs_ap=idx_sb[:, col : col + slots],
                    num_idxs=nidx,
                    num_idxs_reg=nidx,
                    elem_size=elem,
                    single_packet=False,
                    queue_num=rr % 4,
                )
                rr += 1
                col += slots
                boff = 0
                for ch, _off, nb in call["segs"]:
                    w = 2 * nb if ispair else nb
                    chunk_partials[ch].append(
                        tree_reduce(st, boff * elem, w)
                    )
                    boff += nb
                    if len(chunk_partials[ch]) == exp[ch]:
                        finish_chunk(ch)
                        done[ch] = True
                flush_stores()
            flush_stores(force=True)

    nc.compile()
    return nc


# ----------------------------------------------------------------- gpkt ---
# Like gbf16 but with 1024-index calls and single_packet=True so the Q7
# emits aggregated 64-descriptor packets per ring lane. No dummy sentinel:
# the host permutes the neighbors of each partition-127 node so the last
# unwrapped position of every call holds a non-negative offset (the
# trailing-negative trim then never fires).
GPK_BASE = 25000
GPK_KB = 8  # neighbor blocks per call
GPK_CPC = K // GPK_KB  # 4 calls per chunk
GPK_CALL_IDXS = GPK_KB * P  # 1024 = 64 descriptors per ring lane
GPK_CALL_SLOTS = GPK_CALL_IDXS // 16  # 64
GPK_STORE_GROUP = 8


def _build_nc_gpkt():
    import concourse.bacc as bacc
    import concourse.mybir as mybir
    import concourse.tile as tile

    nc = bacc.Bacc(
        "TRN2", target_bir_lowering=False, debug=False,
        dynamic_dma_scratch_size=49152, num_swdge_queues=4,
    )
    table = nc.dram_tensor(
        "table", [N_NODES, D], mybir.dt.bfloat16, kind="ExternalInput"
    ).ap()
    ncalls = CHUNKS * GPK_CPC
    idx = nc.dram_tensor(
        "idx", [P, ncalls * GPK_CALL_SLOTS], mybir.dt.int16, kind="ExternalInput"
    ).ap()
    out = nc.dram_tensor(
        "out", [PADDED, D], mybir.dt.bfloat16, kind="ExternalOutput"
    ).ap()

    with tile.TileContext(nc) as tc:
        with (
            tc.tile_pool(name="pool", bufs=1) as pool,
            tc.tile_pool(name="stage", bufs=12) as stage_pool,
            tc.tile_pool(name="tmp", bufs=8) as tmp_pool,
            tc.tile_pool(name="parts", bufs=12) as part_pool,
        ):
            idx_sb = pool.tile(
                [P, ncalls * GPK_CALL_SLOTS], mybir.dt.int16, name="idx_sb"
            )
            head_cols = 16 * GPK_CALL_SLOTS
            nc.sync.dma_start(out=idx_sb[:, :head_cols], in_=idx[:, :head_cols])
            nc.sync.dma_start(out=idx_sb[:, head_cols:], in_=idx[:, head_cols:])

            res = pool.tile([P, CHUNKS * D], mybir.dt.bfloat16, name="res")
            out_view = out.rearrange("(c p) d -> p c d", p=P)
            res_view = res[:, :].rearrange("p (c d) -> p c d", d=D)

            for c in range(CHUNKS):
                parts = []
                for h in range(GPK_CPC):
                    j = c * GPK_CPC + h
                    st = stage_pool.tile(
                        [P, GPK_KB * D], mybir.dt.bfloat16, tag="stage", name="st"
                    )
                    nc.gpsimd.dma_gather(
                        out_ap=st[:, :].rearrange("p (b d) -> p b d", d=D),
                        in_ap=table[GPK_BASE:, :],
                        idxs_ap=idx_sb[
                            :, j * GPK_CALL_SLOTS : (j + 1) * GPK_CALL_SLOTS
                        ],
                        num_idxs=GPK_CALL_IDXS,
                        num_idxs_reg=GPK_CALL_IDXS,
                        elem_size=D,
                        single_packet=True,
                        queue_num=j % 4,
                    )
                    t = tmp_pool.tile([P, 768], mybir.dt.bfloat16, tag="tmp", name="t")
                    pt = part_pool.tile([P, D], mybir.dt.bfloat16, tag="pt", name="pt")
                    nc.vector.tensor_max(
                        out=t[:, 0:512], in0=st[:, 0:512], in1=st[:, 512:1024]
                    )
                    nc.vector.tensor_max(
                        out=t[:, 512:768], in0=t[:, 0:256], in1=t[:, 256:512]
                    )
                    nc.vector.tensor_max(
                        out=pt[:, :], in0=t[:, 512:640], in1=t[:, 640:768]
                    )
                    parts.append(pt)
                m0 = part_pool.tile([P, D], mybir.dt.bfloat16, tag="pt", name="m0")
                m1 = part_pool.tile([P, D], mybir.dt.bfloat16, tag="pt", name="m1")
                nc.vector.tensor_max(out=m0[:, :], in0=parts[0][:, :], in1=parts[1][:, :])
                nc.vector.tensor_max(out=m1[:, :], in0=parts[2][:, :], in1=parts[3][:, :])
                nc.vector.tensor_max(
                    out=res[:, c * D : (c + 1) * D], in0=m0[:, :], in1=m1[:, :]
                )
                if c % GPK_STORE_GROUP == GPK_STORE_GROUP - 1 or c == CHUNKS - 1:
                    c0 = (c // GPK_STORE_GROUP) * GPK_STORE_GROUP
                    nc.sync.dma_start(
                        out=out_view[:, c0 : c + 1, :], in_=res_view[:, c0 : c + 1, :]
                    )

    nc.compile()
    return nc


def _prep_in_maps_gpkt(s_feats, neighbor_indices):
    import ml_dtypes

    s = np.ascontiguousarray(np.asarray(s_feats), dtype=np.float32).astype(
        ml_dtypes.bfloat16
    )
    nb = np.asarray(neighbor_indices)
    ncalls = CHUNKS * GPK_CPC
    in_maps = []
    for core in range(N_CORES):
        sl = nb[core * NODES_PER_CORE : (core + 1) * NODES_PER_CORE].astype(np.int32)
        if PADDED > NODES_PER_CORE:
            pad = np.full((PADDED - NODES_PER_CORE, K), GPK_BASE, np.int32)
            sl = np.concatenate([sl, pad], axis=0)
        sl3 = sl.reshape(CHUNKS, P, K)
        # Each call's last unwrapped position is (k = h*KB+KB-1, p = 127).
        # Permute the neighbors of every (c, 127) node so those positions
        # hold indices >= BASE (max is order-invariant). Uniform-random
        # indices make < GPK_CPC non-negative neighbors impossible in
        # practice; assert instead of handling it.
        for c in range(CHUNKS):
            neigh = sl3[c, 127].copy()
            nonneg = neigh[neigh >= GPK_BASE]
            neg = neigh[neigh < GPK_BASE]
            assert len(nonneg) >= GPK_CPC, (c, len(nonneg))
            rest = np.concatenate([neg, nonneg[GPK_CPC:]])
            new = np.empty(K, np.int32)
            ends = [h * GPK_KB + GPK_KB - 1 for h in range(GPK_CPC)]
            new[ends] = nonneg[:GPK_CPC]
            new[[k for k in range(K) if k not in ends]] = rest
            sl3[c, 127] = new
        rem = (sl3 - GPK_BASE).astype(np.int16)  # [c, p, k] signed offsets
        # call (c, h) takes k in [h*KB, (h+1)*KB); position m = k_local*128+p
        vals = rem.transpose(0, 2, 1).reshape(CHUNKS * GPK_CPC, GPK_KB * P)
        lanes = vals.reshape(ncalls, GPK_CALL_SLOTS, 16).transpose(2, 0, 1)
        part_block = np.ascontiguousarray(lanes).reshape(16, ncalls * GPK_CALL_SLOTS)
        full = np.tile(part_block, (8, 1))
        in_maps.append({"table": s, "idx": full})
    return in_maps


# ---------------------------------------------------------------- gbf16 ---
GBF_BASE = 25000  # signed int16 offsets reach rows 0..50000 from here
GBF_KB = 16  # neighbor blocks per gather call (half of K)
GBF_CPC = K // GBF_KB  # 2 calls per chunk
# 2049 emitted descriptors per call: 16 k-blocks of 128 plus ONE dummy
# sentinel (offset 0, >= 0) so the Q7's trailing-negative trim can never
# drop real descriptors. Positions 2050.. of the last 16-lane group are -1
# (trimmed if the ucode rounds up). 2049 fits the per-queue descriptor ring
# (dynamic_dma_scratch_size/16 = 3072 descs) so calls pipeline.
GBF_CALL_IDXS = GBF_KB * P + 1  # 2049
GBF_CALL_SLOTS = (GBF_CALL_IDXS + 15) // 16  # 129 int16 slots per partition
GBF_STORE_GROUP = 8


def _build_nc_gbf16():
    import concourse.bacc as bacc
    import concourse.mybir as mybir
    import concourse.tile as tile

    # A 2049-index gather emits ~129 descriptors per SWDGE ring lane (64 B
    # each); 49152 B of scratch gives each queue a 3072-descriptor ring.
    nc = bacc.Bacc(
        "TRN2", target_bir_lowering=False, debug=False,
        dynamic_dma_scratch_size=49152, num_swdge_queues=4,
    )
    table = nc.dram_tensor(
        "table", [N_NODES, D], mybir.dt.bfloat16, kind="ExternalInput"
    ).ap()
    ncalls = CHUNKS * GBF_CPC
    idx = nc.dram_tensor(
        "idx", [P, ncalls * GBF_CALL_SLOTS], mybir.dt.int16, kind="ExternalInput"
    ).ap()
    out = nc.dram_tensor(
        "out", [PADDED, D], mybir.dt.bfloat16, kind="ExternalOutput"
    ).ap()

    blocks = GBF_KB + 1  # 17 gathered blocks per call (last holds the sentinel)

    with tile.TileContext(nc) as tc:
        with (
            tc.tile_pool(name="pool", bufs=1) as pool,
            tc.tile_pool(name="stage", bufs=10) as stage_pool,
            tc.tile_pool(name="tmp", bufs=8) as tmp_pool,
            tc.tile_pool(name="parts", bufs=8) as part_pool,
        ):
            idx_sb = pool.tile(
                [P, ncalls * GBF_CALL_SLOTS], mybir.dt.int16, name="idx_sb"
            )
            # split the idx load so the first gathers don't wait for the
            # whole index transfer
            head_cols = 8 * GBF_CALL_SLOTS
            nc.sync.dma_start(out=idx_sb[:, :head_cols], in_=idx[:, :head_cols])
            nc.sync.dma_start(out=idx_sb[:, head_cols:], in_=idx[:, head_cols:])

            res = pool.tile([P, CHUNKS * D], mybir.dt.bfloat16, name="res")
            out_view = out.rearrange("(c p) d -> p c d", p=P)
            res_view = res[:, :].rearrange("p (c d) -> p c d", d=D)

            for c in range(CHUNKS):
                parts = []
                for h in range(GBF_CPC):
                    j = c * GBF_CPC + h
                    st = stage_pool.tile(
                        [P, blocks * D], mybir.dt.bfloat16, tag="stage", name="st"
                    )
                    nc.gpsimd.dma_gather(
                        out_ap=st[:, :].rearrange("p (b d) -> p b d", d=D),
                        in_ap=table[GBF_BASE:, :],
                        idxs_ap=idx_sb[
                            :, j * GBF_CALL_SLOTS : (j + 1) * GBF_CALL_SLOTS
                        ],
                        num_idxs=GBF_CALL_IDXS,
                        num_idxs_reg=GBF_CALL_IDXS,
                        elem_size=D,
                        single_packet=False,
                        queue_num=j % 4,
                    )
                    # binary max tree over the 16 real blocks (contiguous
                    # bf16 slices keep the DVE in 2x_1p mode; a strided
                    # tensor_reduce has no fast mode)
                    t = tmp_pool.tile(
                        [P, 1792], mybir.dt.bfloat16, tag="tmp", name="t"
                    )
                    pt = part_pool.tile([P, D], mybir.dt.bfloat16, tag="pt", name="pt")
                    nc.vector.tensor_max(
                        out=t[:, 0:1024], in0=st[:, 0:1024], in1=st[:, 1024:2048]
                    )
                    nc.vector.tensor_max(
                        out=t[:, 1024:1536], in0=t[:, 0:512], in1=t[:, 512:1024]
                    )
                    nc.vector.tensor_max(
                        out=t[:, 1536:1792], in0=t[:, 1024:1280], in1=t[:, 1280:1536]
                    )
                    nc.vector.tensor_max(
                        out=pt[:, :], in0=t[:, 1536:1664], in1=t[:, 1664:1792]
                    )
                    parts.append(pt)
                nc.vector.tensor_max(
                    out=res[:, c * D : (c + 1) * D],
                    in0=parts[0][:, :],
                    in1=parts[1][:, :],
                )
                # store finished chunk groups while later gathers still run
                if c % GBF_STORE_GROUP == GBF_STORE_GROUP - 1 or c == CHUNKS - 1:
                    c0 = (c // GBF_STORE_GROUP) * GBF_STORE_GROUP
                    nc.sync.dma_start(
                        out=out_view[:, c0 : c + 1, :], in_=res_view[:, c0 : c + 1, :]
                    )

    nc.compile()
    return nc


def _prep_in_maps_gbf16(s_feats, neighbor_indices):
    import ml_dtypes

    s = np.ascontiguousarray(np.asarray(s_feats), dtype=np.float32).astype(
        ml_dtypes.bfloat16
    )
    nb = np.asarray(neighbor_indices)
    ncalls = CHUNKS * GBF_CPC
    in_maps = []
    for core in range(N_CORES):
        sl = nb[core * NODES_PER_CORE : (core + 1) * NODES_PER_CORE].astype(np.int32)
        if PADDED > NODES_PER_CORE:
            # pad nodes gather row GBF_BASE (offset 0); results discarded
            pad = np.full((PADDED - NODES_PER_CORE, K), GBF_BASE, np.int32)
            sl = np.concatenate([sl, pad], axis=0)
        rem = (sl - GBF_BASE).astype(np.int16)  # signed offsets from row BASE
        rem3 = rem.reshape(CHUNKS, P, K)  # node (c, p), neighbor k
        # per call: GBF_KB k-blocks, position m = k*128 + p, then one zero
        # sentinel (>= 0 stops the trailing-negative trim) and -1 fill for
        # the rest of the final 16-lane group
        vals = rem3.transpose(0, 2, 1).reshape(ncalls, GBF_KB * P)
        tail = np.full((ncalls, GBF_CALL_SLOTS * 16 - GBF_KB * P), -1, np.int16)
        tail[:, 0] = 0  # the sentinel
        vals = np.concatenate([vals, tail], axis=1)  # [call, SLOTS*16]
        # wrap: position m -> (lane m%16, slot m//16), replicated to 8 groups
        lanes = vals.reshape(ncalls, GBF_CALL_SLOTS, 16).transpose(2, 0, 1)
        part_block = np.ascontiguousarray(lanes).reshape(16, ncalls * GBF_CALL_SLOTS)
        full = np.tile(part_block, (8, 1))
        in_maps.append({"table": s, "idx": full})
    return in_maps


# --------------------------------------------------------- f32 "gather" ---
BASE = 32768  # table base row: signed int16 idx reaches rows 0..50001
CALL_KB = 16  # neighbor blocks per gather call
CALLS_PER_CHUNK = K // CALL_KB  # 2
CALL_IDXS = CALL_KB * P + P  # 2176: 16 k-blocks of 128 + one dummy tail block
CALL_SLOTS = CALL_IDXS // 16  # 136 int16 slots per partition per call


def _build_nc_gather():
    """One InstDMAGatherAnt per 128-node chunk half: gathers 16 neighbor rows
    (512 B descriptors) from HBM with signed int16 indices relative to table
    row BASE, then a VectorE strided tensor_reduce(max) over K."""
    import concourse.bacc as bacc
    import concourse.mybir as mybir
    import concourse.tile as tile

    nc = bacc.Bacc(
        "TRN2", target_bir_lowering=False, debug=False,
        dynamic_dma_scratch_size=49152, num_swdge_queues=4,
    )
    table = nc.dram_tensor(
        "table", [N_NODES, D], mybir.dt.float32, kind="ExternalInput"
    ).ap()
    idx = nc.dram_tensor(
        "idx", [P, CHUNKS * CALLS_PER_CHUNK * CALL_SLOTS], mybir.dt.int16,
        kind="ExternalInput"
    ).ap()
    out = nc.dram_tensor(
        "out", [PADDED, D], mybir.dt.float32, kind="ExternalOutput"
    ).ap()

    blocks = CALL_IDXS // P  # 17 output blocks per call (last one is dummy)
    ncalls = CHUNKS * CALLS_PER_CHUNK

    with tile.TileContext(nc) as tc:
        with (
            tc.tile_pool(name="pool", bufs=1) as pool,
            tc.tile_pool(name="stage", bufs=8) as stage_pool,
            tc.tile_pool(name="parts", bufs=8) as part_pool,
        ):
            idx_sb = pool.tile([P, ncalls * CALL_SLOTS], mybir.dt.int16, name="idx_sb")
            head_cols = 8 * CALL_SLOTS
            nc.sync.dma_start(out=idx_sb[:, :head_cols], in_=idx[:, :head_cols])
            nc.sync.dma_start(out=idx_sb[:, head_cols:], in_=idx[:, head_cols:])

            res = pool.tile([P, CHUNKS * D], mybir.dt.float32, name="res")
            out_view = out.rearrange("(c p) d -> p c d", p=P)
            res_view = res[:, :].rearrange("p (c d) -> p c d", d=D)
            STORE_GROUP = 8

            for c in range(CHUNKS):
                parts = []
                for h in range(CALLS_PER_CHUNK):
                    j = c * CALLS_PER_CHUNK + h
                    st = stage_pool.tile(
                        [P, blocks * D], mybir.dt.float32, tag="stage", name="st"
                    )
                    nc.gpsimd.dma_gather(
                        out_ap=st[:, :].rearrange("p (b d) -> p b d", d=D),
                        in_ap=table[BASE:, :],
                        idxs_ap=idx_sb[:, j * CALL_SLOTS : (j + 1) * CALL_SLOTS],
                        num_idxs=CALL_IDXS,
                        num_idxs_reg=CALL_IDXS,
                        elem_size=D,
                        single_packet=False,
                        queue_num=j % 4,
                    )
                    view = st[:, : CALL_KB * D].rearrange("p (k d) -> p d k", k=CALL_KB)
                    pt = part_pool.tile([P, D], mybir.dt.float32, tag="pt", name="pt")
                    nc.vector.tensor_reduce(
                        out=pt[:, :],
                        in_=view,
                        axis=mybir.AxisListType.X,
                        op=mybir.AluOpType.max,
                    )
                    parts.append(pt)
                nc.vector.tensor_max(
                    out=res[:, c * D : (c + 1) * D],
                    in0=parts[0][:, :],
                    in1=parts[1][:, :],
                )
                if c % STORE_GROUP == STORE_GROUP - 1 or c == CHUNKS - 1:
                    c0 = (c // STORE_GROUP) * STORE_GROUP
                    nc.sync.dma_start(
                        out=out_view[:, c0 : c + 1, :], in_=res_view[:, c0 : c + 1, :]
                    )

    nc.compile()
    return nc


def _prep_in_maps_gather(s_feats, neighbor_indices):
    s = np.ascontiguousarray(np.asarray(s_feats), dtype=np.float32)
    nb = np.asarray(neighbor_indices)
    in_maps = []
    for core in range(N_CORES):
        sl = nb[core * NODES_PER_CORE : (core + 1) * NODES_PER_CORE].astype(np.int32)
        if PADDED > NODES_PER_CORE:
            pad = np.full((PADDED - NODES_PER_CORE, K), BASE, np.int32)
            sl = np.concatenate([sl, pad], axis=0)
        rem = (sl - BASE).astype(np.int16)
        rem3 = rem.reshape(CHUNKS, P, K)
        vals = rem3.transpose(0, 2, 1).reshape(CHUNKS, CALLS_PER_CHUNK, CALL_KB * P)
        dummy = np.zeros((CHUNKS, CALLS_PER_CHUNK, P), np.int16)
        vals = np.concatenate([vals, dummy], axis=2)
        ncalls = CHUNKS * CALLS_PER_CHUNK
        lanes = vals.reshape(ncalls, CALL_SLOTS, 16).transpose(2, 0, 1)
        part_block = np.ascontiguousarray(lanes).reshape(16, ncalls * CALL_SLOTS)
        full = np.tile(part_block, (8, 1))
        in_maps.append({"table": s, "idx": full})
    return in_maps


# ------------------------------------------------------------------ api ---
def _get_nc(variant=None):
    variant = variant or VARIANT
    if variant not in _nc_cache:
        if variant == "gpkt":
            _nc_cache[variant] = _build_nc_gpkt()
        elif variant == "gbf16":
            _nc_cache[variant] = _build_nc_gbf16()
        elif variant == "gather":
            _nc_cache[variant] = _build_nc_gather()
        else:
            raise ValueError(variant)
    return _nc_cache[variant]


def _prep_in_maps(variant, s_feats, neighbor_indices):
    if variant == "gpkt":
        return _prep_in_maps_gpkt(s_feats, neighbor_indices)
    if variant == "gbf16":
        return _prep_in_maps_gbf16(s_feats, neighbor_indices)
    return _prep_in_maps_gather(s_feats, neighbor_indices)


def run_variant(np_inputs, **run_kwargs):
    """Run the selected variant; returns (full f32 output, BassKernelResults)."""
    from concourse.bass_utils import run_bass_kernel_spmd

    if VARIANT == "gpair":
        in_maps, P_sched, orders = _prep_gpair(**np_inputs)
        key = ("gpair", P_sched)
        if key not in _nc_cache:
            _nc_cache[key] = _build_nc_gpair(P_sched)
        res = run_bass_kernel_spmd(
            _nc_cache[key], in_maps, core_ids=list(range(N_CORES)), **run_kwargs
        )
        out = np.empty((N_NODES, D), np.float32)
        for core in range(N_CORES):
            r = np.asarray(res.results[core]["out"]).astype(np.float32)
            order = orders[core]
            valid = order >= 0
            out[core * NODES_PER_CORE + order[valid]] = r[valid]
        return out, res

    nc = _get_nc()
    in_maps = _prep_in_maps(VARIANT, **np_inputs)
    res = run_bass_kernel_spmd(
        nc, in_maps, core_ids=list(range(N_CORES)), **run_kwargs
    )
    out = np.concatenate(
        [res.results[c]["out"][:NODES_PER_CORE] for c in range(N_CORES)], axis=0
    )
    return out.astype(np.float32), res


def kernel(s_feats, neighbor_indices):
    out, _ = run_variant(
        {"s_feats": s_feats, "neighbor_indices": neighbor_indices}
    )
    return out


# revision 33
# speedup vs baseline: 1.4366x; 1.1112x over previous
"""GNN max-pool message passing kernel for 8 Trainium2 NeuronCores.

Problem: out[n] = max_k s_feats[neighbor_indices[n, k]]  (N=50000, K=32, D=128)

Strategy: data-parallel over destination nodes per the sharding hint;
s_feats is replicated into every core's HBM and each core handles 6250
destination nodes.

Variant "gbf16" (current): the f32 trace showed the 16 SDMA engines ~88%
busy moving 512 B/descriptor (~21 GB/s/engine) — the gather is DMA-engine
byte-throughput-bound, not Q7 descriptor-emission-bound. So the table is
converted to bf16 on the host (tolerance is 2e-2; bf16 rounding is ~4e-3):

  - One InstDMAGatherAnt per 128-node chunk gathers all K=32 neighbor rows
    (256 B descriptors) from HBM with signed int16 indices relative to
    table row BASE (unsigned-stride x signed-index Q7 address math covers
    rows BASE-32768..BASE+32767 => BASE=25000 spans the whole table).
  - Each call carries one dummy tail block of zero offsets so the Q7's
    trailing-negative trim can never drop real descriptors.
  - Calls round-robin over all 4 SWDGE queues; single_packet=False.
  - The K-reduction is a tensor_tensor(max) binary tree over contiguous
    bf16 slices (TensorReduce has NO DVE perf mode — a strided reduce runs
    1 elem/cycle and was 350 us of DVE busy in the f32 baseline; the
    tensor_max tree on packed 2-byte data runs in 2x_1p mode at 0.5
    cyc/elem: ~2.7 us/chunk).
  - Output stays bf16 on HW (exact — max of bf16 inputs) and is converted
    to f32 on the host.

Layout per core:
  - node n -> (chunk c = n // 128, partition p = n % 128); call position
    m = k*128 + p so gathered block k of partition p is neighbor k of node
    (c, p); the output store is a strided HWDGE DMA every STORE_GROUP
    chunks; the 6250 real rows are a contiguous prefix of the 6272-row
    padded output.
  - idx input [128, ncalls*264] int16: per call 4224 positions wrapped
    16-wide (position m -> lane m%16, slot m//16), replicated to all eight
    16-partition groups as InstDMAGatherAnt expects.

Variant "gather" is the older f32 version (measured 489 us on 8 cores).
"""

import numpy as np

N_NODES = 50000
K = 32
D = 128
N_CORES = 8
P = 128
NODES_PER_CORE = N_NODES // N_CORES  # 6250
SLOTS = (NODES_PER_CORE + P - 1) // P  # 49
PADDED = P * SLOTS  # 6272
CHUNKS = PADDED // P  # 49 chunks of 128 nodes

VARIANT = "gpair"  # "gpair" | "gpkt" | "gbf16" | "gather"

_nc_cache = {}


# ---------------------------------------------------------------- gpair ---
# The Q7 dma_gather ucode runs one instruction at a time across the whole
# GpSimd cluster and its descriptor-emission loop costs ~2.3 ns per index
# POSITION regardless of elem_size (up to 16 KB/descriptor) — so kernel
# time is ~(total index positions) x 2.3 ns. This variant cuts positions
# ~19%: the host builds a per-core table permutation pi (greedy max-weight
# path forest over neighbor co-occurrence pairs) so that many nodes have
# two neighbors at consecutive pi positions; one 512 B "pair" descriptor
# (row j of a [49999, 256] sliding-window pair table = pi-rows j, j+1)
# then serves both. Nodes are re-bucketed into chunks by their pair count
# p_n (descending) and each chunk c uses the shared schedule P_c =
# min(p_n in chunk, over all cores): a pair call of P_c blocks (elem 256)
# plus single calls totalling 32-2*P_c blocks (elem 128). No sentinel:
# the slot-127 node of each chunk is chosen/reordered so every call's
# last index is non-negative (trailing-negative trim never fires).
GPR_BASE = 25000  # signed int16 offsets for both tables
GPR_STORE_GROUP = 8


def _gpair_path_forest(sets, n_rows=N_NODES, seed=0):
    """Greedy max-weight path forest over co-occurrence pairs.
    Returns pi (permutation of rows) maximizing per-set adjacent pairs."""
    rng = np.random.default_rng(seed)
    i, j = np.triu_indices(K, 1)
    pairs = np.stack([sets[:, i], sets[:, j]], axis=2).reshape(-1, 2)
    pairs = np.sort(pairs, axis=1)
    pairs = pairs[pairs[:, 0] != pairs[:, 1]]
    pu, counts = np.unique(
        pairs[:, 0].astype(np.int64) * n_rows + pairs[:, 1], return_counts=True
    )
    u = (pu // n_rows).astype(np.int32)
    v = (pu % n_rows).astype(np.int32)
    order = np.lexsort((rng.random(len(u)), -counts))
    u, v = u[order], v[order]
    deg = np.zeros(n_rows, np.int8)
    parent = np.arange(n_rows, dtype=np.int32)

    def find(x):
        while parent[x] != x:
            parent[x] = parent[parent[x]]
            x = parent[x]
        return x

    adj = [[] for _ in range(n_rows)]
    for uu, vv in zip(u.tolist(), v.tolist()):
        if deg[uu] >= 2 or deg[vv] >= 2:
            continue
        ru, rv = find(uu), find(vv)
        if ru == rv:
            continue
        parent[ru] = rv
        deg[uu] += 1
        deg[vv] += 1
        adj[uu].append(vv)
        adj[vv].append(uu)
    visited = np.zeros(n_rows, bool)
    pi = []
    for s in range(n_rows):
        if visited[s] or len(adj[s]) == 2:
            continue
        cur, prev = s, -1
        while True:
            pi.append(cur)
            visited[cur] = True
            nxt = [x for x in adj[cur] if x != prev and not visited[x]]
            if not nxt:
                break
            prev, cur = cur, nxt[0]
    for s in range(n_rows):
        if not visited[s]:
            pi.append(s)
    pi = np.asarray(pi, np.int32)
    assert len(pi) == n_rows
    return pi


def _gpair_phase1(sets):
    """Per-core: pi, per-node pair cover. Returns dict with pos-sorted rows,
    chosen-pair flags and per-node pair counts."""
    pi = _gpair_path_forest(sets)
    pos = np.empty(N_NODES, np.int64)
    pos[pi] = np.arange(N_NODES)
    ps = np.sort(pos[sets], axis=1).astype(np.int32)  # [M, K] pi positions
    d1 = np.diff(ps, axis=1) == 1
    m = len(sets)
    pair_at = np.zeros((m, K - 1), bool)  # cover takes (col, col+1)
    prev = np.zeros(m, bool)
    for col in range(K - 1):
        can = d1[:, col] & ~prev
        pair_at[:, col] = can
        prev = can
    p_n = pair_at.sum(axis=1).astype(np.int32)
    return {"pi": pi, "ps": ps, "pair_at": pair_at, "p_n": p_n}


def _gpair_calls_for_chunk(pc):
    """Call list for a chunk: (is_pair, blocks) per call."""
    calls = []
    if pc > 0:
        calls.append((True, pc))
    s = K - 2 * pc
    while s > 0:
        b = min(s, 16)
        calls.append((False, b))
        s -= b
    return calls


GPR_CALL_BLOCKS = 16  # gather blocks per merged single call
GPR_PAIR_CALL_BLOCKS = 8  # pair calls: same 4 KB stage footprint as singles


def _gpair_call_plan(P_sched):
    """Merged cross-chunk call plan, a pure function of the schedule.

    The Q7 cluster emits ~2 ns/position aggregate only when the four
    concurrently-running calls are comparable in size, so pair blocks and
    single blocks are each concatenated across chunks and split into
    uniform 16-block calls (segments of a call may span chunks; the
    reduce trees each segment separately).

    Returns (calls, ends): calls is a list of dicts with keys ispair,
    blocks, segs=[(chunk, block_offset_in_chunk, nblocks), ...]; issue
    order interleaves the two streams by first-covered chunk. ends maps
    (chunk, ispair) -> list of block offsets (within the chunk's run)
    that are the LAST block of some call (host must place a non-negative
    index at position (that block, partition 127))."""
    streams = {}
    for ispair in (True, False):
        blocks = []  # (chunk, offset_in_chunk)
        for c in range(CHUNKS):
            n = P_sched[c] if ispair else K - 2 * P_sched[c]
            blocks += [(c, o) for o in range(n)]
        step = GPR_PAIR_CALL_BLOCKS if ispair else GPR_CALL_BLOCKS
        calls = []
        for i in range(0, len(blocks), step):
            chunkb = blocks[i : i + step]
            segs = []
            for ch, off in chunkb:
                if segs and segs[-1][0] == ch:
                    segs[-1] = (ch, segs[-1][1], segs[-1][2] + 1)
                else:
                    segs.append((ch, off, 1))
            calls.append(
                {"ispair": ispair, "blocks": len(chunkb), "segs": segs}
            )
        streams[ispair] = calls
    ends = {}
    for ispair, calls in streams.items():
        for call in calls:
            ch, off, nb = call["segs"][-1]
            ends.setdefault((ch, ispair), []).append(off + nb - 1)
    # pairs first as a uniform phase, then singles: mixing 256 B and 512 B
    # calls across queues measurably degrades the Q7 emission rate, and
    # each phase alone runs at ~2.1 ns/position. Chunk pair-partials are
    # held in SBUF (49 x [P, D] bf16 = 12 KB/partition) until the single
    # phase completes each chunk.
    merged = streams[True] + streams[False]
    return merged, ends


def _place_at(arr, ends, base):
    """Permute 1-D arr so arr[e] >= base for each position in ends."""
    if not len(ends):
        return arr
    nonneg = arr[arr >= base]
    neg = arr[arr < base]
    assert len(nonneg) >= len(ends), (len(nonneg), ends)
    rest = np.concatenate([neg, nonneg[len(ends):]])
    new = np.empty(len(arr), arr.dtype)
    ends = np.asarray(sorted(ends))
    new[ends] = nonneg[: len(ends)]
    mask = np.ones(len(arr), bool)
    mask[ends] = False
    new[mask] = rest
    return new


def _gpair_phase2(core_data, P_sched):
    """Per-core: order nodes, build the merged-call idx array. Returns idx
    array [128, total_slots] int16 and node order (orig local id per
    padded slot)."""
    ps, pair_at, p_n = core_data["ps"], core_data["pair_at"], core_data["p_n"]
    m = len(ps)
    order = np.argsort(-p_n, kind="stable").astype(np.int32)
    order_pad = np.concatenate([order, np.full(PADDED - m, -1, np.int32)])
    plan, ends = _gpair_call_plan(P_sched)
    pairs_by_chunk = []
    singles_by_chunk = []
    for c in range(CHUNKS):
        pc = P_sched[c]
        nodes = order_pad[c * P : (c + 1) * P]
        pairs_l = np.zeros((P, pc), np.int32)
        singles_l = np.zeros((P, K - 2 * pc), np.int32)
        for sl in range(P):
            n = nodes[sl]
            if n < 0:
                pairs_l[sl] = GPR_BASE  # pad: harmless reads, discarded
                singles_l[sl] = GPR_BASE
                continue
            cols = np.nonzero(pair_at[n])[0]
            use = cols[:pc]
            pstarts = ps[n][use]
            covered = np.zeros(K, bool)
            covered[use] = True
            covered[use + 1] = True
            pairs_l[sl] = pstarts
            singles_l[sl] = ps[n][~covered]
        pe = ends.get((c, True), [])
        se = ends.get((c, False), [])

        def fix(sl):
            return (pairs_l[sl] >= GPR_BASE).sum() >= len(pe) and (
                singles_l[sl] >= GPR_BASE
            ).sum() >= len(se)

        if not fix(127):
            for sl in range(P):
                if fix(sl):
                    pairs_l[[127, sl]] = pairs_l[[sl, 127]]
                    singles_l[[127, sl]] = singles_l[[sl, 127]]
                    nodes = nodes.copy()
                    nodes[[127, sl]] = nodes[[sl, 127]]
                    order_pad[c * P : (c + 1) * P] = nodes
                    break
            else:
                raise AssertionError(f"chunk {c}: no slot-127 candidate")
        pairs_l[127] = _place_at(pairs_l[127], pe, GPR_BASE)
        singles_l[127] = _place_at(singles_l[127], se, GPR_BASE)
        pairs_by_chunk.append(pairs_l)
        singles_by_chunk.append(singles_l)
    all_vals = []
    for call in plan:
        src = pairs_by_chunk if call["ispair"] else singles_by_chunk
        for ch, off, nb in call["segs"]:
            # block b, position m = b_in_call*128 + p
            all_vals.append(
                (src[ch][:, off : off + nb].T - GPR_BASE)
                .astype(np.int16)
                .reshape(-1)
            )
    flat = np.concatenate(all_vals)  # multiple of 16
    lanes = flat.reshape(-1, 16).T  # [16, total_slots]
    full = np.tile(np.ascontiguousarray(lanes), (8, 1))
    return full, order_pad


def _prep_gpair(s_feats, neighbor_indices):
    import ml_dtypes

    s = np.ascontiguousarray(np.asarray(s_feats), dtype=np.float32).astype(
        ml_dtypes.bfloat16
    )
    nb = np.asarray(neighbor_indices)
    cores = []
    for core in range(N_CORES):
        sets = nb[core * NODES_PER_CORE : (core + 1) * NODES_PER_CORE].astype(
            np.int32
        )
        cores.append(_gpair_phase1(sets))
    # shared schedule: per-chunk min pair count across cores; chunks
    # containing pad nodes get 0
    sorted_pn = [np.sort(c["p_n"])[::-1] for c in cores]
    P_sched = []
    for c in range(CHUNKS):
        if (c + 1) * P > NODES_PER_CORE:
            P_sched.append(0)
        else:
            P_sched.append(
                min(int(sp[(c + 1) * P - 1]) for sp in sorted_pn)
            )
    P_sched = tuple(P_sched)
    in_maps = []
    orders = []
    for core in range(N_CORES):
        idx_full, order_pad = _gpair_phase2(cores[core], P_sched)
        table = s[cores[core]["pi"]]
        ptable = np.ascontiguousarray(
            np.concatenate([table[:-1], table[1:]], axis=1)
        )
        in_maps.append({"table": table, "ptable": ptable, "idx": idx_full})
        orders.append(order_pad)
    return in_maps, P_sched, orders


def _build_nc_gpair(P_sched):
    import concourse.bacc as bacc
    import concourse.mybir as mybir
    import concourse.tile as tile

    nc = bacc.Bacc(
        "TRN2", target_bir_lowering=False, debug=False,
        dynamic_dma_scratch_size=49152, num_swdge_queues=4,
    )
    table = nc.dram_tensor(
        "table", [N_NODES, D], mybir.dt.bfloat16, kind="ExternalInput"
    ).ap()
    ptable = nc.dram_tensor(
        "ptable", [N_NODES - 1, 2 * D], mybir.dt.bfloat16, kind="ExternalInput"
    ).ap()
    plan, _plan_ends = _gpair_call_plan(P_sched)
    total_slots = sum(call["blocks"] * P // 16 for call in plan)
    idx = nc.dram_tensor(
        "idx", [P, total_slots], mybir.dt.int16, kind="ExternalInput"
    ).ap()
    out = nc.dram_tensor(
        "out", [PADDED, D], mybir.dt.bfloat16, kind="ExternalOutput"
    ).ap()

    max_pair_blocks = 2 * max(P_sched)  # width-128 blocks in a pair call
    with tile.TileContext(nc) as tc:
        with (
            tc.tile_pool(name="pool", bufs=1) as pool,
            tc.tile_pool(name="stage", bufs=16) as stage_pool,
            tc.tile_pool(name="tmp", bufs=8) as tmp_pool,
            tc.tile_pool(name="parts", bufs=80) as part_pool,
        ):
            idx_sb = pool.tile([P, total_slots], mybir.dt.int16, name="idx_sb")
            head_cols = min(total_slots, 256)
            nc.sync.dma_start(out=idx_sb[:, :head_cols], in_=idx[:, :head_cols])
            if head_cols < total_slots:
                nc.sync.dma_start(
                    out=idx_sb[:, head_cols:], in_=idx[:, head_cols:]
                )

            res = pool.tile([P, CHUNKS * D], mybir.dt.bfloat16, name="res")
            out_view = out.rearrange("(c p) d -> p c d", p=P)
            res_view = res[:, :].rearrange("p (c d) -> p c d", d=D)

            TMP_ELEMS = GPR_CALL_BLOCKS * D // 2  # max tree level h = 8 blocks

            def tree_reduce(st, start_elems, wblocks):
                """Max-reduce wblocks width-D blocks at st[:, start_elems:]
                to one [P, D] block. Returns (tile, offset)."""
                stragglers = []
                cur, cur_off, n = st, start_elems, wblocks
                while n > 1:
                    h = n // 2
                    if n % 2:
                        stragglers.append((cur, cur_off + (n - 1) * D))
                    # the final 128-wide result may be held until its chunk
                    # completes: put it in the deep parts pool
                    if h == 1:
                        dst = part_pool.tile(
                            [P, D], mybir.dt.bfloat16, tag="pt", name="tr1"
                        )
                    else:
                        dst = tmp_pool.tile(
                            [P, TMP_ELEMS], mybir.dt.bfloat16, tag="tmp",
                            name="tr",
                        )
                    nc.vector.tensor_max(
                        out=dst[:, : h * D],
                        in0=cur[:, cur_off : cur_off + h * D],
                        in1=cur[:, cur_off + h * D : cur_off + 2 * h * D],
                    )
                    cur, cur_off, n = dst, 0, h
                for sg, off in stragglers:
                    dst = part_pool.tile(
                        [P, D], mybir.dt.bfloat16, tag="pt", name="sg"
                    )
                    nc.vector.tensor_max(
                        out=dst[:, :],
                        in0=cur[:, cur_off : cur_off + D],
                        in1=sg[:, off : off + D],
                    )
                    cur, cur_off = dst, 0
                return cur, cur_off

            # expected number of partials (segments) per chunk, per stream
            exp = [0] * CHUNKS
            exp_pair = [0] * CHUNKS
            for call in plan:
                for ch, _off, _nb in call["segs"]:
                    exp[ch] += 1
                    if call["ispair"]:
                        exp_pair[ch] += 1
            chunk_partials = [[] for _ in range(CHUNKS)]
            done = [False] * CHUNKS
            stored_to = 0  # chunks [0, stored_to) already written out

            def finish_chunk(c):
                partials = chunk_partials[c]
                sink = res[:, c * D : (c + 1) * D]
                if len(partials) == 1:
                    (t0, o0) = partials[0]
                    nc.vector.tensor_max(
                        out=sink, in0=t0[:, o0 : o0 + D], in1=t0[:, o0 : o0 + D]
                    )
                else:
                    while len(partials) > 2:
                        (t0, o0), (t1, o1) = partials[0], partials[1]
                        pt = part_pool.tile(
                            [P, D], mybir.dt.bfloat16, tag="pt", name="cmb"
                        )
                        nc.vector.tensor_max(
                            out=pt[:, :],
                            in0=t0[:, o0 : o0 + D],
                            in1=t1[:, o1 : o1 + D],
                        )
                        partials = [(pt, 0)] + partials[2:]
                    (t0, o0), (t1, o1) = partials[0], partials[1]
                    nc.vector.tensor_max(
                        out=sink, in0=t0[:, o0 : o0 + D], in1=t1[:, o1 : o1 + D]
                    )
                chunk_partials[c] = []

            def flush_stores(force=False):
                nonlocal stored_to
                while stored_to < CHUNKS:
                    hi = min(stored_to + GPR_STORE_GROUP, CHUNKS)
                    if not all(done[stored_to:hi]) and not force:
                        return
                    if not all(done[stored_to:hi]):
                        return
                    nc.sync.dma_start(
                        out=out_view[:, stored_to:hi, :],
                        in_=res_view[:, stored_to:hi, :],
                    )
                    stored_to = hi

            rr = 0
            col = 0
            for call in plan:
                ispair = call["ispair"]
                b = call["blocks"]
                elem = 2 * D if ispair else D
                nidx = b * P
                slots = nidx // 16
                st = stage_pool.tile(
                    [P, GPR_CALL_BLOCKS * D], mybir.dt.bfloat16, tag="sst",
                    name="st",
                )
                nc.gpsimd.dma_gather(
                    out_ap=st[:, : b * elem].rearrange("p (b d) -> p b d", d=elem),
                    in_ap=(ptable if ispair else table)[GPR_BASE:, :],
                    idxs_ap=idx_sb[:, col : col + slots],
                    num_idxs=nidx,
                    num_idxs_reg=nidx,
                    elem_size=elem,
                    single_packet=False,
                    queue_num=rr % 4,
                )
                rr += 1
                col += slots
                boff = 0
                for ch, _off, nb in call["segs"]:
                    w = 2 * nb if ispair else nb
                    chunk_partials[ch].append(
                        tree_reduce(st, boff * elem, w)
                    )
                    boff += nb
                    if (
                        ispair
                        and exp_pair[ch] > 1
                        and sum(
                            1 for _ in chunk_partials[ch]
                        ) == exp_pair[ch]
                    ):
                        # collapse this chunk's pair partials to one so the
                        # parts pool isn't exhausted holding them until the
                        # single phase reaches the chunk
                        ps_ = chunk_partials[ch]
                        while len(ps_) > 1:
                            (t0, o0), (t1, o1) = ps_[0], ps_[1]
                            pt = part_pool.tile(
                                [P, D], mybir.dt.bfloat16, tag="pt", name="pp"
                            )
                            nc.vector.tensor_max(
                                out=pt[:, :],
                                in0=t0[:, o0 : o0 + D],
                                in1=t1[:, o1 : o1 + D],
                            )
                            ps_ = [(pt, 0)] + ps_[2:]
                        chunk_partials[ch] = ps_
                        exp[ch] -= exp_pair[ch] - 1
                        exp_pair[ch] = 1
                    if len(chunk_partials[ch]) == exp[ch]:
                        finish_chunk(ch)
                        done[ch] = True
                flush_stores()
            flush_stores(force=True)

    nc.compile()
    return nc


# ----------------------------------------------------------------- gpkt ---
# Like gbf16 but with 1024-index calls and single_packet=True so the Q7
# emits aggregated 64-descriptor packets per ring lane. No dummy sentinel:
# the host permutes the neighbors of each partition-127 node so the last
# unwrapped position of every call holds a non-negative offset (the
# trailing-negative trim then never fires).
GPK_BASE = 25000
GPK_KB = 8  # neighbor blocks per call
GPK_CPC = K // GPK_KB  # 4 calls per chunk
GPK_CALL_IDXS = GPK_KB * P  # 1024 = 64 descriptors per ring lane
GPK_CALL_SLOTS = GPK_CALL_IDXS // 16  # 64
GPK_STORE_GROUP = 8


def _build_nc_gpkt():
    import concourse.bacc as bacc
    import concourse.mybir as mybir
    import concourse.tile as tile

    nc = bacc.Bacc(
        "TRN2", target_bir_lowering=False, debug=False,
        dynamic_dma_scratch_size=49152, num_swdge_queues=4,
    )
    table = nc.dram_tensor(
        "table", [N_NODES, D], mybir.dt.bfloat16, kind="ExternalInput"
    ).ap()
    ncalls = CHUNKS * GPK_CPC
    idx = nc.dram_tensor(
        "idx", [P, ncalls * GPK_CALL_SLOTS], mybir.dt.int16, kind="ExternalInput"
    ).ap()
    out = nc.dram_tensor(
        "out", [PADDED, D], mybir.dt.bfloat16, kind="ExternalOutput"
    ).ap()

    with tile.TileContext(nc) as tc:
        with (
            tc.tile_pool(name="pool", bufs=1) as pool,
            tc.tile_pool(name="stage", bufs=12) as stage_pool,
            tc.tile_pool(name="tmp", bufs=8) as tmp_pool,
            tc.tile_pool(name="parts", bufs=12) as part_pool,
        ):
            idx_sb = pool.tile(
                [P, ncalls * GPK_CALL_SLOTS], mybir.dt.int16, name="idx_sb"
            )
            head_cols = 16 * GPK_CALL_SLOTS
            nc.sync.dma_start(out=idx_sb[:, :head_cols], in_=idx[:, :head_cols])
            nc.sync.dma_start(out=idx_sb[:, head_cols:], in_=idx[:, head_cols:])

            res = pool.tile([P, CHUNKS * D], mybir.dt.bfloat16, name="res")
            out_view = out.rearrange("(c p) d -> p c d", p=P)
            res_view = res[:, :].rearrange("p (c d) -> p c d", d=D)

            for c in range(CHUNKS):
                parts = []
                for h in range(GPK_CPC):
                    j = c * GPK_CPC + h
                    st = stage_pool.tile(
                        [P, GPK_KB * D], mybir.dt.bfloat16, tag="stage", name="st"
                    )
                    nc.gpsimd.dma_gather(
                        out_ap=st[:, :].rearrange("p (b d) -> p b d", d=D),
                        in_ap=table[GPK_BASE:, :],
                        idxs_ap=idx_sb[
                            :, j * GPK_CALL_SLOTS : (j + 1) * GPK_CALL_SLOTS
                        ],
                        num_idxs=GPK_CALL_IDXS,
                        num_idxs_reg=GPK_CALL_IDXS,
                        elem_size=D,
                        single_packet=True,
                        queue_num=j % 4,
                    )
                    t = tmp_pool.tile([P, 768], mybir.dt.bfloat16, tag="tmp", name="t")
                    pt = part_pool.tile([P, D], mybir.dt.bfloat16, tag="pt", name="pt")
                    nc.vector.tensor_max(
                        out=t[:, 0:512], in0=st[:, 0:512], in1=st[:, 512:1024]
                    )
                    nc.vector.tensor_max(
                        out=t[:, 512:768], in0=t[:, 0:256], in1=t[:, 256:512]
                    )
                    nc.vector.tensor_max(
                        out=pt[:, :], in0=t[:, 512:640], in1=t[:, 640:768]
                    )
                    parts.append(pt)
                m0 = part_pool.tile([P, D], mybir.dt.bfloat16, tag="pt", name="m0")
                m1 = part_pool.tile([P, D], mybir.dt.bfloat16, tag="pt", name="m1")
                nc.vector.tensor_max(out=m0[:, :], in0=parts[0][:, :], in1=parts[1][:, :])
                nc.vector.tensor_max(out=m1[:, :], in0=parts[2][:, :], in1=parts[3][:, :])
                nc.vector.tensor_max(
                    out=res[:, c * D : (c + 1) * D], in0=m0[:, :], in1=m1[:, :]
                )
                if c % GPK_STORE_GROUP == GPK_STORE_GROUP - 1 or c == CHUNKS - 1:
                    c0 = (c // GPK_STORE_GROUP) * GPK_STORE_GROUP
                    nc.sync.dma_start(
                        out=out_view[:, c0 : c + 1, :], in_=res_view[:, c0 : c + 1, :]
                    )

    nc.compile()
    return nc


def _prep_in_maps_gpkt(s_feats, neighbor_indices):
    import ml_dtypes

    s = np.ascontiguousarray(np.asarray(s_feats), dtype=np.float32).astype(
        ml_dtypes.bfloat16
    )
    nb = np.asarray(neighbor_indices)
    ncalls = CHUNKS * GPK_CPC
    in_maps = []
    for core in range(N_CORES):
        sl = nb[core * NODES_PER_CORE : (core + 1) * NODES_PER_CORE].astype(np.int32)
        if PADDED > NODES_PER_CORE:
            pad = np.full((PADDED - NODES_PER_CORE, K), GPK_BASE, np.int32)
            sl = np.concatenate([sl, pad], axis=0)
        sl3 = sl.reshape(CHUNKS, P, K)
        # Each call's last unwrapped position is (k = h*KB+KB-1, p = 127).
        # Permute the neighbors of every (c, 127) node so those positions
        # hold indices >= BASE (max is order-invariant). Uniform-random
        # indices make < GPK_CPC non-negative neighbors impossible in
        # practice; assert instead of handling it.
        for c in range(CHUNKS):
            neigh = sl3[c, 127].copy()
            nonneg = neigh[neigh >= GPK_BASE]
            neg = neigh[neigh < GPK_BASE]
            assert len(nonneg) >= GPK_CPC, (c, len(nonneg))
            rest = np.concatenate([neg, nonneg[GPK_CPC:]])
            new = np.empty(K, np.int32)
            ends = [h * GPK_KB + GPK_KB - 1 for h in range(GPK_CPC)]
            new[ends] = nonneg[:GPK_CPC]
            new[[k for k in range(K) if k not in ends]] = rest
            sl3[c, 127] = new
        rem = (sl3 - GPK_BASE).astype(np.int16)  # [c, p, k] signed offsets
        # call (c, h) takes k in [h*KB, (h+1)*KB); position m = k_local*128+p
        vals = rem.transpose(0, 2, 1).reshape(CHUNKS * GPK_CPC, GPK_KB * P)
        lanes = vals.reshape(ncalls, GPK_CALL_SLOTS, 16).transpose(2, 0, 1)
        part_block = np.ascontiguousarray(lanes).reshape(16, ncalls * GPK_CALL_SLOTS)
        full = np.tile(part_block, (8, 1))
        in_maps.append({"table": s, "idx": full})
    return in_maps


# ---------------------------------------------------------------- gbf16 ---
GBF_BASE = 25000  # signed int16 offsets reach rows 0..50000 from here
GBF_KB = 16  # neighbor blocks per gather call (half of K)
GBF_CPC = K // GBF_KB  # 2 calls per chunk
# 2049 emitted descriptors per call: 16 k-blocks of 128 plus ONE dummy
# sentinel (offset 0, >= 0) so the Q7's trailing-negative trim can never
# drop real descriptors. Positions 2050.. of the last 16-lane group are -1
# (trimmed if the ucode rounds up). 2049 fits the per-queue descriptor ring
# (dynamic_dma_scratch_size/16 = 3072 descs) so calls pipeline.
GBF_CALL_IDXS = GBF_KB * P + 1  # 2049
GBF_CALL_SLOTS = (GBF_CALL_IDXS + 15) // 16  # 129 int16 slots per partition
GBF_STORE_GROUP = 8


def _build_nc_gbf16():
    import concourse.bacc as bacc
    import concourse.mybir as mybir
    import concourse.tile as tile

    # A 2049-index gather emits ~129 descriptors per SWDGE ring lane (64 B
    # each); 49152 B of scratch gives each queue a 3072-descriptor ring.
    nc = bacc.Bacc(
        "TRN2", target_bir_lowering=False, debug=False,
        dynamic_dma_scratch_size=49152, num_swdge_queues=4,
    )
    table = nc.dram_tensor(
        "table", [N_NODES, D], mybir.dt.bfloat16, kind="ExternalInput"
    ).ap()
    ncalls = CHUNKS * GBF_CPC
    idx = nc.dram_tensor(
        "idx", [P, ncalls * GBF_CALL_SLOTS], mybir.dt.int16, kind="ExternalInput"
    ).ap()
    out = nc.dram_tensor(
        "out", [PADDED, D], mybir.dt.bfloat16, kind="ExternalOutput"
    ).ap()

    blocks = GBF_KB + 1  # 17 gathered blocks per call (last holds the sentinel)

    with tile.TileContext(nc) as tc:
        with (
            tc.tile_pool(name="pool", bufs=1) as pool,
            tc.tile_pool(name="stage", bufs=10) as stage_pool,
            tc.tile_pool(name="tmp", bufs=8) as tmp_pool,
            tc.tile_pool(name="parts", bufs=8) as part_pool,
        ):
            idx_sb = pool.tile(
                [P, ncalls * GBF_CALL_SLOTS], mybir.dt.int16, name="idx_sb"
            )
            # split the idx load so the first gathers don't wait for the
            # whole index transfer
            head_cols = 8 * GBF_CALL_SLOTS
            nc.sync.dma_start(out=idx_sb[:, :head_cols], in_=idx[:, :head_cols])
            nc.sync.dma_start(out=idx_sb[:, head_cols:], in_=idx[:, head_cols:])

            res = pool.tile([P, CHUNKS * D], mybir.dt.bfloat16, name="res")
            out_view = out.rearrange("(c p) d -> p c d", p=P)
            res_view = res[:, :].rearrange("p (c d) -> p c d", d=D)

            for c in range(CHUNKS):
                parts = []
                for h in range(GBF_CPC):
                    j = c * GBF_CPC + h
                    st = stage_pool.tile(
                        [P, blocks * D], mybir.dt.bfloat16, tag="stage", name="st"
                    )
                    nc.gpsimd.dma_gather(
                        out_ap=st[:, :].rearrange("p (b d) -> p b d", d=D),
                        in_ap=table[GBF_BASE:, :],
                        idxs_ap=idx_sb[
                            :, j * GBF_CALL_SLOTS : (j + 1) * GBF_CALL_SLOTS
                        ],
                        num_idxs=GBF_CALL_IDXS,
                        num_idxs_reg=GBF_CALL_IDXS,
                        elem_size=D,
                        single_packet=False,
                        queue_num=j % 4,
                    )
                    # binary max tree over the 16 real blocks (contiguous
                    # bf16 slices keep the DVE in 2x_1p mode; a strided
                    # tensor_reduce has no fast mode)
                    t = tmp_pool.tile(
                        [P, 1792], mybir.dt.bfloat16, tag="tmp", name="t"
                    )
                    pt = part_pool.tile([P, D], mybir.dt.bfloat16, tag="pt", name="pt")
                    nc.vector.tensor_max(
                        out=t[:, 0:1024], in0=st[:, 0:1024], in1=st[:, 1024:2048]
                    )
                    nc.vector.tensor_max(
                        out=t[:, 1024:1536], in0=t[:, 0:512], in1=t[:, 512:1024]
                    )
                    nc.vector.tensor_max(
                        out=t[:, 1536:1792], in0=t[:, 1024:1280], in1=t[:, 1280:1536]
                    )
                    nc.vector.tensor_max(
                        out=pt[:, :], in0=t[:, 1536:1664], in1=t[:, 1664:1792]
                    )
                    parts.append(pt)
                nc.vector.tensor_max(
                    out=res[:, c * D : (c + 1) * D],
                    in0=parts[0][:, :],
                    in1=parts[1][:, :],
                )
                # store finished chunk groups while later gathers still run
                if c % GBF_STORE_GROUP == GBF_STORE_GROUP - 1 or c == CHUNKS - 1:
                    c0 = (c // GBF_STORE_GROUP) * GBF_STORE_GROUP
                    nc.sync.dma_start(
                        out=out_view[:, c0 : c + 1, :], in_=res_view[:, c0 : c + 1, :]
                    )

    nc.compile()
    return nc


def _prep_in_maps_gbf16(s_feats, neighbor_indices):
    import ml_dtypes

    s = np.ascontiguousarray(np.asarray(s_feats), dtype=np.float32).astype(
        ml_dtypes.bfloat16
    )
    nb = np.asarray(neighbor_indices)
    ncalls = CHUNKS * GBF_CPC
    in_maps = []
    for core in range(N_CORES):
        sl = nb[core * NODES_PER_CORE : (core + 1) * NODES_PER_CORE].astype(np.int32)
        if PADDED > NODES_PER_CORE:
            # pad nodes gather row GBF_BASE (offset 0); results discarded
            pad = np.full((PADDED - NODES_PER_CORE, K), GBF_BASE, np.int32)
            sl = np.concatenate([sl, pad], axis=0)
        rem = (sl - GBF_BASE).astype(np.int16)  # signed offsets from row BASE
        rem3 = rem.reshape(CHUNKS, P, K)  # node (c, p), neighbor k
        # per call: GBF_KB k-blocks, position m = k*128 + p, then one zero
        # sentinel (>= 0 stops the trailing-negative trim) and -1 fill for
        # the rest of the final 16-lane group
        vals = rem3.transpose(0, 2, 1).reshape(ncalls, GBF_KB * P)
        tail = np.full((ncalls, GBF_CALL_SLOTS * 16 - GBF_KB * P), -1, np.int16)
        tail[:, 0] = 0  # the sentinel
        vals = np.concatenate([vals, tail], axis=1)  # [call, SLOTS*16]
        # wrap: position m -> (lane m%16, slot m//16), replicated to 8 groups
        lanes = vals.reshape(ncalls, GBF_CALL_SLOTS, 16).transpose(2, 0, 1)
        part_block = np.ascontiguousarray(lanes).reshape(16, ncalls * GBF_CALL_SLOTS)
        full = np.tile(part_block, (8, 1))
        in_maps.append({"table": s, "idx": full})
    return in_maps


# --------------------------------------------------------- f32 "gather" ---
BASE = 32768  # table base row: signed int16 idx reaches rows 0..50001
CALL_KB = 16  # neighbor blocks per gather call
CALLS_PER_CHUNK = K // CALL_KB  # 2
CALL_IDXS = CALL_KB * P + P  # 2176: 16 k-blocks of 128 + one dummy tail block
CALL_SLOTS = CALL_IDXS // 16  # 136 int16 slots per partition per call


def _build_nc_gather():
    """One InstDMAGatherAnt per 128-node chunk half: gathers 16 neighbor rows
    (512 B descriptors) from HBM with signed int16 indices relative to table
    row BASE, then a VectorE strided tensor_reduce(max) over K."""
    import concourse.bacc as bacc
    import concourse.mybir as mybir
    import concourse.tile as tile

    nc = bacc.Bacc(
        "TRN2", target_bir_lowering=False, debug=False,
        dynamic_dma_scratch_size=49152, num_swdge_queues=4,
    )
    table = nc.dram_tensor(
        "table", [N_NODES, D], mybir.dt.float32, kind="ExternalInput"
    ).ap()
    idx = nc.dram_tensor(
        "idx", [P, CHUNKS * CALLS_PER_CHUNK * CALL_SLOTS], mybir.dt.int16,
        kind="ExternalInput"
    ).ap()
    out = nc.dram_tensor(
        "out", [PADDED, D], mybir.dt.float32, kind="ExternalOutput"
    ).ap()

    blocks = CALL_IDXS // P  # 17 output blocks per call (last one is dummy)
    ncalls = CHUNKS * CALLS_PER_CHUNK

    with tile.TileContext(nc) as tc:
        with (
            tc.tile_pool(name="pool", bufs=1) as pool,
            tc.tile_pool(name="stage", bufs=8) as stage_pool,
            tc.tile_pool(name="parts", bufs=8) as part_pool,
        ):
            idx_sb = pool.tile([P, ncalls * CALL_SLOTS], mybir.dt.int16, name="idx_sb")
            head_cols = 8 * CALL_SLOTS
            nc.sync.dma_start(out=idx_sb[:, :head_cols], in_=idx[:, :head_cols])
            nc.sync.dma_start(out=idx_sb[:, head_cols:], in_=idx[:, head_cols:])

            res = pool.tile([P, CHUNKS * D], mybir.dt.float32, name="res")
            out_view = out.rearrange("(c p) d -> p c d", p=P)
            res_view = res[:, :].rearrange("p (c d) -> p c d", d=D)
            STORE_GROUP = 8

            for c in range(CHUNKS):
                parts = []
                for h in range(CALLS_PER_CHUNK):
                    j = c * CALLS_PER_CHUNK + h
                    st = stage_pool.tile(
                        [P, blocks * D], mybir.dt.float32, tag="stage", name="st"
                    )
                    nc.gpsimd.dma_gather(
                        out_ap=st[:, :].rearrange("p (b d) -> p b d", d=D),
                        in_ap=table[BASE:, :],
                        idxs_ap=idx_sb[:, j * CALL_SLOTS : (j + 1) * CALL_SLOTS],
                        num_idxs=CALL_IDXS,
                        num_idxs_reg=CALL_IDXS,
                        elem_size=D,
                        single_packet=False,
                        queue_num=j % 4,
                    )
                    view = st[:, : CALL_KB * D].rearrange("p (k d) -> p d k", k=CALL_KB)
                    pt = part_pool.tile([P, D], mybir.dt.float32, tag="pt", name="pt")
                    nc.vector.tensor_reduce(
                        out=pt[:, :],
                        in_=view,
                        axis=mybir.AxisListType.X,
                        op=mybir.AluOpType.max,
                    )
                    parts.append(pt)
                nc.vector.tensor_max(
                    out=res[:, c * D : (c + 1) * D],
                    in0=parts[0][:, :],
                    in1=parts[1][:, :],
                )
                if c % STORE_GROUP == STORE_GROUP - 1 or c == CHUNKS - 1:
                    c0 = (c // STORE_GROUP) * STORE_GROUP
                    nc.sync.dma_start(
                        out=out_view[:, c0 : c + 1, :], in_=res_view[:, c0 : c + 1, :]
                    )

    nc.compile()
    return nc


def _prep_in_maps_gather(s_feats, neighbor_indices):
    s = np.ascontiguousarray(np.asarray(s_feats), dtype=np.float32)
    nb = np.asarray(neighbor_indices)
    in_maps = []
    for core in range(N_CORES):
        sl = nb[core * NODES_PER_CORE : (core + 1) * NODES_PER_CORE].astype(np.int32)
        if PADDED > NODES_PER_CORE:
            pad = np.full((PADDED - NODES_PER_CORE, K), BASE, np.int32)
            sl = np.concatenate([sl, pad], axis=0)
        rem = (sl - BASE).astype(np.int16)
        rem3 = rem.reshape(CHUNKS, P, K)
        vals = rem3.transpose(0, 2, 1).reshape(CHUNKS, CALLS_PER_CHUNK, CALL_KB * P)
        dummy = np.zeros((CHUNKS, CALLS_PER_CHUNK, P), np.int16)
        vals = np.concatenate([vals, dummy], axis=2)
        ncalls = CHUNKS * CALLS_PER_CHUNK
        lanes = vals.reshape(ncalls, CALL_SLOTS, 16).transpose(2, 0, 1)
        part_block = np.ascontiguousarray(lanes).reshape(16, ncalls * CALL_SLOTS)
        full = np.tile(part_block, (8, 1))
        in_maps.append({"table": s, "idx": full})
    return in_maps


# ------------------------------------------------------------------ api ---
def _get_nc(variant=None):
    variant = variant or VARIANT
    if variant not in _nc_cache:
        if variant == "gpkt":
            _nc_cache[variant] = _build_nc_gpkt()
        elif variant == "gbf16":
            _nc_cache[variant] = _build_nc_gbf16()
        elif variant == "gather":
            _nc_cache[variant] = _build_nc_gather()
        else:
            raise ValueError(variant)
    return _nc_cache[variant]


def _prep_in_maps(variant, s_feats, neighbor_indices):
    if variant == "gpkt":
        return _prep_in_maps_gpkt(s_feats, neighbor_indices)
    if variant == "gbf16":
        return _prep_in_maps_gbf16(s_feats, neighbor_indices)
    return _prep_in_maps_gather(s_feats, neighbor_indices)


def run_variant(np_inputs, **run_kwargs):
    """Run the selected variant; returns (full f32 output, BassKernelResults)."""
    from concourse.bass_utils import run_bass_kernel_spmd

    if VARIANT == "gpair":
        in_maps, P_sched, orders = _prep_gpair(**np_inputs)
        key = ("gpair", P_sched)
        if key not in _nc_cache:
            _nc_cache[key] = _build_nc_gpair(P_sched)
        res = run_bass_kernel_spmd(
            _nc_cache[key], in_maps, core_ids=list(range(N_CORES)), **run_kwargs
        )
        out = np.empty((N_NODES, D), np.float32)
        for core in range(N_CORES):
            r = np.asarray(res.results[core]["out"]).astype(np.float32)
            order = orders[core]
            valid = order >= 0
            out[core * NODES_PER_CORE + order[valid]] = r[valid]
        return out, res

    nc = _get_nc()
    in_maps = _prep_in_maps(VARIANT, **np_inputs)
    res = run_bass_kernel_spmd(
        nc, in_maps, core_ids=list(range(N_CORES)), **run_kwargs
    )
    out = np.concatenate(
        [res.results[c]["out"][:NODES_PER_CORE] for c in range(N_CORES)], axis=0
    )
    return out.astype(np.float32), res


def kernel(s_feats, neighbor_indices):
    out, _ = run_variant(
        {"s_feats": s_feats, "neighbor_indices": neighbor_indices}
    )
    return out


# revision 34
# speedup vs baseline: 1.6133x; 1.1230x over previous
"""GNN max-pool message passing kernel for 8 Trainium2 NeuronCores.

Problem: out[n] = max_k s_feats[neighbor_indices[n, k]]  (N=50000, K=32, D=128)

Strategy: data-parallel over destination nodes per the sharding hint;
s_feats is replicated into every core's HBM (bf16; tolerance is 2e-2 and
bf16 rounding is ~4e-3) and each core handles 6250 destination nodes.

The gather runs on InstDMAGatherAnt (SWDGE). Measured laws on real HW:
  - The Q7 cluster's descriptor-emission loop costs ~2.1 ns per index
    POSITION aggregate (positions = ceil(num_idxs/128)*128 per call),
    independent of elem_size (up to 16 KB/descriptor), queue count, or
    single_packet. Kernel time ~= head + positions*2.1ns + tail.
  - Mixing calls of different elem_size across the four SWDGE queues
    degrades the rate to ~2.4-3.3 ns/pos; uniform-size phases restore it.

So the optimization is INDEX-COUNT COMPRESSION ("gpair" variant): one
512 B descriptor can fetch TWO neighbor rows if they are adjacent under a
host-chosen table permutation. The host runs R=3 rounds of a greedy
max-weight path-forest over neighbor co-occurrence pairs (round r+1 on
the rows left uncovered by round r), giving permutations pi_0..pi_2 and
per-node pair lists. Pair probes read row j of a sliding-window pair
table ptable_r[j] = [s[pi_r[j]], s[pi_r[j+1]]] (elem 256); leftover rows
are single probes into the main table s[pi_0] (elem 128). This removes
~34% of index positions (~200k -> ~132k per core).

Scheduling: the gather grid needs a uniform per-chunk block count, so
nodes are re-bucketed into chunks by their per-round pair counts
(lexicographic sort) and chunk c uses P_r[c] = min over chunk nodes and
cores; dropped pairs fall back to singles. Calls are merged ACROSS
chunks (segments of a call may span chunks) into uniform sizes (8 blocks
for pairs, 16 for singles) and issued in uniform phases: pairs round 0,
1, 2, then singles. Per-chunk partial maxes are combined as streams
complete; trailing-negative trim is defused by reordering each chunk's
slot-127 node lists so every call's last index is non-negative.

The K-reduction is a tensor_tensor(max) binary tree over contiguous bf16
slices (TensorReduce has NO DVE perf mode; tensor_max on packed 2-byte
data runs in 2x_1p mode at 0.5 cyc/elem). Output stays bf16 on HW
(exact) and is converted to f32 on the host, which also un-permutes the
node order.

History (8 cores, HW exec): f32 one-row-per-desc 489 us -> bf16 480 ->
pairs v1 443 -> phase-separated 418 -> uniform stage tiles 376 ->
3-round pairs (this version).
"""

import numpy as np

N_NODES = 50000
K = 32
D = 128
N_CORES = 8
P = 128
NODES_PER_CORE = N_NODES // N_CORES  # 6250
SLOTS = (NODES_PER_CORE + P - 1) // P  # 49
PADDED = P * SLOTS  # 6272
CHUNKS = PADDED // P  # 49 chunks of 128 nodes

VARIANT = "gpair"

_nc_cache = {}

GPR_BASE = 25000  # signed int16 offsets for all tables
GPR_ROUNDS = 3  # pairing rounds (one permutation + pair table each)
GPR_STORE_GROUP = 8
GPR_CALL_BLOCKS = 16  # gather blocks per merged single call
GPR_PAIR_CALL_BLOCKS = 8  # pair calls: same 4 KB stage footprint as singles


# ----------------------------------------------------------- host: pairs ---
def _gpair_path_forest(cand_sets, seed):
    """Greedy max-weight path forest over co-occurrence pairs of the given
    per-node row lists (list of int arrays). Returns pi (permutation of all
    N_NODES rows) maximizing per-set adjacent pairs."""
    rng = np.random.default_rng(seed)
    pairs = []
    for r in cand_sets:
        n = len(r)
        if n < 2:
            continue
        i, j = np.triu_indices(n, 1)
        pairs.append(np.stack([r[i], r[j]], axis=1))
    pairs = np.concatenate(pairs, axis=0)
    pairs = np.sort(pairs, axis=1)
    pairs = pairs[pairs[:, 0] != pairs[:, 1]]
    pu, counts = np.unique(
        pairs[:, 0].astype(np.int64) * N_NODES + pairs[:, 1], return_counts=True
    )
    u = (pu // N_NODES).astype(np.int32)
    v = (pu % N_NODES).astype(np.int32)
    order = np.lexsort((rng.random(len(u)), -counts))
    u, v = u[order], v[order]
    deg = np.zeros(N_NODES, np.int8)
    parent = np.arange(N_NODES, dtype=np.int32)

    def find(x):
        while parent[x] != x:
            parent[x] = parent[parent[x]]
            x = parent[x]
        return x

    adj = [[] for _ in range(N_NODES)]
    for uu, vv in zip(u.tolist(), v.tolist()):
        if deg[uu] >= 2 or deg[vv] >= 2:
            continue
        ru, rv = find(uu), find(vv)
        if ru == rv:
            continue
        parent[ru] = rv
        deg[uu] += 1
        deg[vv] += 1
        adj[uu].append(vv)
        adj[vv].append(uu)
    visited = np.zeros(N_NODES, bool)
    pi = []
    for s in range(N_NODES):
        if visited[s] or len(adj[s]) == 2:
            continue
        cur, prev = s, -1
        while True:
            pi.append(cur)
            visited[cur] = True
            nxt = [x for x in adj[cur] if x != prev and not visited[x]]
            if not nxt:
                break
            prev, cur = cur, nxt[0]
    for s in range(N_NODES):
        if not visited[s]:
            pi.append(s)
    pi = np.asarray(pi, np.int32)
    assert len(pi) == N_NODES
    return pi


def _gpair_phase1(sets):
    """Per-core multi-round pairing.

    Returns dict with:
      pis[r]: permutation per round
      pos0: row -> position in pi_0
      pair_pos[r]: per node, array of pi_r start positions of its pairs
      pair_rows[r]: per node, [p, 2] rows of those pairs
      rows_left: per node, rows not covered by any round
      pn: [M, R] per-node pair counts
    """
    m = len(sets)
    rows_left = [sets[i].astype(np.int32) for i in range(m)]
    pis, pair_pos, pair_rows = [], [], []
    pn = np.zeros((m, GPR_ROUNDS), np.int32)
    for rnd in range(GPR_ROUNDS):
        pi = _gpair_path_forest(rows_left, seed=rnd)
        pos = np.empty(N_NODES, np.int64)
        pos[pi] = np.arange(N_NODES)
        pp_r, prow_r = [], []
        new_left = []
        for i in range(m):
            r = rows_left[i]
            if len(r) < 2:
                pp_r.append(np.empty(0, np.int32))
                prow_r.append(np.empty((0, 2), np.int32))
                new_left.append(r)
                continue
            pr = np.sort(pos[r]).astype(np.int64)
            starts = []
            j = 0
            taken = np.zeros(len(r), bool)
            while j < len(r) - 1:
                if pr[j + 1] == pr[j] + 1:
                    starts.append(pr[j])
                    taken[j] = taken[j + 1] = True
                    j += 2
                else:
                    j += 1
            starts = np.asarray(starts, np.int64)
            pp_r.append(starts.astype(np.int32))
            prow_r.append(
                np.stack([pi[starts], pi[starts + 1]], axis=1).astype(np.int32)
                if len(starts)
                else np.empty((0, 2), np.int32)
            )
            pn[i, rnd] = len(starts)
            new_left.append(pi[pr[~taken]].astype(np.int32))
        rows_left = new_left
        pis.append(pi)
        pair_pos.append(pp_r)
        pair_rows.append(prow_r)
    pos0 = np.empty(N_NODES, np.int64)
    pos0[pis[0]] = np.arange(N_NODES)
    return {
        "pis": pis,
        "pos0": pos0,
        "pair_pos": pair_pos,
        "pair_rows": pair_rows,
        "rows_left": rows_left,
        "pn": pn,
    }


# ------------------------------------------------------------- call plan ---
def _gpair_call_plan(P_scheds):
    """Merged cross-chunk call plan, a pure function of the schedule.

    P_scheds: tuple of GPR_ROUNDS tuples of per-chunk pair counts.
    Streams: pair rounds 0..R-1 (uniform 8-block calls, elem 256), then
    singles (uniform 16-block calls, elem 128) — uniform phases keep the
    Q7 emission at ~2.1 ns/position.

    Returns (calls, ends): calls have keys stream (round index or -1 for
    singles), blocks, segs=[(chunk, off_in_chunk, nblocks), ...]; ends
    maps (chunk, stream) -> block offsets that end some call (host places
    a non-negative index at (that block, partition 127))."""
    plan = []
    ends = {}
    streams = [(r, True) for r in range(GPR_ROUNDS)] + [(-1, False)]
    for stream, ispair in streams:
        blocks = []
        for c in range(CHUNKS):
            if ispair:
                n = P_scheds[stream][c]
            else:
                n = K - 2 * sum(P_scheds[r][c] for r in range(GPR_ROUNDS))
            blocks += [(c, o) for o in range(n)]
        step = GPR_PAIR_CALL_BLOCKS if ispair else GPR_CALL_BLOCKS
        for i in range(0, len(blocks), step):
            chunkb = blocks[i : i + step]
            segs = []
            for ch, off in chunkb:
                if segs and segs[-1][0] == ch:
                    segs[-1] = (ch, segs[-1][1], segs[-1][2] + 1)
                else:
                    segs.append((ch, off, 1))
            plan.append({"stream": stream, "blocks": len(chunkb), "segs": segs})
            ch, off, nb = segs[-1]
            ends.setdefault((ch, stream), []).append(off + nb - 1)
    return plan, ends


def _place_at(arr, ends, base):
    """Permute 1-D arr so arr[e] >= base for each position in ends."""
    if not len(ends):
        return arr
    nonneg = arr[arr >= base]
    neg = arr[arr < base]
    assert len(nonneg) >= len(ends), (len(nonneg), ends)
    rest = np.concatenate([neg, nonneg[len(ends):]])
    new = np.empty(len(arr), arr.dtype)
    e = np.asarray(sorted(ends))
    new[e] = nonneg[: len(e)]
    mask = np.ones(len(arr), bool)
    mask[e] = False
    new[mask] = rest
    return new


def _gpair_phase2(core_data, P_scheds):
    """Per-core: order nodes, build the merged-call idx array. Returns idx
    array [128, total_slots] int16 and node order (orig local id per
    padded slot)."""
    pn = core_data["pn"]
    m = len(pn)
    order = np.lexsort(
        tuple(-pn[:, r] for r in reversed(range(GPR_ROUNDS)))
    ).astype(np.int32)
    order_pad = np.concatenate([order, np.full(PADDED - m, -1, np.int32)])
    plan, ends = _gpair_call_plan(P_scheds)
    lists_by_chunk = []  # per chunk: dict stream -> [P, n] int32 positions
    for c in range(CHUNKS):
        caps = [P_scheds[r][c] for r in range(GPR_ROUNDS)]
        s_c = K - 2 * sum(caps)
        nodes = order_pad[c * P : (c + 1) * P]
        lists = {r: np.zeros((P, caps[r]), np.int32) for r in range(GPR_ROUNDS)}
        lists[-1] = np.zeros((P, s_c), np.int32)
        for sl in range(P):
            n = nodes[sl]
            if n < 0:
                for r in range(GPR_ROUNDS):
                    lists[r][sl] = GPR_BASE
                lists[-1][sl] = GPR_BASE
                continue
            extra_rows = []
            for r in range(GPR_ROUNDS):
                pp = core_data["pair_pos"][r][n]
                lists[r][sl] = pp[: caps[r]]
                if len(pp) > caps[r]:
                    extra_rows.append(
                        core_data["pair_rows"][r][n][caps[r] :].reshape(-1)
                    )
            sing_rows = np.concatenate(
                [core_data["rows_left"][n]] + extra_rows
            ) if extra_rows else core_data["rows_left"][n]
            assert len(sing_rows) == s_c, (c, sl, len(sing_rows), s_c)
            lists[-1][sl] = core_data["pos0"][sing_rows]
        # slot-127 must satisfy every stream's call-end positions
        req = {
            s: len(ends.get((c, s), []))
            for s in list(range(GPR_ROUNDS)) + [-1]
        }

        def fits(sl):
            return all(
                (lists[s][sl] >= GPR_BASE).sum() >= r_ for s, r_ in req.items()
            )

        if not fits(127):
            for sl in range(P):
                if fits(sl):
                    for s in lists:
                        lists[s][[127, sl]] = lists[s][[sl, 127]]
                    nodes = nodes.copy()
                    nodes[[127, sl]] = nodes[[sl, 127]]
                    order_pad[c * P : (c + 1) * P] = nodes
                    break
            else:
                raise AssertionError(f"chunk {c}: no slot-127 candidate")
        for s in lists:
            lists[s][127] = _place_at(
                lists[s][127], ends.get((c, s), []), GPR_BASE
            )
        lists_by_chunk.append(lists)
    all_vals = []
    for call in plan:
        s = call["stream"]
        for ch, off, nb in call["segs"]:
            all_vals.append(
                (lists_by_chunk[ch][s][:, off : off + nb].T - GPR_BASE)
                .astype(np.int16)
                .reshape(-1)
            )
    flat = np.concatenate(all_vals)
    lanes = flat.reshape(-1, 16).T
    full = np.tile(np.ascontiguousarray(lanes), (8, 1))
    return full, order_pad


def _prep_gpair(s_feats, neighbor_indices):
    import ml_dtypes

    s = np.ascontiguousarray(np.asarray(s_feats), dtype=np.float32).astype(
        ml_dtypes.bfloat16
    )
    nb = np.asarray(neighbor_indices)
    cores = []
    for core in range(N_CORES):
        sets = nb[core * NODES_PER_CORE : (core + 1) * NODES_PER_CORE].astype(
            np.int32
        )
        cores.append(_gpair_phase1(sets))
    # shared schedule: per chunk, per round, min pair count across cores
    # after the lexicographic node sort; pad chunks get 0
    sorted_pn = []
    for cdat in cores:
        pn = cdat["pn"]
        o = np.lexsort(tuple(-pn[:, r] for r in reversed(range(GPR_ROUNDS))))
        sorted_pn.append(pn[o])
    P_scheds = []
    for r in range(GPR_ROUNDS):
        ps = []
        for c in range(CHUNKS):
            if (c + 1) * P > NODES_PER_CORE:
                ps.append(0)
                continue
            lo, hi = c * P, (c + 1) * P
            ps.append(min(int(sp[lo:hi, r].min()) for sp in sorted_pn))
        P_scheds.append(tuple(ps))
    P_scheds = tuple(P_scheds)
    in_maps = []
    orders = []
    for core in range(N_CORES):
        idx_full, order_pad = _gpair_phase2(cores[core], P_scheds)
        tabs = {"idx": idx_full}
        t0 = s[cores[core]["pis"][0]]
        tabs["table"] = t0
        for r in range(GPR_ROUNDS):
            tr = s[cores[core]["pis"][r]]
            tabs[f"ptable{r}"] = np.ascontiguousarray(
                np.concatenate([tr[:-1], tr[1:]], axis=1)
            )
        in_maps.append(tabs)
        orders.append(order_pad)
    return in_maps, P_scheds, orders


# ---------------------------------------------------------------- kernel ---
def _build_nc_gpair(P_scheds):
    import concourse.bacc as bacc
    import concourse.mybir as mybir
    import concourse.tile as tile

    nc = bacc.Bacc(
        "TRN2", target_bir_lowering=False, debug=False,
        dynamic_dma_scratch_size=49152, num_swdge_queues=4,
    )
    table = nc.dram_tensor(
        "table", [N_NODES, D], mybir.dt.bfloat16, kind="ExternalInput"
    ).ap()
    ptables = [
        nc.dram_tensor(
            f"ptable{r}", [N_NODES - 1, 2 * D], mybir.dt.bfloat16,
            kind="ExternalInput",
        ).ap()
        for r in range(GPR_ROUNDS)
    ]
    plan, _plan_ends = _gpair_call_plan(P_scheds)
    total_slots = sum(call["blocks"] * P // 16 for call in plan)
    idx = nc.dram_tensor(
        "idx", [P, total_slots], mybir.dt.int16, kind="ExternalInput"
    ).ap()
    out = nc.dram_tensor(
        "out", [PADDED, D], mybir.dt.bfloat16, kind="ExternalOutput"
    ).ap()

    with tile.TileContext(nc) as tc:
        with (
            tc.tile_pool(name="pool", bufs=1) as pool,
            tc.tile_pool(name="stage", bufs=16) as stage_pool,
            tc.tile_pool(name="tmp", bufs=8) as tmp_pool,
            tc.tile_pool(name="parts", bufs=80) as part_pool,
        ):
            idx_sb = pool.tile([P, total_slots], mybir.dt.int16, name="idx_sb")
            head_cols = min(total_slots, 256)
            nc.sync.dma_start(out=idx_sb[:, :head_cols], in_=idx[:, :head_cols])
            if head_cols < total_slots:
                nc.sync.dma_start(
                    out=idx_sb[:, head_cols:], in_=idx[:, head_cols:]
                )

            res = pool.tile([P, CHUNKS * D], mybir.dt.bfloat16, name="res")
            out_view = out.rearrange("(c p) d -> p c d", p=P)
            res_view = res[:, :].rearrange("p (c d) -> p c d", d=D)

            TMP_ELEMS = GPR_CALL_BLOCKS * D // 2  # max tree level = 8 blocks

            def tree_reduce(st, start_elems, wblocks):
                """Max-reduce wblocks width-D blocks at st[:, start_elems:]
                to one [P, D] block. Returns (tile, offset)."""
                stragglers = []
                cur, cur_off, n = st, start_elems, wblocks
                while n > 1:
                    h = n // 2
                    if n % 2:
                        stragglers.append((cur, cur_off + (n - 1) * D))
                    if h == 1:
                        dst = part_pool.tile(
                            [P, D], mybir.dt.bfloat16, tag="pt", name="tr1"
                        )
                    else:
                        dst = tmp_pool.tile(
                            [P, TMP_ELEMS], mybir.dt.bfloat16, tag="tmp",
                            name="tr",
                        )
                    nc.vector.tensor_max(
                        out=dst[:, : h * D],
                        in0=cur[:, cur_off : cur_off + h * D],
                        in1=cur[:, cur_off + h * D : cur_off + 2 * h * D],
                    )
                    cur, cur_off, n = dst, 0, h
                for sg, off in stragglers:
                    dst = part_pool.tile(
                        [P, D], mybir.dt.bfloat16, tag="pt", name="sg"
                    )
                    nc.vector.tensor_max(
                        out=dst[:, :],
                        in0=cur[:, cur_off : cur_off + D],
                        in1=sg[:, off : off + D],
                    )
                    cur, cur_off = dst, 0
                return cur, cur_off

            # per chunk, per stream: expected segment count
            exp_s = {}
            for call in plan:
                for ch, _o, _nb in call["segs"]:
                    exp_s[(ch, call["stream"])] = (
                        exp_s.get((ch, call["stream"]), 0) + 1
                    )
            got_s = {k: 0 for k in exp_s}
            n_streams_left = [0] * CHUNKS
            for (ch, _s), _v in exp_s.items():
                n_streams_left[ch] += 1
            chunk_partials = [[] for _ in range(CHUNKS)]
            done = [False] * CHUNKS
            stored_to = 0

            def collapse(ch, sink=None):
                ps_ = chunk_partials[ch]
                if sink is None and len(ps_) <= 1:
                    return
                while len(ps_) > 2:
                    (t0, o0), (t1, o1) = ps_[0], ps_[1]
                    pt = part_pool.tile(
                        [P, D], mybir.dt.bfloat16, tag="pt", name="cl"
                    )
                    nc.vector.tensor_max(
                        out=pt[:, :],
                        in0=t0[:, o0 : o0 + D],
                        in1=t1[:, o1 : o1 + D],
                    )
                    ps_ = [(pt, 0)] + ps_[2:]
                if sink is not None:
                    if len(ps_) == 1:
                        (t0, o0) = ps_[0]
                        nc.vector.tensor_max(
                            out=sink,
                            in0=t0[:, o0 : o0 + D],
                            in1=t0[:, o0 : o0 + D],
                        )
                    else:
                        (t0, o0), (t1, o1) = ps_[0], ps_[1]
                        nc.vector.tensor_max(
                            out=sink,
                            in0=t0[:, o0 : o0 + D],
                            in1=t1[:, o1 : o1 + D],
                        )
                    chunk_partials[ch] = []
                    return
                if len(ps_) == 2:
                    (t0, o0), (t1, o1) = ps_[0], ps_[1]
                    pt = part_pool.tile(
                        [P, D], mybir.dt.bfloat16, tag="pt", name="cl2"
                    )
                    nc.vector.tensor_max(
                        out=pt[:, :],
                        in0=t0[:, o0 : o0 + D],
                        in1=t1[:, o1 : o1 + D],
                    )
                    ps_ = [(pt, 0)]
                chunk_partials[ch] = ps_

            def flush_stores():
                nonlocal stored_to
                while stored_to < CHUNKS:
                    hi = min(stored_to + GPR_STORE_GROUP, CHUNKS)
                    if not all(done[stored_to:hi]):
                        return
                    nc.sync.dma_start(
                        out=out_view[:, stored_to:hi, :],
                        in_=res_view[:, stored_to:hi, :],
                    )
                    stored_to = hi

            rr = 0
            col = 0
            for call in plan:
                stream = call["stream"]
                ispair = stream >= 0
                b = call["blocks"]
                elem = 2 * D if ispair else D
                nidx = b * P
                slots = nidx // 16
                st = stage_pool.tile(
                    [P, GPR_CALL_BLOCKS * D], mybir.dt.bfloat16, tag="sst",
                    name="st",
                )
                nc.gpsimd.dma_gather(
                    out_ap=st[:, : b * elem].rearrange("p (b d) -> p b d", d=elem),
                    in_ap=(ptables[stream] if ispair else table)[GPR_BASE:, :],
                    idxs_ap=idx_sb[:, col : col + slots],
                    num_idxs=nidx,
                    num_idxs_reg=nidx,
                    elem_size=elem,
                    single_packet=False,
                    queue_num=rr % 4,
                )
                rr += 1
                col += slots
                boff = 0
                for ch, _off, nb in call["segs"]:
                    w = 2 * nb if ispair else nb
                    chunk_partials[ch].append(tree_reduce(st, boff * elem, w))
                    boff += nb
                    key = (ch, stream)
                    got_s[key] += 1
                    if got_s[key] == exp_s[key]:
                        n_streams_left[ch] -= 1
                        if n_streams_left[ch] == 0:
                            collapse(ch, sink=res[:, ch * D : (ch + 1) * D])
                            done[ch] = True
                        else:
                            # stream finished with this chunk: shrink held
                            # partials to one tile
                            collapse(ch)
                flush_stores()
            flush_stores()

    nc.compile()
    return nc


# -------------------------------------------------------------------- api ---
def run_variant(np_inputs, **run_kwargs):
    """Run the kernel; returns (full f32 output, BassKernelResults)."""
    from concourse.bass_utils import run_bass_kernel_spmd

    in_maps, P_scheds, orders = _prep_gpair(**np_inputs)
    key = ("gpair", P_scheds)
    if key not in _nc_cache:
        _nc_cache[key] = _build_nc_gpair(P_scheds)
    res = run_bass_kernel_spmd(
        _nc_cache[key], in_maps, core_ids=list(range(N_CORES)), **run_kwargs
    )
    out = np.empty((N_NODES, D), np.float32)
    for core in range(N_CORES):
        r = np.asarray(res.results[core]["out"]).astype(np.float32)
        order = orders[core]
        valid = order >= 0
        out[core * NODES_PER_CORE + order[valid]] = r[valid]
    return out, res


def kernel(s_feats, neighbor_indices):
    out, _ = run_variant(
        {"s_feats": s_feats, "neighbor_indices": neighbor_indices}
    )
    return out


# revision 41
# speedup vs baseline: 1.8187x; 1.1273x over previous
"""GNN max-pool message passing kernel for 8 Trainium2 NeuronCores.

Problem: out[n] = max_k s_feats[neighbor_indices[n, k]]  (N=50000, K=32, D=128)

Strategy: data-parallel over destination nodes per the sharding hint;
s_feats is replicated into every core's HBM (bf16; tolerance is 2e-2 and
bf16 rounding is ~4e-3) and each core handles 6250 destination nodes.

The gather runs on InstDMAGatherAnt (SWDGE). Measured laws on real HW:
  - The Q7 cluster's descriptor-emission loop costs ~2.1 ns per index
    POSITION aggregate (positions = ceil(num_idxs/128)*128 per call),
    independent of elem_size (up to 16 KB/descriptor), queue count, or
    single_packet. Kernel time ~= head + positions*2.1ns + tail.
  - Mixing calls of different elem_size across the four SWDGE queues
    degrades the rate to ~2.4-3.3 ns/pos; uniform-size phases restore it.

So the optimization is INDEX-COUNT COMPRESSION ("gpair" variant): one
512 B descriptor can fetch TWO neighbor rows if they are adjacent under a
host-chosen table permutation. The host runs R=3 rounds of a greedy
max-weight path-forest over neighbor co-occurrence pairs (round r+1 on
the rows left uncovered by round r), giving permutations pi_0..pi_2 and
per-node pair lists. Pair probes read row j of a sliding-window pair
table ptable_r[j] = [s[pi_r[j]], s[pi_r[j+1]]] (elem 256); leftover rows
are single probes into the main table s[pi_0] (elem 128). This removes
~34% of index positions (~200k -> ~132k per core).

Scheduling: the gather grid needs a uniform per-chunk block count, so
nodes are re-bucketed into chunks by their per-round pair counts
(lexicographic sort) and chunk c uses P_r[c] = min over chunk nodes and
cores; dropped pairs fall back to singles. Calls are merged ACROSS
chunks (segments of a call may span chunks) into uniform sizes (8 blocks
for pairs, 16 for singles) and issued in uniform phases: pairs round 0,
1, 2, then singles. Per-chunk partial maxes are combined as streams
complete; trailing-negative trim is defused by reordering each chunk's
slot-127 node lists so every call's last index is non-negative.

The K-reduction is a tensor_tensor(max) binary tree over contiguous bf16
slices (TensorReduce has NO DVE perf mode; tensor_max on packed 2-byte
data runs in 2x_1p mode at 0.5 cyc/elem). Output stays bf16 on HW
(exact) and is converted to f32 on the host, which also un-permutes the
node order.

History (8 cores, HW exec): f32 one-row-per-desc 489 us -> bf16 480 ->
pairs v1 443 -> phase-separated 418 -> uniform stage tiles 376 ->
3-round pairs (this version).
"""

import numpy as np

N_NODES = 50000
K = 32
D = 128
N_CORES = 8
P = 128
NODES_PER_CORE = N_NODES // N_CORES  # 6250
SLOTS = (NODES_PER_CORE + P - 1) // P  # 49
PADDED = P * SLOTS  # 6272
CHUNKS = PADDED // P  # 49 chunks of 128 nodes

VARIANT = "gpair"

_nc_cache = {}

GPR_BASE = 25000  # signed int16 offsets for all tables
# Pairing rounds (one permutation + pair table each). Per-round per-node
# pair-count caps level the counts so the per-chunk min-capping keeps
# ~90% of the pairs (uncapped greedy loses ~25% to chunk minima).
GPR_CAPS = (3, 3, 3, 3, 3, 2, 2, 2, 2, 2)
GPR_ROUNDS = len(GPR_CAPS)
GPR_STORE_GROUP = 8
GPR_CALL_BLOCKS = 16  # gather blocks per merged single call
GPR_PAIR_CALL_BLOCKS = 8  # pair calls: same 4 KB stage footprint as singles
GPR_PAIR_CAP = 20480  # compact pair-table capacity (rows), fixed for SPMD
GPR_SING_CAP = 24576  # compact single-table capacity (rows)


# ----------------------------------------------------------- host: pairs ---
def _gpair_path_forest(cand_sets, seed):
    """Greedy max-weight path forest over co-occurrence pairs of the given
    per-node row lists (list of int arrays). Returns pi (permutation of all
    N_NODES rows) maximizing per-set adjacent pairs."""
    rng = np.random.default_rng(seed)
    pairs = []
    for r in cand_sets:
        n = len(r)
        if n < 2:
            continue
        i, j = np.triu_indices(n, 1)
        pairs.append(np.stack([r[i], r[j]], axis=1))
    pairs = np.concatenate(pairs, axis=0)
    pairs = np.sort(pairs, axis=1)
    pairs = pairs[pairs[:, 0] != pairs[:, 1]]
    pu, counts = np.unique(
        pairs[:, 0].astype(np.int64) * N_NODES + pairs[:, 1], return_counts=True
    )
    u = (pu // N_NODES).astype(np.int32)
    v = (pu % N_NODES).astype(np.int32)
    order = np.lexsort((rng.random(len(u)), -counts))
    u, v = u[order], v[order]
    deg = np.zeros(N_NODES, np.int8)
    parent = np.arange(N_NODES, dtype=np.int32)

    def find(x):
        while parent[x] != x:
            parent[x] = parent[parent[x]]
            x = parent[x]
        return x

    adj = [[] for _ in range(N_NODES)]
    for uu, vv in zip(u.tolist(), v.tolist()):
        if deg[uu] >= 2 or deg[vv] >= 2:
            continue
        ru, rv = find(uu), find(vv)
        if ru == rv:
            continue
        parent[ru] = rv
        deg[uu] += 1
        deg[vv] += 1
        adj[uu].append(vv)
        adj[vv].append(uu)
    visited = np.zeros(N_NODES, bool)
    pi = []
    for s in range(N_NODES):
        if visited[s] or len(adj[s]) == 2:
            continue
        cur, prev = s, -1
        while True:
            pi.append(cur)
            visited[cur] = True
            nxt = [x for x in adj[cur] if x != prev and not visited[x]]
            if not nxt:
                break
            prev, cur = cur, nxt[0]
    for s in range(N_NODES):
        if not visited[s]:
            pi.append(s)
    pi = np.asarray(pi, np.int32)
    assert len(pi) == N_NODES
    return pi


def _gpair_phase1(sets):
    """Per-core multi-round pairing.

    Returns dict with:
      pis[r]: permutation per round
      pos0: row -> position in pi_0
      pair_pos[r]: per node, array of pi_r start positions of its pairs
      pair_rows[r]: per node, [p, 2] rows of those pairs
      rows_left: per node, rows not covered by any round
      pn: [M, R] per-node pair counts
    """
    m = len(sets)
    rows_left = [sets[i].astype(np.int32) for i in range(m)]
    pis, pair_pos, pair_rows = [], [], []
    pn = np.zeros((m, GPR_ROUNDS), np.int32)
    for rnd in range(GPR_ROUNDS):
        pi = _gpair_path_forest(rows_left, seed=rnd)
        pos = np.empty(N_NODES, np.int64)
        pos[pi] = np.arange(N_NODES)
        pp_r, prow_r = [], []
        new_left = []
        for i in range(m):
            r = rows_left[i]
            if len(r) < 2:
                pp_r.append(np.empty(0, np.int32))
                prow_r.append(np.empty((0, 2), np.int32))
                new_left.append(r)
                continue
            pr = np.sort(pos[r]).astype(np.int64)
            starts = []
            j = 0
            taken = np.zeros(len(r), bool)
            while j < len(r) - 1 and len(starts) < GPR_CAPS[rnd]:
                if pr[j + 1] == pr[j] + 1:
                    starts.append(pr[j])
                    taken[j] = taken[j + 1] = True
                    j += 2
                else:
                    j += 1
            starts = np.asarray(starts, np.int64)
            pp_r.append(starts.astype(np.int32))
            prow_r.append(
                np.stack([pi[starts], pi[starts + 1]], axis=1).astype(np.int32)
                if len(starts)
                else np.empty((0, 2), np.int32)
            )
            pn[i, rnd] = len(starts)
            new_left.append(pi[pr[~taken]].astype(np.int32))
        rows_left = new_left
        pis.append(pi)
        pair_pos.append(pp_r)
        pair_rows.append(prow_r)
    pos0 = np.empty(N_NODES, np.int64)
    pos0[pis[0]] = np.arange(N_NODES)
    return {
        "pis": pis,
        "pos0": pos0,
        "pair_pos": pair_pos,
        "pair_rows": pair_rows,
        "rows_left": rows_left,
        "pn": pn,
    }


# ------------------------------------------------------------- call plan ---
def _gpair_call_plan(P_scheds):
    """Merged cross-chunk call plan, a pure function of the schedule.

    P_scheds: tuple of GPR_ROUNDS tuples of per-chunk pair counts.
    Streams: pair rounds 0..R-1 (uniform 8-block calls, elem 256), then
    singles (uniform 16-block calls, elem 128) — uniform phases keep the
    Q7 emission at ~2.1 ns/position.

    Returns (calls, ends): calls have keys stream (round index or -1 for
    singles), blocks, segs=[(chunk, off_in_chunk, nblocks), ...]; ends
    maps (chunk, stream) -> block offsets that end some call (host places
    a non-negative index at (that block, partition 127))."""
    plan = []
    ends = {}
    streams = [(r, True) for r in range(GPR_ROUNDS)] + [(-1, False)]
    for stream, ispair in streams:
        blocks = []
        for c in range(CHUNKS):
            if ispair:
                n = P_scheds[stream][c]
            else:
                n = K - 2 * sum(P_scheds[r][c] for r in range(GPR_ROUNDS))
            blocks += [(c, o) for o in range(n)]
        step = GPR_PAIR_CALL_BLOCKS if ispair else GPR_CALL_BLOCKS
        for i in range(0, len(blocks), step):
            chunkb = blocks[i : i + step]
            segs = []
            for ch, off in chunkb:
                if segs and segs[-1][0] == ch:
                    segs[-1] = (ch, segs[-1][1], segs[-1][2] + 1)
                else:
                    segs.append((ch, off, 1))
            plan.append({"stream": stream, "blocks": len(chunkb), "segs": segs})
            ch, off, nb = segs[-1]
            ends.setdefault((ch, stream), []).append(off + nb - 1)
    return plan, ends


def _gpair_phase2(core_data, P_scheds):
    """Per-core: order nodes, build the merged-call idx array with COMPACT
    per-stream indexing: each stream's used pair-starts (or single
    positions) get ids 0..U-1 (U < 32768, so every int16 index is
    non-negative and the trailing-negative trim can never fire).

    Returns (idx array [128, total_slots] int16, node order, used):
    used[stream] = array of pi positions in id order (pair starts for
    pair streams, pi_0 positions for singles)."""
    pn = core_data["pn"]
    m = len(pn)
    order = np.lexsort(
        tuple(-pn[:, r] for r in reversed(range(GPR_ROUNDS)))
    ).astype(np.int32)
    order_pad = np.concatenate([order, np.full(PADDED - m, -1, np.int32)])
    plan, _ends = _gpair_call_plan(P_scheds)
    idmaps = {s: {} for s in list(range(GPR_ROUNDS)) + [-1]}

    def to_id(stream, pos):
        d = idmaps[stream]
        i = d.get(pos)
        if i is None:
            i = len(d)
            d[pos] = i
        return i

    lists_by_chunk = []  # per chunk: dict stream -> [P, n] int32 compact ids
    for c in range(CHUNKS):
        caps = [P_scheds[r][c] for r in range(GPR_ROUNDS)]
        s_c = K - 2 * sum(caps)
        nodes = order_pad[c * P : (c + 1) * P]
        lists = {r: np.zeros((P, caps[r]), np.int32) for r in range(GPR_ROUNDS)}
        lists[-1] = np.zeros((P, s_c), np.int32)
        for sl in range(P):
            n = nodes[sl]
            if n < 0:
                continue  # pads keep id 0: harmless duplicate reads
            extra_rows = []
            for r in range(GPR_ROUNDS):
                pp = core_data["pair_pos"][r][n]
                lists[r][sl] = [to_id(r, int(p)) for p in pp[: caps[r]]]
                if len(pp) > caps[r]:
                    extra_rows.append(
                        core_data["pair_rows"][r][n][caps[r] :].reshape(-1)
                    )
            sing_rows = np.concatenate(
                [core_data["rows_left"][n]] + extra_rows
            ) if extra_rows else core_data["rows_left"][n]
            assert len(sing_rows) == s_c, (c, sl, len(sing_rows), s_c)
            lists[-1][sl] = [
                to_id(-1, int(p)) for p in core_data["pos0"][sing_rows]
            ]
        lists_by_chunk.append(lists)
    used = {}
    for s, d in idmaps.items():
        u = np.empty(max(len(d), 1), np.int64)
        u[0] = 0
        for pos, i in d.items():
            u[i] = pos
        assert len(d) <= 32768, (s, len(d))
        used[s] = u
    all_vals = []
    for call in plan:
        s = call["stream"]
        for ch, off, nb in call["segs"]:
            all_vals.append(
                lists_by_chunk[ch][s][:, off : off + nb]
                .T.astype(np.int16)
                .reshape(-1)
            )
    flat = np.concatenate(all_vals)
    lanes = flat.reshape(-1, 16).T
    full = np.tile(np.ascontiguousarray(lanes), (8, 1))
    return full, order_pad, used


def _prep_gpair(s_feats, neighbor_indices):
    import ml_dtypes

    s = np.ascontiguousarray(np.asarray(s_feats), dtype=np.float32).astype(
        ml_dtypes.bfloat16
    )
    nb = np.asarray(neighbor_indices)
    cores = []
    for core in range(N_CORES):
        sets = nb[core * NODES_PER_CORE : (core + 1) * NODES_PER_CORE].astype(
            np.int32
        )
        cores.append(_gpair_phase1(sets))
    # shared schedule: per chunk, per round, min pair count across cores
    # after the lexicographic node sort; pad chunks get 0
    sorted_pn = []
    for cdat in cores:
        pn = cdat["pn"]
        o = np.lexsort(tuple(-pn[:, r] for r in reversed(range(GPR_ROUNDS))))
        sorted_pn.append(pn[o])
    P_scheds = []
    for r in range(GPR_ROUNDS):
        ps = []
        for c in range(CHUNKS):
            if (c + 1) * P > NODES_PER_CORE:
                ps.append(0)
                continue
            lo, hi = c * P, (c + 1) * P
            ps.append(min(int(sp[lo:hi, r].min()) for sp in sorted_pn))
        P_scheds.append(tuple(ps))
    P_scheds = tuple(P_scheds)
    in_maps = []
    orders = []
    for core in range(N_CORES):
        idx_full, order_pad, used = _gpair_phase2(cores[core], P_scheds)
        tabs = {"idx": idx_full}
        sing = used[-1]
        assert len(sing) <= GPR_SING_CAP, len(sing)
        t = np.zeros((GPR_SING_CAP, D), s.dtype)
        t[: len(sing)] = s[cores[core]["pis"][0][sing]]
        tabs["table"] = t
        for r in range(GPR_ROUNDS):
            starts = used[r]
            assert len(starts) <= GPR_PAIR_CAP, (r, len(starts))
            pi = cores[core]["pis"][r]
            pt = np.zeros((GPR_PAIR_CAP, 2 * D), s.dtype)
            pt[: len(starts), :D] = s[pi[starts]]
            pt[: len(starts), D:] = s[pi[starts + 1]]
            tabs[f"ptable{r}"] = pt
        in_maps.append(tabs)
        orders.append(order_pad)
    return in_maps, P_scheds, orders


# ---------------------------------------------------------------- kernel ---
def _build_nc_gpair(P_scheds):
    import concourse.bacc as bacc
    import concourse.mybir as mybir
    import concourse.tile as tile

    nc = bacc.Bacc(
        "TRN2", target_bir_lowering=False, debug=False,
        dynamic_dma_scratch_size=49152, num_swdge_queues=4,
    )
    table = nc.dram_tensor(
        "table", [GPR_SING_CAP, D], mybir.dt.bfloat16, kind="ExternalInput"
    ).ap()
    ptables = [
        nc.dram_tensor(
            f"ptable{r}", [GPR_PAIR_CAP, 2 * D], mybir.dt.bfloat16,
            kind="ExternalInput",
        ).ap()
        for r in range(GPR_ROUNDS)
    ]
    plan, _plan_ends = _gpair_call_plan(P_scheds)
    total_slots = sum(call["blocks"] * P // 16 for call in plan)
    idx = nc.dram_tensor(
        "idx", [P, total_slots], mybir.dt.int16, kind="ExternalInput"
    ).ap()
    out = nc.dram_tensor(
        "out", [PADDED, D], mybir.dt.bfloat16, kind="ExternalOutput"
    ).ap()

    with tile.TileContext(nc) as tc:
        with (
            tc.tile_pool(name="pool", bufs=1) as pool,
            tc.tile_pool(name="stage", bufs=16) as stage_pool,
            tc.tile_pool(name="tmp", bufs=8) as tmp_pool,
            tc.tile_pool(name="parts", bufs=80) as part_pool,
        ):
            idx_sb = pool.tile([P, total_slots], mybir.dt.int16, name="idx_sb")
            head_cols = min(total_slots, 256)
            nc.sync.dma_start(out=idx_sb[:, :head_cols], in_=idx[:, :head_cols])
            if head_cols < total_slots:
                nc.sync.dma_start(
                    out=idx_sb[:, head_cols:], in_=idx[:, head_cols:]
                )

            res = pool.tile([P, CHUNKS * D], mybir.dt.bfloat16, name="res")
            out_view = out.rearrange("(c p) d -> p c d", p=P)
            res_view = res[:, :].rearrange("p (c d) -> p c d", d=D)

            TMP_ELEMS = GPR_CALL_BLOCKS * D // 2  # max tree level = 8 blocks

            def tree_reduce(st, start_elems, wblocks):
                """Max-reduce wblocks width-D blocks at st[:, start_elems:]
                to one [P, D] block. Returns (tile, offset)."""
                stragglers = []
                cur, cur_off, n = st, start_elems, wblocks
                while n > 1:
                    h = n // 2
                    if n % 2:
                        stragglers.append((cur, cur_off + (n - 1) * D))
                    if h == 1:
                        dst = part_pool.tile(
                            [P, D], mybir.dt.bfloat16, tag="pt", name="tr1"
                        )
                    else:
                        dst = tmp_pool.tile(
                            [P, TMP_ELEMS], mybir.dt.bfloat16, tag="tmp",
                            name="tr",
                        )
                    nc.vector.tensor_max(
                        out=dst[:, : h * D],
                        in0=cur[:, cur_off : cur_off + h * D],
                        in1=cur[:, cur_off + h * D : cur_off + 2 * h * D],
                    )
                    cur, cur_off, n = dst, 0, h
                for sg, off in stragglers:
                    dst = part_pool.tile(
                        [P, D], mybir.dt.bfloat16, tag="pt", name="sg"
                    )
                    nc.vector.tensor_max(
                        out=dst[:, :],
                        in0=cur[:, cur_off : cur_off + D],
                        in1=sg[:, off : off + D],
                    )
                    cur, cur_off = dst, 0
                return cur, cur_off

            # per chunk, per stream: expected segment count
            exp_s = {}
            for call in plan:
                for ch, _o, _nb in call["segs"]:
                    exp_s[(ch, call["stream"])] = (
                        exp_s.get((ch, call["stream"]), 0) + 1
                    )
            got_s = {k: 0 for k in exp_s}
            n_streams_left = [0] * CHUNKS
            for (ch, _s), _v in exp_s.items():
                n_streams_left[ch] += 1
            chunk_partials = [[] for _ in range(CHUNKS)]
            done = [False] * CHUNKS
            stored_to = 0

            def collapse(ch, sink=None):
                ps_ = chunk_partials[ch]
                if sink is None and len(ps_) <= 1:
                    return
                while len(ps_) > 2:
                    (t0, o0), (t1, o1) = ps_[0], ps_[1]
                    pt = part_pool.tile(
                        [P, D], mybir.dt.bfloat16, tag="pt", name="cl"
                    )
                    nc.vector.tensor_max(
                        out=pt[:, :],
                        in0=t0[:, o0 : o0 + D],
                        in1=t1[:, o1 : o1 + D],
                    )
                    ps_ = [(pt, 0)] + ps_[2:]
                if sink is not None:
                    if len(ps_) == 1:
                        (t0, o0) = ps_[0]
                        nc.vector.tensor_max(
                            out=sink,
                            in0=t0[:, o0 : o0 + D],
                            in1=t0[:, o0 : o0 + D],
                        )
                    else:
                        (t0, o0), (t1, o1) = ps_[0], ps_[1]
                        nc.vector.tensor_max(
                            out=sink,
                            in0=t0[:, o0 : o0 + D],
                            in1=t1[:, o1 : o1 + D],
                        )
                    chunk_partials[ch] = []
                    return
                if len(ps_) == 2:
                    (t0, o0), (t1, o1) = ps_[0], ps_[1]
                    pt = part_pool.tile(
                        [P, D], mybir.dt.bfloat16, tag="pt", name="cl2"
                    )
                    nc.vector.tensor_max(
                        out=pt[:, :],
                        in0=t0[:, o0 : o0 + D],
                        in1=t1[:, o1 : o1 + D],
                    )
                    ps_ = [(pt, 0)]
                chunk_partials[ch] = ps_

            def flush_stores():
                nonlocal stored_to
                while stored_to < CHUNKS:
                    hi = min(stored_to + GPR_STORE_GROUP, CHUNKS)
                    if not all(done[stored_to:hi]):
                        return
                    nc.sync.dma_start(
                        out=out_view[:, stored_to:hi, :],
                        in_=res_view[:, stored_to:hi, :],
                    )
                    stored_to = hi

            rr = 0
            col = 0
            for call in plan:
                stream = call["stream"]
                ispair = stream >= 0
                b = call["blocks"]
                elem = 2 * D if ispair else D
                nidx = b * P
                slots = nidx // 16
                st = stage_pool.tile(
                    [P, GPR_CALL_BLOCKS * D], mybir.dt.bfloat16, tag="sst",
                    name="st",
                )
                nc.gpsimd.dma_gather(
                    out_ap=st[:, : b * elem].rearrange("p (b d) -> p b d", d=elem),
                    in_ap=(ptables[stream] if ispair else table)[:, :],
                    idxs_ap=idx_sb[:, col : col + slots],
                    num_idxs=nidx,
                    num_idxs_reg=nidx,
                    elem_size=elem,
                    single_packet=False,
                    queue_num=rr % 4,
                )
                rr += 1
                col += slots
                boff = 0
                for ch, _off, nb in call["segs"]:
                    w = 2 * nb if ispair else nb
                    chunk_partials[ch].append(tree_reduce(st, boff * elem, w))
                    boff += nb
                    key = (ch, stream)
                    got_s[key] += 1
                    if got_s[key] == exp_s[key]:
                        n_streams_left[ch] -= 1
                        if n_streams_left[ch] == 0:
                            collapse(ch, sink=res[:, ch * D : (ch + 1) * D])
                            done[ch] = True
                        else:
                            # stream finished with this chunk: shrink held
                            # partials to one tile
                            collapse(ch)
                flush_stores()
            flush_stores()

    nc.compile()
    return nc


# -------------------------------------------------------------------- api ---
def run_variant(np_inputs, **run_kwargs):
    """Run the kernel; returns (full f32 output, BassKernelResults)."""
    from concourse.bass_utils import run_bass_kernel_spmd

    in_maps, P_scheds, orders = _prep_gpair(**np_inputs)
    key = ("gpair", P_scheds)
    if key not in _nc_cache:
        _nc_cache[key] = _build_nc_gpair(P_scheds)
    res = run_bass_kernel_spmd(
        _nc_cache[key], in_maps, core_ids=list(range(N_CORES)), **run_kwargs
    )
    out = np.empty((N_NODES, D), np.float32)
    for core in range(N_CORES):
        r = np.asarray(res.results[core]["out"]).astype(np.float32)
        order = orders[core]
        valid = order >= 0
        out[core * NODES_PER_CORE + order[valid]] = r[valid]
    return out, res


def kernel(s_feats, neighbor_indices):
    out, _ = run_variant(
        {"s_feats": s_feats, "neighbor_indices": neighbor_indices}
    )
    return out


# revision 51
# speedup vs baseline: 1.8377x; 1.0104x over previous
"""GNN max-pool message passing kernel for 8 Trainium2 NeuronCores.

Problem: out[n] = max_k s_feats[neighbor_indices[n, k]]  (N=50000, K=32, D=128)

Strategy: data-parallel over destination nodes per the sharding hint;
s_feats is replicated into every core's HBM (bf16; tolerance is 2e-2 and
bf16 rounding is ~4e-3) and each core handles 6250 destination nodes.

The gather runs on InstDMAGatherAnt (SWDGE). Measured laws on real HW:
  - The Q7 cluster's descriptor-emission loop costs ~2.1 ns per index
    POSITION aggregate (positions = ceil(num_idxs/128)*128 per call),
    independent of elem_size (up to 16 KB/descriptor), queue count, or
    single_packet. Kernel time ~= head + positions*2.1ns + tail.
  - Mixing calls of different elem_size across the four SWDGE queues
    degrades the rate to ~2.4-3.3 ns/pos; uniform-size phases restore it.

So the optimization is INDEX-COUNT COMPRESSION ("gpair" variant): one
512 B descriptor can fetch TWO neighbor rows if they are adjacent under a
host-chosen table permutation. The host runs R=3 rounds of a greedy
max-weight path-forest over neighbor co-occurrence pairs (round r+1 on
the rows left uncovered by round r), giving permutations pi_0..pi_2 and
per-node pair lists. Pair probes read row j of a sliding-window pair
table ptable_r[j] = [s[pi_r[j]], s[pi_r[j+1]]] (elem 256); leftover rows
are single probes into the main table s[pi_0] (elem 128). This removes
~34% of index positions (~200k -> ~132k per core).

Scheduling: the gather grid needs a uniform per-chunk block count, so
nodes are re-bucketed into chunks by their per-round pair counts
(lexicographic sort) and chunk c uses P_r[c] = min over chunk nodes and
cores; dropped pairs fall back to singles. Calls are merged ACROSS
chunks (segments of a call may span chunks) into uniform sizes (8 blocks
for pairs, 16 for singles) and issued in uniform phases: pairs round 0,
1, 2, then singles. Per-chunk partial maxes are combined as streams
complete; trailing-negative trim is defused by reordering each chunk's
slot-127 node lists so every call's last index is non-negative.

The K-reduction is a tensor_tensor(max) binary tree over contiguous bf16
slices (TensorReduce has NO DVE perf mode; tensor_max on packed 2-byte
data runs in 2x_1p mode at 0.5 cyc/elem). Output stays bf16 on HW
(exact) and is converted to f32 on the host, which also un-permutes the
node order.

History (8 cores, HW exec): f32 one-row-per-desc 489 us -> bf16 480 ->
pairs v1 443 -> phase-separated 418 -> uniform stage tiles 376 ->
3-round pairs (this version).
"""

import numpy as np

N_NODES = 50000
K = 32
D = 128
N_CORES = 8
P = 128
NODES_PER_CORE = N_NODES // N_CORES  # 6250
SLOTS = (NODES_PER_CORE + P - 1) // P  # 49
PADDED = P * SLOTS  # 6272
CHUNKS = PADDED // P  # 49 chunks of 128 nodes

VARIANT = "gpair"

_nc_cache = {}

GPR_BASE = 25000  # signed int16 offsets for all tables
# Pairing rounds (one permutation + pair table each). Per-round per-node
# pair-count caps level the counts so the per-chunk min-capping keeps
# ~90% of the pairs (uncapped greedy loses ~25% to chunk minima).
GPR_CAPS = (3, 3, 3, 3, 3, 2, 2, 2, 2, 2)
GPR_ROUNDS = len(GPR_CAPS)
GPR_STORE_GROUP = 8
GPR_CALL_BLOCKS = 16  # gather blocks per merged single call
GPR_PAIR_CALL_BLOCKS = 8  # pair calls: same 4 KB stage footprint as singles
# Pairs are compact-indexed per GROUP of chunks (all rounds together) so a
# chunk's pairs form ONE contiguous gather segment -> one big DVE tree per
# chunk instead of one per round (per-op overhead dominates the DVE).
GPR_GROUPS = 3
GPR_PAIR_CAP = 32768  # compact per-group pair-table capacity (rows)
GPR_SING_CAP = 24576  # compact single-table capacity (rows)


# ----------------------------------------------------------- host: pairs ---
def _gpair_path_forest(cand_sets, seed):
    """Greedy max-weight path forest over co-occurrence pairs of the given
    per-node row lists (list of int arrays). Returns pi (permutation of all
    N_NODES rows) maximizing per-set adjacent pairs."""
    rng = np.random.default_rng(seed)
    pairs = []
    for r in cand_sets:
        n = len(r)
        if n < 2:
            continue
        i, j = np.triu_indices(n, 1)
        pairs.append(np.stack([r[i], r[j]], axis=1))
    pairs = np.concatenate(pairs, axis=0)
    pairs = np.sort(pairs, axis=1)
    pairs = pairs[pairs[:, 0] != pairs[:, 1]]
    pu, counts = np.unique(
        pairs[:, 0].astype(np.int64) * N_NODES + pairs[:, 1], return_counts=True
    )
    u = (pu // N_NODES).astype(np.int32)
    v = (pu % N_NODES).astype(np.int32)
    order = np.lexsort((rng.random(len(u)), -counts))
    u, v = u[order], v[order]
    deg = np.zeros(N_NODES, np.int8)
    parent = np.arange(N_NODES, dtype=np.int32)

    def find(x):
        while parent[x] != x:
            parent[x] = parent[parent[x]]
            x = parent[x]
        return x

    adj = [[] for _ in range(N_NODES)]
    for uu, vv in zip(u.tolist(), v.tolist()):
        if deg[uu] >= 2 or deg[vv] >= 2:
            continue
        ru, rv = find(uu), find(vv)
        if ru == rv:
            continue
        parent[ru] = rv
        deg[uu] += 1
        deg[vv] += 1
        adj[uu].append(vv)
        adj[vv].append(uu)
    visited = np.zeros(N_NODES, bool)
    pi = []
    for s in range(N_NODES):
        if visited[s] or len(adj[s]) == 2:
            continue
        cur, prev = s, -1
        while True:
            pi.append(cur)
            visited[cur] = True
            nxt = [x for x in adj[cur] if x != prev and not visited[x]]
            if not nxt:
                break
            prev, cur = cur, nxt[0]
    for s in range(N_NODES):
        if not visited[s]:
            pi.append(s)
    pi = np.asarray(pi, np.int32)
    assert len(pi) == N_NODES
    return pi


def _gpair_phase1(sets):
    """Per-core multi-round pairing.

    Returns dict with:
      pis[r]: permutation per round
      pos0: row -> position in pi_0
      pair_pos[r]: per node, array of pi_r start positions of its pairs
      pair_rows[r]: per node, [p, 2] rows of those pairs
      rows_left: per node, rows not covered by any round
      pn: [M, R] per-node pair counts
    """
    m = len(sets)
    rows_left = [sets[i].astype(np.int32) for i in range(m)]
    pis, pair_pos, pair_rows = [], [], []
    pn = np.zeros((m, GPR_ROUNDS), np.int32)
    for rnd in range(GPR_ROUNDS):
        pi = _gpair_path_forest(rows_left, seed=rnd)
        pos = np.empty(N_NODES, np.int64)
        pos[pi] = np.arange(N_NODES)
        pp_r, prow_r = [], []
        new_left = []
        for i in range(m):
            r = rows_left[i]
            if len(r) < 2:
                pp_r.append(np.empty(0, np.int32))
                prow_r.append(np.empty((0, 2), np.int32))
                new_left.append(r)
                continue
            pr = np.sort(pos[r]).astype(np.int64)
            starts = []
            j = 0
            taken = np.zeros(len(r), bool)
            while j < len(r) - 1 and len(starts) < GPR_CAPS[rnd]:
                if pr[j + 1] == pr[j] + 1:
                    starts.append(pr[j])
                    taken[j] = taken[j + 1] = True
                    j += 2
                else:
                    j += 1
            starts = np.asarray(starts, np.int64)
            pp_r.append(starts.astype(np.int32))
            prow_r.append(
                np.stack([pi[starts], pi[starts + 1]], axis=1).astype(np.int32)
                if len(starts)
                else np.empty((0, 2), np.int32)
            )
            pn[i, rnd] = len(starts)
            new_left.append(pi[pr[~taken]].astype(np.int32))
        rows_left = new_left
        pis.append(pi)
        pair_pos.append(pp_r)
        pair_rows.append(prow_r)
    pos0 = np.empty(N_NODES, np.int64)
    pos0[pis[0]] = np.arange(N_NODES)
    return {
        "pis": pis,
        "pos0": pos0,
        "pair_pos": pair_pos,
        "pair_rows": pair_rows,
        "rows_left": rows_left,
        "pn": pn,
    }


# ------------------------------------------------------------- call plan ---
def _gpair_groups(P_scheds):
    """Per-chunk group id, balancing total pair instances per group (each
    group's distinct pairs must fit the 32768-row compact table)."""
    per_chunk = [
        P * sum(P_scheds[r][c] for r in range(GPR_ROUNDS))
        for c in range(CHUNKS)
    ]
    total = sum(per_chunk)
    groups = []
    acc = 0
    for c in range(CHUNKS):
        g = min(int(acc * GPR_GROUPS / max(total, 1)), GPR_GROUPS - 1)
        groups.append(g)
        acc += per_chunk[c]
    return groups


def _gpair_call_plan(P_scheds):
    """Merged cross-chunk call plan, a pure function of the schedule.

    P_scheds: tuple of GPR_ROUNDS tuples of per-chunk pair counts.
    Streams: one pair stream per chunk GROUP (a chunk's pairs from all
    rounds are contiguous in its group's compact table; uniform 8-block
    calls, elem 256), then singles (uniform 16-block calls, elem 128).
    Uniform phases keep the Q7 emission at ~2.1 ns/position.

    Returns list of calls with keys stream (group index, or -1 for
    singles), blocks, segs=[(chunk, off_in_chunk, nblocks), ...]."""
    groups = _gpair_groups(P_scheds)
    plan = []
    for g in range(GPR_GROUPS):
        blocks = []
        for c in range(CHUNKS):
            if groups[c] != g:
                continue
            n = sum(P_scheds[r][c] for r in range(GPR_ROUNDS))
            blocks += [(c, o) for o in range(n)]
        for i in range(0, len(blocks), GPR_PAIR_CALL_BLOCKS):
            chunkb = blocks[i : i + GPR_PAIR_CALL_BLOCKS]
            segs = []
            for ch, off in chunkb:
                if segs and segs[-1][0] == ch:
                    segs[-1] = (ch, segs[-1][1], segs[-1][2] + 1)
                else:
                    segs.append((ch, off, 1))
            plan.append({"stream": g, "blocks": len(chunkb), "segs": segs})
    blocks = []
    for c in range(CHUNKS):
        n = K - 2 * sum(P_scheds[r][c] for r in range(GPR_ROUNDS))
        blocks += [(c, o) for o in range(n)]
    for i in range(0, len(blocks), GPR_CALL_BLOCKS):
        chunkb = blocks[i : i + GPR_CALL_BLOCKS]
        segs = []
        for ch, off in chunkb:
            if segs and segs[-1][0] == ch:
                segs[-1] = (ch, segs[-1][1], segs[-1][2] + 1)
            else:
                segs.append((ch, off, 1))
        plan.append({"stream": -1, "blocks": len(chunkb), "segs": segs})
    return plan


def _gpair_phase2(core_data, P_scheds):
    """Per-core: order nodes, build the merged-call idx array with COMPACT
    per-stream indexing: each stream's used pair-starts (or single
    positions) get ids 0..U-1 (U < 32768, so every int16 index is
    non-negative and the trailing-negative trim can never fire).

    Returns (idx array [128, total_slots] int16, node order, used):
    used[stream] = array of pi positions in id order (pair starts for
    pair streams, pi_0 positions for singles)."""
    pn = core_data["pn"]
    m = len(pn)
    order = np.lexsort(
        tuple(-pn[:, r] for r in reversed(range(GPR_ROUNDS)))
    ).astype(np.int32)
    order_pad = np.concatenate([order, np.full(PADDED - m, -1, np.int32)])
    plan = _gpair_call_plan(P_scheds)
    idmaps = {s: {} for s in list(range(GPR_GROUPS)) + [-1]}

    def to_id(stream, key):
        d = idmaps[stream]
        i = d.get(key)
        if i is None:
            i = len(d)
            d[key] = i
        return i

    groups = _gpair_groups(P_scheds)
    lists_by_chunk = []  # per chunk: {group: pair ids [P, n], -1: single ids}
    for c in range(CHUNKS):
        g = groups[c]
        caps = [P_scheds[r][c] for r in range(GPR_ROUNDS)]
        s_c = K - 2 * sum(caps)
        nodes = order_pad[c * P : (c + 1) * P]
        lists = {
            g: np.zeros((P, sum(caps)), np.int32),
            -1: np.zeros((P, s_c), np.int32),
        }
        for sl in range(P):
            n = nodes[sl]
            if n < 0:
                continue  # pads keep id 0: harmless duplicate reads
            extra_rows = []
            ids = []
            for r in range(GPR_ROUNDS):
                pp = core_data["pair_pos"][r][n]
                ids += [to_id(g, (r, int(p))) for p in pp[: caps[r]]]
                if len(pp) > caps[r]:
                    extra_rows.append(
                        core_data["pair_rows"][r][n][caps[r] :].reshape(-1)
                    )
            lists[g][sl] = ids
            sing_rows = np.concatenate(
                [core_data["rows_left"][n]] + extra_rows
            ) if extra_rows else core_data["rows_left"][n]
            assert len(sing_rows) == s_c, (c, sl, len(sing_rows), s_c)
            lists[-1][sl] = [
                to_id(-1, int(p)) for p in core_data["pos0"][sing_rows]
            ]
        lists_by_chunk.append(lists)
    used = {}
    for s, d in idmaps.items():
        cap = GPR_SING_CAP if s == -1 else GPR_PAIR_CAP
        assert len(d) <= cap, (s, len(d))
        u = [0] * max(len(d), 1)
        for key, i in d.items():
            u[i] = key
        used[s] = u
    all_vals = []
    for call in plan:
        s = call["stream"]
        for ch, off, nb in call["segs"]:
            all_vals.append(
                lists_by_chunk[ch][s][:, off : off + nb]
                .T.astype(np.int16)
                .reshape(-1)
            )
    flat = np.concatenate(all_vals)
    lanes = flat.reshape(-1, 16).T
    full = np.tile(np.ascontiguousarray(lanes), (8, 1))
    return full, order_pad, used


def _prep_gpair(s_feats, neighbor_indices):
    import ml_dtypes

    s = np.ascontiguousarray(np.asarray(s_feats), dtype=np.float32).astype(
        ml_dtypes.bfloat16
    )
    nb = np.asarray(neighbor_indices)
    cores = []
    for core in range(N_CORES):
        sets = nb[core * NODES_PER_CORE : (core + 1) * NODES_PER_CORE].astype(
            np.int32
        )
        cores.append(_gpair_phase1(sets))
    # shared schedule: per chunk, per round, min pair count across cores
    # after the lexicographic node sort; pad chunks get 0
    sorted_pn = []
    for cdat in cores:
        pn = cdat["pn"]
        o = np.lexsort(tuple(-pn[:, r] for r in reversed(range(GPR_ROUNDS))))
        sorted_pn.append(pn[o])
    P_scheds = []
    for r in range(GPR_ROUNDS):
        ps = []
        for c in range(CHUNKS):
            if (c + 1) * P > NODES_PER_CORE:
                ps.append(0)
                continue
            lo, hi = c * P, (c + 1) * P
            ps.append(min(int(sp[lo:hi, r].min()) for sp in sorted_pn))
        P_scheds.append(tuple(ps))
    P_scheds = tuple(P_scheds)
    in_maps = []
    orders = []
    for core in range(N_CORES):
        idx_full, order_pad, used = _gpair_phase2(cores[core], P_scheds)
        tabs = {"idx": idx_full}
        sing = np.asarray(used[-1], np.int64)
        t = np.zeros((GPR_SING_CAP, D), s.dtype)
        t[: len(sing)] = s[cores[core]["pis"][0][sing]]
        tabs["table"] = t
        pis = cores[core]["pis"]
        for g in range(GPR_GROUPS):
            keys = used[g]
            pt = np.zeros((GPR_PAIR_CAP, 2 * D), s.dtype)
            if keys and isinstance(keys[0], tuple):
                rr_ = np.asarray([k[0] for k in keys])
                st_ = np.asarray([k[1] for k in keys], np.int64)
                for r in range(GPR_ROUNDS):
                    sel = rr_ == r
                    if not sel.any():
                        continue
                    rows = np.nonzero(sel)[0]
                    pt[rows, :D] = s[pis[r][st_[sel]]]
                    pt[rows, D:] = s[pis[r][st_[sel] + 1]]
            tabs[f"gtable{g}"] = pt
        in_maps.append(tabs)
        orders.append(order_pad)
    return in_maps, P_scheds, orders


# ---------------------------------------------------------------- kernel ---
def _build_nc_gpair(P_scheds):
    import concourse.bacc as bacc
    import concourse.mybir as mybir
    import concourse.tile as tile

    nc = bacc.Bacc(
        "TRN2", target_bir_lowering=False, debug=False,
        dynamic_dma_scratch_size=49152, num_swdge_queues=4,
    )
    table = nc.dram_tensor(
        "table", [GPR_SING_CAP, D], mybir.dt.bfloat16, kind="ExternalInput"
    ).ap()
    ptables = [
        nc.dram_tensor(
            f"gtable{g}", [GPR_PAIR_CAP, 2 * D], mybir.dt.bfloat16,
            kind="ExternalInput",
        ).ap()
        for g in range(GPR_GROUPS)
    ]
    plan = _gpair_call_plan(P_scheds)
    total_slots = sum(call["blocks"] * P // 16 for call in plan)
    idx = nc.dram_tensor(
        "idx", [P, total_slots], mybir.dt.int16, kind="ExternalInput"
    ).ap()
    out = nc.dram_tensor(
        "out", [PADDED, D], mybir.dt.bfloat16, kind="ExternalOutput"
    ).ap()

    with tile.TileContext(nc) as tc:
        with (
            tc.tile_pool(name="pool", bufs=1) as pool,
            tc.tile_pool(name="stage", bufs=16) as stage_pool,
            tc.tile_pool(name="tmp", bufs=8) as tmp_pool,
            tc.tile_pool(name="parts", bufs=80) as part_pool,
        ):
            idx_sb = pool.tile([P, total_slots], mybir.dt.int16, name="idx_sb")
            head_cols = min(total_slots, 256)
            nc.sync.dma_start(out=idx_sb[:, :head_cols], in_=idx[:, :head_cols])
            if head_cols < total_slots:
                nc.sync.dma_start(
                    out=idx_sb[:, head_cols:], in_=idx[:, head_cols:]
                )

            res = pool.tile([P, CHUNKS * D], mybir.dt.bfloat16, name="res")
            out_view = out.rearrange("(c p) d -> p c d", p=P)
            res_view = res[:, :].rearrange("p (c d) -> p c d", d=D)

            TMP_ELEMS = GPR_CALL_BLOCKS * D // 2  # max tree level = 8 blocks

            def tree_reduce(st, start_elems, wblocks):
                """Max-reduce wblocks width-D blocks at st[:, start_elems:]
                to one [P, D] block. Returns (tile, offset)."""
                stragglers = []
                cur, cur_off, n = st, start_elems, wblocks
                while n > 1:
                    h = n // 2
                    if n % 2:
                        stragglers.append((cur, cur_off + (n - 1) * D))
                    if h == 1:
                        dst = part_pool.tile(
                            [P, D], mybir.dt.bfloat16, tag="pt", name="tr1"
                        )
                    else:
                        dst = tmp_pool.tile(
                            [P, TMP_ELEMS], mybir.dt.bfloat16, tag="tmp",
                            name="tr",
                        )
                    nc.vector.tensor_max(
                        out=dst[:, : h * D],
                        in0=cur[:, cur_off : cur_off + h * D],
                        in1=cur[:, cur_off + h * D : cur_off + 2 * h * D],
                    )
                    cur, cur_off, n = dst, 0, h
                for sg, off in stragglers:
                    dst = part_pool.tile(
                        [P, D], mybir.dt.bfloat16, tag="pt", name="sg"
                    )
                    nc.vector.tensor_max(
                        out=dst[:, :],
                        in0=cur[:, cur_off : cur_off + D],
                        in1=sg[:, off : off + D],
                    )
                    cur, cur_off = dst, 0
                return cur, cur_off

            # per chunk, per stream: expected segment count
            exp_s = {}
            for call in plan:
                for ch, _o, _nb in call["segs"]:
                    exp_s[(ch, call["stream"])] = (
                        exp_s.get((ch, call["stream"]), 0) + 1
                    )
            got_s = {k: 0 for k in exp_s}
            n_streams_left = [0] * CHUNKS
            for (ch, _s), _v in exp_s.items():
                n_streams_left[ch] += 1
            chunk_partials = [[] for _ in range(CHUNKS)]
            done = [False] * CHUNKS
            stored_to = 0

            def collapse(ch, sink=None):
                ps_ = chunk_partials[ch]
                if sink is None and len(ps_) <= 1:
                    return
                while len(ps_) > 2:
                    (t0, o0), (t1, o1) = ps_[0], ps_[1]
                    pt = part_pool.tile(
                        [P, D], mybir.dt.bfloat16, tag="pt", name="cl"
                    )
                    nc.vector.tensor_max(
                        out=pt[:, :],
                        in0=t0[:, o0 : o0 + D],
                        in1=t1[:, o1 : o1 + D],
                    )
                    ps_ = [(pt, 0)] + ps_[2:]
                if sink is not None:
                    if len(ps_) == 1:
                        (t0, o0) = ps_[0]
                        nc.vector.tensor_max(
                            out=sink,
                            in0=t0[:, o0 : o0 + D],
                            in1=t0[:, o0 : o0 + D],
                        )
                    else:
                        (t0, o0), (t1, o1) = ps_[0], ps_[1]
                        nc.vector.tensor_max(
                            out=sink,
                            in0=t0[:, o0 : o0 + D],
                            in1=t1[:, o1 : o1 + D],
                        )
                    chunk_partials[ch] = []
                    return
                if len(ps_) == 2:
                    (t0, o0), (t1, o1) = ps_[0], ps_[1]
                    pt = part_pool.tile(
                        [P, D], mybir.dt.bfloat16, tag="pt", name="cl2"
                    )
                    nc.vector.tensor_max(
                        out=pt[:, :],
                        in0=t0[:, o0 : o0 + D],
                        in1=t1[:, o1 : o1 + D],
                    )
                    ps_ = [(pt, 0)]
                chunk_partials[ch] = ps_

            def flush_stores():
                nonlocal stored_to
                while stored_to < CHUNKS:
                    hi = min(stored_to + GPR_STORE_GROUP, CHUNKS)
                    if not all(done[stored_to:hi]):
                        return
                    nc.sync.dma_start(
                        out=out_view[:, stored_to:hi, :],
                        in_=res_view[:, stored_to:hi, :],
                    )
                    stored_to = hi

            rr = 0
            col = 0
            for call in plan:
                stream = call["stream"]
                ispair = stream >= 0
                b = call["blocks"]
                elem = 2 * D if ispair else D
                nidx = b * P
                slots = nidx // 16
                st = stage_pool.tile(
                    [P, GPR_CALL_BLOCKS * D], mybir.dt.bfloat16, tag="sst",
                    name="st",
                )
                nc.gpsimd.dma_gather(
                    out_ap=st[:, : b * elem].rearrange("p (b d) -> p b d", d=elem),
                    in_ap=(ptables[stream] if ispair else table)[:, :],
                    idxs_ap=idx_sb[:, col : col + slots],
                    num_idxs=nidx,
                    num_idxs_reg=nidx,
                    elem_size=elem,
                    single_packet=False,
                    queue_num=rr % 4,
                )
                rr += 1
                col += slots
                boff = 0
                for ch, _off, nb in call["segs"]:
                    w = 2 * nb if ispair else nb
                    chunk_partials[ch].append(tree_reduce(st, boff * elem, w))
                    boff += nb
                    key = (ch, stream)
                    got_s[key] += 1
                    if got_s[key] == exp_s[key]:
                        n_streams_left[ch] -= 1
                        if n_streams_left[ch] == 0:
                            collapse(ch, sink=res[:, ch * D : (ch + 1) * D])
                            done[ch] = True
                        else:
                            # stream finished with this chunk: shrink held
                            # partials to one tile
                            collapse(ch)
                flush_stores()
            flush_stores()

    nc.compile()
    return nc


# -------------------------------------------------------------------- api ---
def run_variant(np_inputs, **run_kwargs):
    """Run the kernel; returns (full f32 output, BassKernelResults)."""
    from concourse.bass_utils import run_bass_kernel_spmd

    in_maps, P_scheds, orders = _prep_gpair(**np_inputs)
    key = ("gpair", P_scheds)
    if key not in _nc_cache:
        _nc_cache[key] = _build_nc_gpair(P_scheds)
    res = run_bass_kernel_spmd(
        _nc_cache[key], in_maps, core_ids=list(range(N_CORES)), **run_kwargs
    )
    out = np.empty((N_NODES, D), np.float32)
    for core in range(N_CORES):
        r = np.asarray(res.results[core]["out"]).astype(np.float32)
        order = orders[core]
        valid = order >= 0
        out[core * NODES_PER_CORE + order[valid]] = r[valid]
    return out, res


def kernel(s_feats, neighbor_indices):
    out, _ = run_variant(
        {"s_feats": s_feats, "neighbor_indices": neighbor_indices}
    )
    return out


# revision 54
# speedup vs baseline: 1.8781x; 1.0220x over previous
"""GNN max-pool message passing kernel for 8 Trainium2 NeuronCores.

Problem: out[n] = max_k s_feats[neighbor_indices[n, k]]  (N=50000, K=32, D=128)

Strategy: data-parallel over destination nodes per the sharding hint;
s_feats is replicated into every core's HBM (bf16; tolerance is 2e-2 and
bf16 rounding is ~4e-3) and each core handles 6250 destination nodes.

The gather runs on InstDMAGatherAnt (SWDGE). Measured laws on real HW:
  - The Q7 cluster's descriptor-emission loop costs ~2.1 ns per index
    POSITION aggregate (positions = ceil(num_idxs/128)*128 per call),
    independent of elem_size (up to 16 KB/descriptor), queue count, or
    single_packet. Kernel time ~= head + positions*2.1ns + tail.
  - Mixing calls of different elem_size across the four SWDGE queues
    degrades the rate to ~2.4-3.3 ns/pos; uniform-size phases restore it.

So the optimization is INDEX-COUNT COMPRESSION ("gpair" variant): one
512 B descriptor can fetch TWO neighbor rows if they are adjacent under a
host-chosen table permutation. The host runs R=3 rounds of a greedy
max-weight path-forest over neighbor co-occurrence pairs (round r+1 on
the rows left uncovered by round r), giving permutations pi_0..pi_2 and
per-node pair lists. Pair probes read row j of a sliding-window pair
table ptable_r[j] = [s[pi_r[j]], s[pi_r[j+1]]] (elem 256); leftover rows
are single probes into the main table s[pi_0] (elem 128). This removes
~34% of index positions (~200k -> ~132k per core).

Scheduling: the gather grid needs a uniform per-chunk block count, so
nodes are re-bucketed into chunks by their per-round pair counts
(lexicographic sort) and chunk c uses P_r[c] = min over chunk nodes and
cores; dropped pairs fall back to singles. Calls are merged ACROSS
chunks (segments of a call may span chunks) into uniform sizes (8 blocks
for pairs, 16 for singles) and issued in uniform phases: pairs round 0,
1, 2, then singles. Per-chunk partial maxes are combined as streams
complete; trailing-negative trim is defused by reordering each chunk's
slot-127 node lists so every call's last index is non-negative.

The K-reduction is a tensor_tensor(max) binary tree over contiguous bf16
slices (TensorReduce has NO DVE perf mode; tensor_max on packed 2-byte
data runs in 2x_1p mode at 0.5 cyc/elem). Output stays bf16 on HW
(exact) and is converted to f32 on the host, which also un-permutes the
node order.

History (8 cores, HW exec): f32 one-row-per-desc 489 us -> bf16 480 ->
pairs v1 443 -> phase-separated 418 -> uniform stage tiles 376 ->
3-round pairs (this version).
"""

import numpy as np

N_NODES = 50000
K = 32
D = 128
N_CORES = 8
P = 128
NODES_PER_CORE = N_NODES // N_CORES  # 6250
SLOTS = (NODES_PER_CORE + P - 1) // P  # 49
PADDED = P * SLOTS  # 6272
CHUNKS = PADDED // P  # 49 chunks of 128 nodes

VARIANT = "gpair"

_nc_cache = {}

GPR_BASE = 25000  # signed int16 offsets for all tables
# Pairing rounds (one permutation + pair table each). Per-round per-node
# pair-count caps level the counts so the per-chunk min-capping keeps
# ~90% of the pairs (uncapped greedy loses ~25% to chunk minima).
GPR_CAPS = (3, 3, 2, 2, 2, 2, 2, 2, 2, 2, 2, 2, 2)
GPR_ROUNDS = len(GPR_CAPS)
GPR_STORE_GROUP = 8
GPR_CALL_BLOCKS = 16  # gather blocks per merged single call
GPR_PAIR_CALL_BLOCKS = 8  # pair calls: same 4 KB stage footprint as singles
# Pairs are compact-indexed per GROUP of chunks (all rounds together) so a
# chunk's pairs form ONE contiguous gather segment -> one big DVE tree per
# chunk instead of one per round (per-op overhead dominates the DVE).
GPR_GROUPS = 4
GPR_PAIR_CAP = 32768  # compact per-group pair-table capacity (rows)
GPR_SING_CAP = 24576  # compact single-table capacity (rows)


# ----------------------------------------------------------- host: pairs ---
def _gpair_path_forest(cand_sets, seed):
    """Greedy max-weight path forest over co-occurrence pairs of the given
    per-node row lists (list of int arrays). Returns pi (permutation of all
    N_NODES rows) maximizing per-set adjacent pairs."""
    rng = np.random.default_rng(seed)
    pairs = []
    for r in cand_sets:
        n = len(r)
        if n < 2:
            continue
        i, j = np.triu_indices(n, 1)
        pairs.append(np.stack([r[i], r[j]], axis=1))
    if not pairs:
        return np.arange(N_NODES, dtype=np.int32)
    pairs = np.concatenate(pairs, axis=0)
    pairs = np.sort(pairs, axis=1)
    pairs = pairs[pairs[:, 0] != pairs[:, 1]]
    pu, counts = np.unique(
        pairs[:, 0].astype(np.int64) * N_NODES + pairs[:, 1], return_counts=True
    )
    u = (pu // N_NODES).astype(np.int32)
    v = (pu % N_NODES).astype(np.int32)
    order = np.lexsort((rng.random(len(u)), -counts))
    u, v = u[order], v[order]
    deg = np.zeros(N_NODES, np.int8)
    parent = np.arange(N_NODES, dtype=np.int32)

    def find(x):
        while parent[x] != x:
            parent[x] = parent[parent[x]]
            x = parent[x]
        return x

    adj = [[] for _ in range(N_NODES)]
    for uu, vv in zip(u.tolist(), v.tolist()):
        if deg[uu] >= 2 or deg[vv] >= 2:
            continue
        ru, rv = find(uu), find(vv)
        if ru == rv:
            continue
        parent[ru] = rv
        deg[uu] += 1
        deg[vv] += 1
        adj[uu].append(vv)
        adj[vv].append(uu)
    visited = np.zeros(N_NODES, bool)
    pi = []
    for s in range(N_NODES):
        if visited[s] or len(adj[s]) == 2:
            continue
        cur, prev = s, -1
        while True:
            pi.append(cur)
            visited[cur] = True
            nxt = [x for x in adj[cur] if x != prev and not visited[x]]
            if not nxt:
                break
            prev, cur = cur, nxt[0]
    for s in range(N_NODES):
        if not visited[s]:
            pi.append(s)
    pi = np.asarray(pi, np.int32)
    assert len(pi) == N_NODES
    return pi


def _gpair_phase1(sets):
    """Per-core multi-round pairing.

    Returns dict with:
      pis[r]: permutation per round
      pos0: row -> position in pi_0
      pair_pos[r]: per node, array of pi_r start positions of its pairs
      pair_rows[r]: per node, [p, 2] rows of those pairs
      rows_left: per node, rows not covered by any round
      pn: [M, R] per-node pair counts
    """
    m = len(sets)
    rows_left = [sets[i].astype(np.int32) for i in range(m)]
    pis, pair_pos, pair_rows = [], [], []
    pn = np.zeros((m, GPR_ROUNDS), np.int32)
    for rnd in range(GPR_ROUNDS):
        pi = _gpair_path_forest(rows_left, seed=rnd)
        pos = np.empty(N_NODES, np.int64)
        pos[pi] = np.arange(N_NODES)
        pp_r, prow_r = [], []
        new_left = []
        for i in range(m):
            r = rows_left[i]
            if len(r) < 2:
                pp_r.append(np.empty(0, np.int32))
                prow_r.append(np.empty((0, 2), np.int32))
                new_left.append(r)
                continue
            pr = np.sort(pos[r]).astype(np.int64)
            starts = []
            j = 0
            taken = np.zeros(len(r), bool)
            while j < len(r) - 1 and len(starts) < GPR_CAPS[rnd]:
                if pr[j + 1] == pr[j] + 1:
                    starts.append(pr[j])
                    taken[j] = taken[j + 1] = True
                    j += 2
                else:
                    j += 1
            starts = np.asarray(starts, np.int64)
            pp_r.append(starts.astype(np.int32))
            prow_r.append(
                np.stack([pi[starts], pi[starts + 1]], axis=1).astype(np.int32)
                if len(starts)
                else np.empty((0, 2), np.int32)
            )
            pn[i, rnd] = len(starts)
            new_left.append(pi[pr[~taken]].astype(np.int32))
        rows_left = new_left
        pis.append(pi)
        pair_pos.append(pp_r)
        pair_rows.append(prow_r)
    pos0 = np.empty(N_NODES, np.int64)
    pos0[pis[0]] = np.arange(N_NODES)
    return {
        "pis": pis,
        "pos0": pos0,
        "pair_pos": pair_pos,
        "pair_rows": pair_rows,
        "rows_left": rows_left,
        "pn": pn,
    }


# ------------------------------------------------------------- call plan ---
def _gpair_groups(P_scheds):
    """Per-chunk group id, balancing total pair instances per group (each
    group's distinct pairs must fit the 32768-row compact table)."""
    per_chunk = [
        P * sum(P_scheds[r][c] for r in range(GPR_ROUNDS))
        for c in range(CHUNKS)
    ]
    total = sum(per_chunk)
    groups = []
    acc = 0
    for c in range(CHUNKS):
        g = min(int(acc * GPR_GROUPS / max(total, 1)), GPR_GROUPS - 1)
        groups.append(g)
        acc += per_chunk[c]
    return groups


def _gpair_call_plan(P_scheds):
    """Merged cross-chunk call plan, a pure function of the schedule.

    P_scheds: tuple of GPR_ROUNDS tuples of per-chunk pair counts.
    Streams: one pair stream per chunk GROUP (a chunk's pairs from all
    rounds are contiguous in its group's compact table; uniform 8-block
    calls, elem 256), then singles (uniform 16-block calls, elem 128).
    Uniform phases keep the Q7 emission at ~2.1 ns/position.

    Returns list of calls with keys stream (group index, or -1 for
    singles), blocks, segs=[(chunk, off_in_chunk, nblocks), ...]."""
    groups = _gpair_groups(P_scheds)
    plan = []
    for g in range(GPR_GROUPS):
        blocks = []
        for c in range(CHUNKS):
            if groups[c] != g:
                continue
            n = sum(P_scheds[r][c] for r in range(GPR_ROUNDS))
            blocks += [(c, o) for o in range(n)]
        for i in range(0, len(blocks), GPR_PAIR_CALL_BLOCKS):
            chunkb = blocks[i : i + GPR_PAIR_CALL_BLOCKS]
            segs = []
            for ch, off in chunkb:
                if segs and segs[-1][0] == ch:
                    segs[-1] = (ch, segs[-1][1], segs[-1][2] + 1)
                else:
                    segs.append((ch, off, 1))
            plan.append({"stream": g, "blocks": len(chunkb), "segs": segs})
    blocks = []
    for c in range(CHUNKS):
        n = K - 2 * sum(P_scheds[r][c] for r in range(GPR_ROUNDS))
        blocks += [(c, o) for o in range(n)]
    for i in range(0, len(blocks), GPR_CALL_BLOCKS):
        chunkb = blocks[i : i + GPR_CALL_BLOCKS]
        segs = []
        for ch, off in chunkb:
            if segs and segs[-1][0] == ch:
                segs[-1] = (ch, segs[-1][1], segs[-1][2] + 1)
            else:
                segs.append((ch, off, 1))
        plan.append({"stream": -1, "blocks": len(chunkb), "segs": segs})
    return plan


def _gpair_phase2(core_data, P_scheds):
    """Per-core: order nodes, build the merged-call idx array with COMPACT
    per-stream indexing: each stream's used pair-starts (or single
    positions) get ids 0..U-1 (U < 32768, so every int16 index is
    non-negative and the trailing-negative trim can never fire).

    Returns (idx array [128, total_slots] int16, node order, used):
    used[stream] = array of pi positions in id order (pair starts for
    pair streams, pi_0 positions for singles)."""
    pn = core_data["pn"]
    m = len(pn)
    order = np.lexsort(
        tuple(-pn[:, r] for r in reversed(range(GPR_ROUNDS)))
    ).astype(np.int32)
    order_pad = np.concatenate([order, np.full(PADDED - m, -1, np.int32)])
    plan = _gpair_call_plan(P_scheds)
    idmaps = {s: {} for s in list(range(GPR_GROUPS)) + [-1]}

    def to_id(stream, key):
        d = idmaps[stream]
        i = d.get(key)
        if i is None:
            i = len(d)
            d[key] = i
        return i

    groups = _gpair_groups(P_scheds)
    lists_by_chunk = []  # per chunk: {group: pair ids [P, n], -1: single ids}
    for c in range(CHUNKS):
        g = groups[c]
        caps = [P_scheds[r][c] for r in range(GPR_ROUNDS)]
        s_c = K - 2 * sum(caps)
        nodes = order_pad[c * P : (c + 1) * P]
        lists = {
            g: np.zeros((P, sum(caps)), np.int32),
            -1: np.zeros((P, s_c), np.int32),
        }
        for sl in range(P):
            n = nodes[sl]
            if n < 0:
                continue  # pads keep id 0: harmless duplicate reads
            extra_rows = []
            ids = []
            for r in range(GPR_ROUNDS):
                pp = core_data["pair_pos"][r][n]
                ids += [to_id(g, (r, int(p))) for p in pp[: caps[r]]]
                if len(pp) > caps[r]:
                    extra_rows.append(
                        core_data["pair_rows"][r][n][caps[r] :].reshape(-1)
                    )
            lists[g][sl] = ids
            sing_rows = np.concatenate(
                [core_data["rows_left"][n]] + extra_rows
            ) if extra_rows else core_data["rows_left"][n]
            assert len(sing_rows) == s_c, (c, sl, len(sing_rows), s_c)
            lists[-1][sl] = [
                to_id(-1, int(p)) for p in core_data["pos0"][sing_rows]
            ]
        lists_by_chunk.append(lists)
    used = {}
    for s, d in idmaps.items():
        cap = GPR_SING_CAP if s == -1 else GPR_PAIR_CAP
        assert len(d) <= cap, (s, len(d))
        u = [0] * max(len(d), 1)
        for key, i in d.items():
            u[i] = key
        used[s] = u
    all_vals = []
    for call in plan:
        s = call["stream"]
        for ch, off, nb in call["segs"]:
            all_vals.append(
                lists_by_chunk[ch][s][:, off : off + nb]
                .T.astype(np.int16)
                .reshape(-1)
            )
    flat = np.concatenate(all_vals)
    lanes = flat.reshape(-1, 16).T
    full = np.tile(np.ascontiguousarray(lanes), (8, 1))
    return full, order_pad, used


def _prep_gpair(s_feats, neighbor_indices):
    import ml_dtypes

    s = np.ascontiguousarray(np.asarray(s_feats), dtype=np.float32).astype(
        ml_dtypes.bfloat16
    )
    nb = np.asarray(neighbor_indices)
    cores = []
    for core in range(N_CORES):
        sets = nb[core * NODES_PER_CORE : (core + 1) * NODES_PER_CORE].astype(
            np.int32
        )
        cores.append(_gpair_phase1(sets))
    # shared schedule: per chunk, per round, min pair count across cores
    # after the lexicographic node sort; pad chunks get 0
    sorted_pn = []
    for cdat in cores:
        pn = cdat["pn"]
        o = np.lexsort(tuple(-pn[:, r] for r in reversed(range(GPR_ROUNDS))))
        sorted_pn.append(pn[o])
    P_scheds = []
    for r in range(GPR_ROUNDS):
        ps = []
        for c in range(CHUNKS):
            if (c + 1) * P > NODES_PER_CORE:
                ps.append(0)
                continue
            lo, hi = c * P, (c + 1) * P
            ps.append(min(int(sp[lo:hi, r].min()) for sp in sorted_pn))
        P_scheds.append(tuple(ps))
    P_scheds = tuple(P_scheds)
    in_maps = []
    orders = []
    for core in range(N_CORES):
        idx_full, order_pad, used = _gpair_phase2(cores[core], P_scheds)
        tabs = {"idx": idx_full}
        sing = np.asarray(used[-1], np.int64)
        t = np.zeros((GPR_SING_CAP, D), s.dtype)
        t[: len(sing)] = s[cores[core]["pis"][0][sing]]
        tabs["table"] = t
        pis = cores[core]["pis"]
        for g in range(GPR_GROUPS):
            keys = used[g]
            pt = np.zeros((GPR_PAIR_CAP, 2 * D), s.dtype)
            if keys and isinstance(keys[0], tuple):
                rr_ = np.asarray([k[0] for k in keys])
                st_ = np.asarray([k[1] for k in keys], np.int64)
                for r in range(GPR_ROUNDS):
                    sel = rr_ == r
                    if not sel.any():
                        continue
                    rows = np.nonzero(sel)[0]
                    pt[rows, :D] = s[pis[r][st_[sel]]]
                    pt[rows, D:] = s[pis[r][st_[sel] + 1]]
            tabs[f"gtable{g}"] = pt
        in_maps.append(tabs)
        orders.append(order_pad)
    return in_maps, P_scheds, orders


# ---------------------------------------------------------------- kernel ---
def _build_nc_gpair(P_scheds):
    import concourse.bacc as bacc
    import concourse.mybir as mybir
    import concourse.tile as tile

    nc = bacc.Bacc(
        "TRN2", target_bir_lowering=False, debug=False,
        dynamic_dma_scratch_size=49152, num_swdge_queues=4,
    )
    table = nc.dram_tensor(
        "table", [GPR_SING_CAP, D], mybir.dt.bfloat16, kind="ExternalInput"
    ).ap()
    ptables = [
        nc.dram_tensor(
            f"gtable{g}", [GPR_PAIR_CAP, 2 * D], mybir.dt.bfloat16,
            kind="ExternalInput",
        ).ap()
        for g in range(GPR_GROUPS)
    ]
    plan = _gpair_call_plan(P_scheds)
    total_slots = sum(call["blocks"] * P // 16 for call in plan)
    idx = nc.dram_tensor(
        "idx", [P, total_slots], mybir.dt.int16, kind="ExternalInput"
    ).ap()
    out = nc.dram_tensor(
        "out", [PADDED, D], mybir.dt.bfloat16, kind="ExternalOutput"
    ).ap()

    with tile.TileContext(nc) as tc:
        with (
            tc.tile_pool(name="pool", bufs=1) as pool,
            tc.tile_pool(name="stage", bufs=16) as stage_pool,
            tc.tile_pool(name="tmp", bufs=8) as tmp_pool,
            tc.tile_pool(name="parts", bufs=80) as part_pool,
        ):
            idx_sb = pool.tile([P, total_slots], mybir.dt.int16, name="idx_sb")
            head_cols = min(total_slots, 256)
            nc.sync.dma_start(out=idx_sb[:, :head_cols], in_=idx[:, :head_cols])
            if head_cols < total_slots:
                nc.sync.dma_start(
                    out=idx_sb[:, head_cols:], in_=idx[:, head_cols:]
                )

            res = pool.tile([P, CHUNKS * D], mybir.dt.bfloat16, name="res")
            out_view = out.rearrange("(c p) d -> p c d", p=P)
            res_view = res[:, :].rearrange("p (c d) -> p c d", d=D)

            TMP_ELEMS = GPR_CALL_BLOCKS * D // 2  # max tree level = 8 blocks

            def tree_reduce(st, start_elems, wblocks):
                """Max-reduce wblocks width-D blocks at st[:, start_elems:]
                to one [P, D] block. Returns (tile, offset)."""
                stragglers = []
                cur, cur_off, n = st, start_elems, wblocks
                while n > 1:
                    h = n // 2
                    if n % 2:
                        stragglers.append((cur, cur_off + (n - 1) * D))
                    if h == 1:
                        dst = part_pool.tile(
                            [P, D], mybir.dt.bfloat16, tag="pt", name="tr1"
                        )
                    else:
                        dst = tmp_pool.tile(
                            [P, TMP_ELEMS], mybir.dt.bfloat16, tag="tmp",
                            name="tr",
                        )
                    nc.vector.tensor_max(
                        out=dst[:, : h * D],
                        in0=cur[:, cur_off : cur_off + h * D],
                        in1=cur[:, cur_off + h * D : cur_off + 2 * h * D],
                    )
                    cur, cur_off, n = dst, 0, h
                for sg, off in stragglers:
                    dst = part_pool.tile(
                        [P, D], mybir.dt.bfloat16, tag="pt", name="sg"
                    )
                    nc.vector.tensor_max(
                        out=dst[:, :],
                        in0=cur[:, cur_off : cur_off + D],
                        in1=sg[:, off : off + D],
                    )
                    cur, cur_off = dst, 0
                return cur, cur_off

            # per chunk, per stream: expected segment count
            exp_s = {}
            for call in plan:
                for ch, _o, _nb in call["segs"]:
                    exp_s[(ch, call["stream"])] = (
                        exp_s.get((ch, call["stream"]), 0) + 1
                    )
            got_s = {k: 0 for k in exp_s}
            n_streams_left = [0] * CHUNKS
            for (ch, _s), _v in exp_s.items():
                n_streams_left[ch] += 1
            chunk_partials = [[] for _ in range(CHUNKS)]
            done = [False] * CHUNKS
            stored_to = 0

            def collapse(ch, sink=None):
                ps_ = chunk_partials[ch]
                if sink is None and len(ps_) <= 1:
                    return
                while len(ps_) > 2:
                    (t0, o0), (t1, o1) = ps_[0], ps_[1]
                    pt = part_pool.tile(
                        [P, D], mybir.dt.bfloat16, tag="pt", name="cl"
                    )
                    nc.vector.tensor_max(
                        out=pt[:, :],
                        in0=t0[:, o0 : o0 + D],
                        in1=t1[:, o1 : o1 + D],
                    )
                    ps_ = [(pt, 0)] + ps_[2:]
                if sink is not None:
                    if len(ps_) == 1:
                        (t0, o0) = ps_[0]
                        nc.vector.tensor_max(
                            out=sink,
                            in0=t0[:, o0 : o0 + D],
                            in1=t0[:, o0 : o0 + D],
                        )
                    else:
                        (t0, o0), (t1, o1) = ps_[0], ps_[1]
                        nc.vector.tensor_max(
                            out=sink,
                            in0=t0[:, o0 : o0 + D],
                            in1=t1[:, o1 : o1 + D],
                        )
                    chunk_partials[ch] = []
                    return
                if len(ps_) == 2:
                    (t0, o0), (t1, o1) = ps_[0], ps_[1]
                    pt = part_pool.tile(
                        [P, D], mybir.dt.bfloat16, tag="pt", name="cl2"
                    )
                    nc.vector.tensor_max(
                        out=pt[:, :],
                        in0=t0[:, o0 : o0 + D],
                        in1=t1[:, o1 : o1 + D],
                    )
                    ps_ = [(pt, 0)]
                chunk_partials[ch] = ps_

            def flush_stores():
                nonlocal stored_to
                while stored_to < CHUNKS:
                    hi = min(stored_to + GPR_STORE_GROUP, CHUNKS)
                    if not all(done[stored_to:hi]):
                        return
                    nc.sync.dma_start(
                        out=out_view[:, stored_to:hi, :],
                        in_=res_view[:, stored_to:hi, :],
                    )
                    stored_to = hi

            rr = 0
            col = 0
            for call in plan:
                stream = call["stream"]
                ispair = stream >= 0
                b = call["blocks"]
                elem = 2 * D if ispair else D
                nidx = b * P
                slots = nidx // 16
                st = stage_pool.tile(
                    [P, GPR_CALL_BLOCKS * D], mybir.dt.bfloat16, tag="sst",
                    name="st",
                )
                nc.gpsimd.dma_gather(
                    out_ap=st[:, : b * elem].rearrange("p (b d) -> p b d", d=elem),
                    in_ap=(ptables[stream] if ispair else table)[:, :],
                    idxs_ap=idx_sb[:, col : col + slots],
                    num_idxs=nidx,
                    num_idxs_reg=nidx,
                    elem_size=elem,
                    single_packet=False,
                    queue_num=rr % 4,
                )
                rr += 1
                col += slots
                boff = 0
                for ch, _off, nb in call["segs"]:
                    w = 2 * nb if ispair else nb
                    chunk_partials[ch].append(tree_reduce(st, boff * elem, w))
                    boff += nb
                    key = (ch, stream)
                    got_s[key] += 1
                    if got_s[key] == exp_s[key]:
                        n_streams_left[ch] -= 1
                        if n_streams_left[ch] == 0:
                            collapse(ch, sink=res[:, ch * D : (ch + 1) * D])
                            done[ch] = True
                        else:
                            # stream finished with this chunk: shrink held
                            # partials to one tile
                            collapse(ch)
                flush_stores()
            flush_stores()

    nc.compile()
    return nc


# -------------------------------------------------------------------- api ---
def run_variant(np_inputs, **run_kwargs):
    """Run the kernel; returns (full f32 output, BassKernelResults)."""
    from concourse.bass_utils import run_bass_kernel_spmd

    in_maps, P_scheds, orders = _prep_gpair(**np_inputs)
    key = ("gpair", P_scheds)
    if key not in _nc_cache:
        _nc_cache[key] = _build_nc_gpair(P_scheds)
    res = run_bass_kernel_spmd(
        _nc_cache[key], in_maps, core_ids=list(range(N_CORES)), **run_kwargs
    )
    out = np.empty((N_NODES, D), np.float32)
    for core in range(N_CORES):
        r = np.asarray(res.results[core]["out"]).astype(np.float32)
        order = orders[core]
        valid = order >= 0
        out[core * NODES_PER_CORE + order[valid]] = r[valid]
    return out, res


def kernel(s_feats, neighbor_indices):
    out, _ = run_variant(
        {"s_feats": s_feats, "neighbor_indices": neighbor_indices}
    )
    return out


# revision 57
# speedup vs baseline: 1.9412x; 1.0336x over previous
"""GNN max-pool message passing kernel for 8 Trainium2 NeuronCores.

Problem: out[n] = max_k s_feats[neighbor_indices[n, k]]  (N=50000, K=32, D=128)

Strategy: data-parallel over destination nodes per the sharding hint;
s_feats is replicated into every core's HBM (bf16; tolerance is 2e-2 and
bf16 rounding is ~4e-3) and each core handles 6250 destination nodes.

The gather runs on InstDMAGatherAnt (SWDGE). Measured laws on real HW:
  - The Q7 cluster's descriptor-emission loop costs ~2.1 ns per index
    POSITION aggregate (positions = ceil(num_idxs/128)*128 per call),
    independent of elem_size (up to 16 KB/descriptor), queue count, or
    single_packet. Kernel time ~= head + positions*2.1ns + tail.
  - Mixing calls of different elem_size across the four SWDGE queues
    degrades the rate to ~2.4-3.3 ns/pos; uniform-size phases restore it.

So the optimization is INDEX-COUNT COMPRESSION ("gpair" variant): one
512 B descriptor can fetch TWO neighbor rows if they are adjacent under a
host-chosen table permutation. The host runs R=3 rounds of a greedy
max-weight path-forest over neighbor co-occurrence pairs (round r+1 on
the rows left uncovered by round r), giving permutations pi_0..pi_2 and
per-node pair lists. Pair probes read row j of a sliding-window pair
table ptable_r[j] = [s[pi_r[j]], s[pi_r[j+1]]] (elem 256); leftover rows
are single probes into the main table s[pi_0] (elem 128). This removes
~34% of index positions (~200k -> ~132k per core).

Scheduling: the gather grid needs a uniform per-chunk block count, so
nodes are re-bucketed into chunks by their per-round pair counts
(lexicographic sort) and chunk c uses P_r[c] = min over chunk nodes and
cores; dropped pairs fall back to singles. Calls are merged ACROSS
chunks (segments of a call may span chunks) into uniform sizes (8 blocks
for pairs, 16 for singles) and issued in uniform phases: pairs round 0,
1, 2, then singles. Per-chunk partial maxes are combined as streams
complete; trailing-negative trim is defused by reordering each chunk's
slot-127 node lists so every call's last index is non-negative.

The K-reduction is a tensor_tensor(max) binary tree over contiguous bf16
slices (TensorReduce has NO DVE perf mode; tensor_max on packed 2-byte
data runs in 2x_1p mode at 0.5 cyc/elem). Output stays bf16 on HW
(exact) and is converted to f32 on the host, which also un-permutes the
node order.

History (8 cores, HW exec): f32 one-row-per-desc 489 us -> bf16 480 ->
pairs v1 443 -> phase-separated 418 -> uniform stage tiles 376 ->
3-round pairs (this version).
"""

import numpy as np

N_NODES = 50000
K = 32
D = 128
N_CORES = 8
P = 128
NODES_PER_CORE = N_NODES // N_CORES  # 6250
SLOTS = (NODES_PER_CORE + P - 1) // P  # 49
PADDED = P * SLOTS  # 6272
CHUNKS = PADDED // P  # 49 chunks of 128 nodes

VARIANT = "gpair"

_nc_cache = {}

GPR_BASE = 25000  # signed int16 offsets for all tables
# Pairing rounds (one permutation + pair table each). Per-round per-node
# pair-count caps level the counts so the per-chunk min-capping keeps
# ~90% of the pairs (uncapped greedy loses ~25% to chunk minima).
GPR_CAPS = (3, 3, 2, 2, 2, 2, 2, 2, 2, 2, 2, 2, 2)
GPR_ROUNDS = len(GPR_CAPS)
GPR_STORE_GROUP = 8
GPR_CALL_BLOCKS = 16  # gather blocks per merged single call
GPR_PAIR_CALL_BLOCKS = 8  # pair calls: same 4 KB stage footprint as singles
# Pairs are compact-indexed per GROUP of chunks (all rounds together) so a
# chunk's pairs form ONE contiguous gather segment -> one big DVE tree per
# chunk instead of one per round (per-op overhead dominates the DVE).
GPR_GROUPS = 4
GPR_PAIR_CAP = 32768  # compact per-group pair-table capacity (rows)
GPR_SING_CAP = 24576  # compact single-table capacity (rows)


# ----------------------------------------------------------- host: pairs ---
def _gpair_path_forest(cand_sets, seed):
    """Greedy max-weight path forest over co-occurrence pairs of the given
    per-node row lists (list of int arrays). Returns pi (permutation of all
    N_NODES rows) maximizing per-set adjacent pairs."""
    rng = np.random.default_rng(seed)
    pairs = []
    for r in cand_sets:
        n = len(r)
        if n < 2:
            continue
        i, j = np.triu_indices(n, 1)
        pairs.append(np.stack([r[i], r[j]], axis=1))
    if not pairs:
        return np.arange(N_NODES, dtype=np.int32)
    pairs = np.concatenate(pairs, axis=0)
    pairs = np.sort(pairs, axis=1)
    pairs = pairs[pairs[:, 0] != pairs[:, 1]]
    pu, counts = np.unique(
        pairs[:, 0].astype(np.int64) * N_NODES + pairs[:, 1], return_counts=True
    )
    u = (pu // N_NODES).astype(np.int32)
    v = (pu % N_NODES).astype(np.int32)
    order = np.lexsort((rng.random(len(u)), -counts))
    u, v = u[order], v[order]
    deg = np.zeros(N_NODES, np.int8)
    parent = np.arange(N_NODES, dtype=np.int32)

    def find(x):
        while parent[x] != x:
            parent[x] = parent[parent[x]]
            x = parent[x]
        return x

    adj = [[] for _ in range(N_NODES)]
    for uu, vv in zip(u.tolist(), v.tolist()):
        if deg[uu] >= 2 or deg[vv] >= 2:
            continue
        ru, rv = find(uu), find(vv)
        if ru == rv:
            continue
        parent[ru] = rv
        deg[uu] += 1
        deg[vv] += 1
        adj[uu].append(vv)
        adj[vv].append(uu)
    visited = np.zeros(N_NODES, bool)
    pi = []
    for s in range(N_NODES):
        if visited[s] or len(adj[s]) == 2:
            continue
        cur, prev = s, -1
        while True:
            pi.append(cur)
            visited[cur] = True
            nxt = [x for x in adj[cur] if x != prev and not visited[x]]
            if not nxt:
                break
            prev, cur = cur, nxt[0]
    for s in range(N_NODES):
        if not visited[s]:
            pi.append(s)
    pi = np.asarray(pi, np.int32)
    assert len(pi) == N_NODES
    return pi


def _gpair_phase1(sets):
    """Per-core multi-round pairing.

    Returns dict with:
      pis[r]: permutation per round
      pos0: row -> position in pi_0
      pair_pos[r]: per node, array of pi_r start positions of its pairs
      pair_rows[r]: per node, [p, 2] rows of those pairs
      rows_left: per node, rows not covered by any round
      pn: [M, R] per-node pair counts
    """
    m = len(sets)
    rows_left = [sets[i].astype(np.int32) for i in range(m)]
    pis, pair_pos, pair_rows = [], [], []
    pn = np.zeros((m, GPR_ROUNDS), np.int32)
    for rnd in range(GPR_ROUNDS):
        pi = _gpair_path_forest(rows_left, seed=rnd)
        pos = np.empty(N_NODES, np.int64)
        pos[pi] = np.arange(N_NODES)
        pp_r, prow_r = [], []
        new_left = []
        for i in range(m):
            r = rows_left[i]
            if len(r) < 2:
                pp_r.append(np.empty(0, np.int32))
                prow_r.append(np.empty((0, 2), np.int32))
                new_left.append(r)
                continue
            pr = np.sort(pos[r]).astype(np.int64)
            starts = []
            j = 0
            taken = np.zeros(len(r), bool)
            while j < len(r) - 1 and len(starts) < GPR_CAPS[rnd]:
                if pr[j + 1] == pr[j] + 1:
                    starts.append(pr[j])
                    taken[j] = taken[j + 1] = True
                    j += 2
                else:
                    j += 1
            starts = np.asarray(starts, np.int64)
            pp_r.append(starts.astype(np.int32))
            prow_r.append(
                np.stack([pi[starts], pi[starts + 1]], axis=1).astype(np.int32)
                if len(starts)
                else np.empty((0, 2), np.int32)
            )
            pn[i, rnd] = len(starts)
            new_left.append(pi[pr[~taken]].astype(np.int32))
        rows_left = new_left
        pis.append(pi)
        pair_pos.append(pp_r)
        pair_rows.append(prow_r)
    pos0 = np.empty(N_NODES, np.int64)
    pos0[pis[0]] = np.arange(N_NODES)
    return {
        "pis": pis,
        "pos0": pos0,
        "pair_pos": pair_pos,
        "pair_rows": pair_rows,
        "rows_left": rows_left,
        "pn": pn,
    }


# ------------------------------------------------------------- call plan ---
def _gpair_groups(P_scheds):
    """Per-chunk group id, balancing total pair instances per group (each
    group's distinct pairs must fit the 32768-row compact table)."""
    per_chunk = [
        P * sum(P_scheds[r][c] for r in range(GPR_ROUNDS))
        for c in range(CHUNKS)
    ]
    total = sum(per_chunk)
    groups = []
    acc = 0
    for c in range(CHUNKS):
        g = min(int(acc * GPR_GROUPS / max(total, 1)), GPR_GROUPS - 1)
        groups.append(g)
        acc += per_chunk[c]
    return groups


def _gpair_call_plan(P_scheds):
    """Merged cross-chunk call plan, a pure function of the schedule.

    P_scheds: tuple of GPR_ROUNDS tuples of per-chunk pair counts.
    Streams: one pair stream per chunk GROUP (a chunk's pairs from all
    rounds are contiguous in its group's compact table; uniform 8-block
    calls, elem 256), then singles (uniform 16-block calls, elem 128).
    Uniform phases keep the Q7 emission at ~2.1 ns/position.

    Returns list of calls with keys stream (group index, or -1 for
    singles), blocks, segs=[(chunk, off_in_chunk, nblocks), ...]."""
    groups = _gpair_groups(P_scheds)
    plan = []
    for g in range(GPR_GROUPS):
        blocks = []
        for c in range(CHUNKS):
            if groups[c] != g:
                continue
            n = sum(P_scheds[r][c] for r in range(GPR_ROUNDS))
            blocks += [(c, o) for o in range(n)]
        for i in range(0, len(blocks), GPR_PAIR_CALL_BLOCKS):
            chunkb = blocks[i : i + GPR_PAIR_CALL_BLOCKS]
            segs = []
            for ch, off in chunkb:
                if segs and segs[-1][0] == ch:
                    segs[-1] = (ch, segs[-1][1], segs[-1][2] + 1)
                else:
                    segs.append((ch, off, 1))
            plan.append({"stream": g, "blocks": len(chunkb), "segs": segs})
    # the last chunk (pads + low-pair nodes) has the most single blocks:
    # emit it FIRST so the kernel tail isn't gated on its big reduce
    blocks = []
    for c in [CHUNKS - 1] + list(range(CHUNKS - 1)):
        n = K - 2 * sum(P_scheds[r][c] for r in range(GPR_ROUNDS))
        blocks += [(c, o) for o in range(n)]
    for i in range(0, len(blocks), GPR_CALL_BLOCKS):
        chunkb = blocks[i : i + GPR_CALL_BLOCKS]
        segs = []
        for ch, off in chunkb:
            if segs and segs[-1][0] == ch:
                segs[-1] = (ch, segs[-1][1], segs[-1][2] + 1)
            else:
                segs.append((ch, off, 1))
        plan.append({"stream": -1, "blocks": len(chunkb), "segs": segs})
    return plan


def _gpair_phase2(core_data, P_scheds):
    """Per-core: order nodes, build the merged-call idx array with COMPACT
    per-stream indexing: each stream's used pair-starts (or single
    positions) get ids 0..U-1 (U < 32768, so every int16 index is
    non-negative and the trailing-negative trim can never fire).

    Returns (idx array [128, total_slots] int16, node order, used):
    used[stream] = array of pi positions in id order (pair starts for
    pair streams, pi_0 positions for singles)."""
    pn = core_data["pn"]
    m = len(pn)
    order = np.lexsort(
        tuple(-pn[:, r] for r in reversed(range(GPR_ROUNDS)))
    ).astype(np.int32)
    order_pad = np.concatenate([order, np.full(PADDED - m, -1, np.int32)])
    plan = _gpair_call_plan(P_scheds)
    idmaps = {s: {} for s in list(range(GPR_GROUPS)) + [-1]}

    def to_id(stream, key):
        d = idmaps[stream]
        i = d.get(key)
        if i is None:
            i = len(d)
            d[key] = i
        return i

    groups = _gpair_groups(P_scheds)
    lists_by_chunk = []  # per chunk: {group: pair ids [P, n], -1: single ids}
    for c in range(CHUNKS):
        g = groups[c]
        caps = [P_scheds[r][c] for r in range(GPR_ROUNDS)]
        s_c = K - 2 * sum(caps)
        nodes = order_pad[c * P : (c + 1) * P]
        lists = {
            g: np.zeros((P, sum(caps)), np.int32),
            -1: np.zeros((P, s_c), np.int32),
        }
        for sl in range(P):
            n = nodes[sl]
            if n < 0:
                continue  # pads keep id 0: harmless duplicate reads
            extra_rows = []
            ids = []
            for r in range(GPR_ROUNDS):
                pp = core_data["pair_pos"][r][n]
                ids += [to_id(g, (r, int(p))) for p in pp[: caps[r]]]
                if len(pp) > caps[r]:
                    extra_rows.append(
                        core_data["pair_rows"][r][n][caps[r] :].reshape(-1)
                    )
            lists[g][sl] = ids
            sing_rows = np.concatenate(
                [core_data["rows_left"][n]] + extra_rows
            ) if extra_rows else core_data["rows_left"][n]
            assert len(sing_rows) == s_c, (c, sl, len(sing_rows), s_c)
            lists[-1][sl] = [
                to_id(-1, int(p)) for p in core_data["pos0"][sing_rows]
            ]
        lists_by_chunk.append(lists)
    used = {}
    for s, d in idmaps.items():
        cap = GPR_SING_CAP if s == -1 else GPR_PAIR_CAP
        assert len(d) <= cap, (s, len(d))
        u = [0] * max(len(d), 1)
        for key, i in d.items():
            u[i] = key
        used[s] = u
    all_vals = []
    for call in plan:
        s = call["stream"]
        for ch, off, nb in call["segs"]:
            all_vals.append(
                lists_by_chunk[ch][s][:, off : off + nb]
                .T.astype(np.int16)
                .reshape(-1)
            )
    flat = np.concatenate(all_vals)
    lanes = flat.reshape(-1, 16).T
    full = np.tile(np.ascontiguousarray(lanes), (8, 1))
    return full, order_pad, used


def _prep_gpair(s_feats, neighbor_indices):
    import ml_dtypes

    s = np.ascontiguousarray(np.asarray(s_feats), dtype=np.float32).astype(
        ml_dtypes.bfloat16
    )
    nb = np.asarray(neighbor_indices)
    cores = []
    for core in range(N_CORES):
        sets = nb[core * NODES_PER_CORE : (core + 1) * NODES_PER_CORE].astype(
            np.int32
        )
        cores.append(_gpair_phase1(sets))
    # shared schedule: per chunk, per round, min pair count across cores
    # after the lexicographic node sort; pad chunks get 0
    sorted_pn = []
    for cdat in cores:
        pn = cdat["pn"]
        o = np.lexsort(tuple(-pn[:, r] for r in reversed(range(GPR_ROUNDS))))
        sorted_pn.append(pn[o])
    P_scheds = []
    for r in range(GPR_ROUNDS):
        ps = []
        for c in range(CHUNKS):
            if (c + 1) * P > NODES_PER_CORE:
                ps.append(0)
                continue
            lo, hi = c * P, (c + 1) * P
            ps.append(min(int(sp[lo:hi, r].min()) for sp in sorted_pn))
        P_scheds.append(tuple(ps))
    P_scheds = tuple(P_scheds)
    in_maps = []
    orders = []
    for core in range(N_CORES):
        idx_full, order_pad, used = _gpair_phase2(cores[core], P_scheds)
        tabs = {"idx": idx_full}
        sing = np.asarray(used[-1], np.int64)
        t = np.zeros((GPR_SING_CAP, D), s.dtype)
        t[: len(sing)] = s[cores[core]["pis"][0][sing]]
        tabs["table"] = t
        pis = cores[core]["pis"]
        for g in range(GPR_GROUPS):
            keys = used[g]
            pt = np.zeros((GPR_PAIR_CAP, 2 * D), s.dtype)
            if keys and isinstance(keys[0], tuple):
                rr_ = np.asarray([k[0] for k in keys])
                st_ = np.asarray([k[1] for k in keys], np.int64)
                for r in range(GPR_ROUNDS):
                    sel = rr_ == r
                    if not sel.any():
                        continue
                    rows = np.nonzero(sel)[0]
                    pt[rows, :D] = s[pis[r][st_[sel]]]
                    pt[rows, D:] = s[pis[r][st_[sel] + 1]]
            tabs[f"gtable{g}"] = pt
        in_maps.append(tabs)
        orders.append(order_pad)
    return in_maps, P_scheds, orders


# ---------------------------------------------------------------- kernel ---
def _build_nc_gpair(P_scheds):
    import concourse.bacc as bacc
    import concourse.mybir as mybir
    import concourse.tile as tile
    from concourse import library_config

    nc = bacc.Bacc(
        "TRN2", target_bir_lowering=False, debug=False,
        dynamic_dma_scratch_size=49152, num_swdge_queues=4,
    )
    table = nc.dram_tensor(
        "table", [GPR_SING_CAP, D], mybir.dt.bfloat16, kind="ExternalInput"
    ).ap()
    ptables = [
        nc.dram_tensor(
            f"gtable{g}", [GPR_PAIR_CAP, 2 * D], mybir.dt.bfloat16,
            kind="ExternalInput",
        ).ap()
        for g in range(GPR_GROUPS)
    ]
    plan = _gpair_call_plan(P_scheds)
    total_slots = sum(call["blocks"] * P // 16 for call in plan)
    idx = nc.dram_tensor(
        "idx", [P, total_slots], mybir.dt.int16, kind="ExternalInput"
    ).ap()
    out = nc.dram_tensor(
        "out", [PADDED, D], mybir.dt.bfloat16, kind="ExternalOutput"
    ).ap()

    with tile.TileContext(nc) as tc:
        with (
            tc.tile_pool(name="pool", bufs=1) as pool,
            tc.tile_pool(name="stage", bufs=16) as stage_pool,
            tc.tile_pool(name="tmp", bufs=8) as tmp_pool,
            tc.tile_pool(name="parts", bufs=80) as part_pool,
        ):
            # preload the Q7 ucode library so its IRAM load overlaps the
            # idx DMA instead of delaying the first gather
            nc.gpsimd.load_library(library_config.mlp)
            idx_sb = pool.tile([P, total_slots], mybir.dt.int16, name="idx_sb")
            head_cols = min(total_slots, 256)
            nc.sync.dma_start(out=idx_sb[:, :head_cols], in_=idx[:, :head_cols])
            if head_cols < total_slots:
                nc.sync.dma_start(
                    out=idx_sb[:, head_cols:], in_=idx[:, head_cols:]
                )

            res = pool.tile([P, CHUNKS * D], mybir.dt.bfloat16, name="res")
            out_view = out.rearrange("(c p) d -> p c d", p=P)
            res_view = res[:, :].rearrange("p (c d) -> p c d", d=D)

            TMP_ELEMS = GPR_CALL_BLOCKS * D // 2  # max tree level = 8 blocks

            def tree_reduce(st, start_elems, wblocks):
                """Max-reduce wblocks width-D blocks at st[:, start_elems:]
                to one [P, D] block. Returns (tile, offset)."""
                stragglers = []
                cur, cur_off, n = st, start_elems, wblocks
                while n > 1:
                    h = n // 2
                    if n % 2:
                        stragglers.append((cur, cur_off + (n - 1) * D))
                    if h == 1:
                        dst = part_pool.tile(
                            [P, D], mybir.dt.bfloat16, tag="pt", name="tr1"
                        )
                    else:
                        dst = tmp_pool.tile(
                            [P, TMP_ELEMS], mybir.dt.bfloat16, tag="tmp",
                            name="tr",
                        )
                    nc.vector.tensor_max(
                        out=dst[:, : h * D],
                        in0=cur[:, cur_off : cur_off + h * D],
                        in1=cur[:, cur_off + h * D : cur_off + 2 * h * D],
                    )
                    cur, cur_off, n = dst, 0, h
                for sg, off in stragglers:
                    dst = part_pool.tile(
                        [P, D], mybir.dt.bfloat16, tag="pt", name="sg"
                    )
                    nc.vector.tensor_max(
                        out=dst[:, :],
                        in0=cur[:, cur_off : cur_off + D],
                        in1=sg[:, off : off + D],
                    )
                    cur, cur_off = dst, 0
                return cur, cur_off

            # per chunk, per stream: expected segment count
            exp_s = {}
            for call in plan:
                for ch, _o, _nb in call["segs"]:
                    exp_s[(ch, call["stream"])] = (
                        exp_s.get((ch, call["stream"]), 0) + 1
                    )
            got_s = {k: 0 for k in exp_s}
            n_streams_left = [0] * CHUNKS
            for (ch, _s), _v in exp_s.items():
                n_streams_left[ch] += 1
            chunk_partials = [[] for _ in range(CHUNKS)]
            done = [False] * CHUNKS
            stored_to = 0

            def collapse(ch, sink=None):
                ps_ = chunk_partials[ch]
                if sink is None and len(ps_) <= 1:
                    return
                while len(ps_) > 2:
                    (t0, o0), (t1, o1) = ps_[0], ps_[1]
                    pt = part_pool.tile(
                        [P, D], mybir.dt.bfloat16, tag="pt", name="cl"
                    )
                    nc.vector.tensor_max(
                        out=pt[:, :],
                        in0=t0[:, o0 : o0 + D],
                        in1=t1[:, o1 : o1 + D],
                    )
                    ps_ = [(pt, 0)] + ps_[2:]
                if sink is not None:
                    if len(ps_) == 1:
                        (t0, o0) = ps_[0]
                        nc.vector.tensor_max(
                            out=sink,
                            in0=t0[:, o0 : o0 + D],
                            in1=t0[:, o0 : o0 + D],
                        )
                    else:
                        (t0, o0), (t1, o1) = ps_[0], ps_[1]
                        nc.vector.tensor_max(
                            out=sink,
                            in0=t0[:, o0 : o0 + D],
                            in1=t1[:, o1 : o1 + D],
                        )
                    chunk_partials[ch] = []
                    return
                if len(ps_) == 2:
                    (t0, o0), (t1, o1) = ps_[0], ps_[1]
                    pt = part_pool.tile(
                        [P, D], mybir.dt.bfloat16, tag="pt", name="cl2"
                    )
                    nc.vector.tensor_max(
                        out=pt[:, :],
                        in0=t0[:, o0 : o0 + D],
                        in1=t1[:, o1 : o1 + D],
                    )
                    ps_ = [(pt, 0)]
                chunk_partials[ch] = ps_

            def flush_stores():
                nonlocal stored_to
                while stored_to < CHUNKS:
                    hi = min(stored_to + GPR_STORE_GROUP, CHUNKS)
                    if not all(done[stored_to:hi]):
                        return
                    nc.sync.dma_start(
                        out=out_view[:, stored_to:hi, :],
                        in_=res_view[:, stored_to:hi, :],
                    )
                    stored_to = hi

            rr = 0
            col = 0
            for call in plan:
                stream = call["stream"]
                ispair = stream >= 0
                b = call["blocks"]
                elem = 2 * D if ispair else D
                nidx = b * P
                slots = nidx // 16
                st = stage_pool.tile(
                    [P, GPR_CALL_BLOCKS * D], mybir.dt.bfloat16, tag="sst",
                    name="st",
                )
                nc.gpsimd.dma_gather(
                    out_ap=st[:, : b * elem].rearrange("p (b d) -> p b d", d=elem),
                    in_ap=(ptables[stream] if ispair else table)[:, :],
                    idxs_ap=idx_sb[:, col : col + slots],
                    num_idxs=nidx,
                    num_idxs_reg=nidx,
                    elem_size=elem,
                    single_packet=False,
                    queue_num=rr % 4,
                )
                rr += 1
                col += slots
                boff = 0
                for ch, _off, nb in call["segs"]:
                    w = 2 * nb if ispair else nb
                    chunk_partials[ch].append(tree_reduce(st, boff * elem, w))
                    boff += nb
                    key = (ch, stream)
                    got_s[key] += 1
                    if got_s[key] == exp_s[key]:
                        n_streams_left[ch] -= 1
                        if n_streams_left[ch] == 0:
                            collapse(ch, sink=res[:, ch * D : (ch + 1) * D])
                            done[ch] = True
                        else:
                            # stream finished with this chunk: shrink held
                            # partials to one tile
                            collapse(ch)
                flush_stores()
            flush_stores()

    nc.compile()
    return nc


# -------------------------------------------------------------------- api ---
def run_variant(np_inputs, **run_kwargs):
    """Run the kernel; returns (full f32 output, BassKernelResults)."""
    from concourse.bass_utils import run_bass_kernel_spmd

    in_maps, P_scheds, orders = _prep_gpair(**np_inputs)
    key = ("gpair", P_scheds)
    if key not in _nc_cache:
        _nc_cache[key] = _build_nc_gpair(P_scheds)
    res = run_bass_kernel_spmd(
        _nc_cache[key], in_maps, core_ids=list(range(N_CORES)), **run_kwargs
    )
    out = np.empty((N_NODES, D), np.float32)
    for core in range(N_CORES):
        r = np.asarray(res.results[core]["out"]).astype(np.float32)
        order = orders[core]
        valid = order >= 0
        out[core * NODES_PER_CORE + order[valid]] = r[valid]
    return out, res


def kernel(s_feats, neighbor_indices):
    out, _ = run_variant(
        {"s_feats": s_feats, "neighbor_indices": neighbor_indices}
    )
    return out


# revision 61
# speedup vs baseline: 1.9543x; 1.0067x over previous
"""GNN max-pool message passing kernel for 8 Trainium2 NeuronCores.

Problem: out[n] = max_k s_feats[neighbor_indices[n, k]]  (N=50000, K=32, D=128)

Strategy: data-parallel over destination nodes per the sharding hint;
s_feats is replicated into every core's HBM (bf16; tolerance is 2e-2 and
bf16 rounding is ~4e-3) and each core handles 6250 destination nodes.

The gather runs on InstDMAGatherAnt (SWDGE). Measured laws on real HW:
  - The Q7 cluster's descriptor-emission loop costs ~2.1 ns per index
    POSITION aggregate (positions = ceil(num_idxs/128)*128 per call),
    independent of elem_size (up to 16 KB/descriptor), queue count, or
    single_packet. Kernel time ~= head + positions*2.1ns + tail.
  - Mixing calls of different elem_size across the four SWDGE queues
    degrades the rate to ~2.4-3.3 ns/pos; uniform-size phases restore it.

So the optimization is INDEX-COUNT COMPRESSION ("gpair" variant): one
512 B descriptor can fetch TWO neighbor rows if they are adjacent under a
host-chosen table permutation. The host runs R=3 rounds of a greedy
max-weight path-forest over neighbor co-occurrence pairs (round r+1 on
the rows left uncovered by round r), giving permutations pi_0..pi_2 and
per-node pair lists. Pair probes read row j of a sliding-window pair
table ptable_r[j] = [s[pi_r[j]], s[pi_r[j+1]]] (elem 256); leftover rows
are single probes into the main table s[pi_0] (elem 128). This removes
~34% of index positions (~200k -> ~132k per core).

Scheduling: the gather grid needs a uniform per-chunk block count, so
nodes are re-bucketed into chunks by their per-round pair counts
(lexicographic sort) and chunk c uses P_r[c] = min over chunk nodes and
cores; dropped pairs fall back to singles. Calls are merged ACROSS
chunks (segments of a call may span chunks) into uniform sizes (8 blocks
for pairs, 16 for singles) and issued in uniform phases: pairs round 0,
1, 2, then singles. Per-chunk partial maxes are combined as streams
complete; trailing-negative trim is defused by reordering each chunk's
slot-127 node lists so every call's last index is non-negative.

The K-reduction is a tensor_tensor(max) binary tree over contiguous bf16
slices (TensorReduce has NO DVE perf mode; tensor_max on packed 2-byte
data runs in 2x_1p mode at 0.5 cyc/elem). Output stays bf16 on HW
(exact) and is converted to f32 on the host, which also un-permutes the
node order.

History (8 cores, HW exec): f32 one-row-per-desc 489 us -> bf16 480 ->
pairs v1 443 -> phase-separated 418 -> uniform stage tiles 376 ->
3-round pairs (this version).
"""

import numpy as np

N_NODES = 50000
K = 32
D = 128
N_CORES = 8
P = 128
NODES_PER_CORE = N_NODES // N_CORES  # 6250
SLOTS = (NODES_PER_CORE + P - 1) // P  # 49
PADDED = P * SLOTS  # 6272
CHUNKS = PADDED // P  # 49 chunks of 128 nodes

VARIANT = "gpair"

_nc_cache = {}

GPR_BASE = 25000  # signed int16 offsets for all tables
# Pairing rounds (one permutation + pair table each). Per-round per-node
# pair-count caps level the counts so the per-chunk min-capping keeps
# ~90% of the pairs (uncapped greedy loses ~25% to chunk minima).
GPR_CAPS = (3, 3, 2, 2, 2, 2, 2, 2, 2, 2, 2, 2, 2)
GPR_ROUNDS = len(GPR_CAPS)
GPR_STORE_GROUP = 8
GPR_CALL_BLOCKS = 16  # gather blocks per merged single call
GPR_PAIR_CALL_BLOCKS = 16  # pair calls: 8 KB stage tiles, fewer call overheads
# Pairs are compact-indexed per GROUP of chunks (all rounds together) so a
# chunk's pairs form ONE contiguous gather segment -> one big DVE tree per
# chunk instead of one per round (per-op overhead dominates the DVE).
GPR_GROUPS = 4
GPR_PAIR_CAP = 32768  # compact per-group pair-table capacity (rows)
GPR_SING_CAP = 24576  # compact single-table capacity (rows)


# ----------------------------------------------------------- host: pairs ---
def _gpair_path_forest(cand_sets, seed):
    """Greedy max-weight path forest over co-occurrence pairs of the given
    per-node row lists (list of int arrays). Returns pi (permutation of all
    N_NODES rows) maximizing per-set adjacent pairs."""
    rng = np.random.default_rng(seed)
    pairs = []
    for r in cand_sets:
        n = len(r)
        if n < 2:
            continue
        i, j = np.triu_indices(n, 1)
        pairs.append(np.stack([r[i], r[j]], axis=1))
    if not pairs:
        return np.arange(N_NODES, dtype=np.int32)
    pairs = np.concatenate(pairs, axis=0)
    pairs = np.sort(pairs, axis=1)
    pairs = pairs[pairs[:, 0] != pairs[:, 1]]
    pu, counts = np.unique(
        pairs[:, 0].astype(np.int64) * N_NODES + pairs[:, 1], return_counts=True
    )
    u = (pu // N_NODES).astype(np.int32)
    v = (pu % N_NODES).astype(np.int32)
    order = np.lexsort((rng.random(len(u)), -counts))
    u, v = u[order], v[order]
    deg = np.zeros(N_NODES, np.int8)
    parent = np.arange(N_NODES, dtype=np.int32)

    def find(x):
        while parent[x] != x:
            parent[x] = parent[parent[x]]
            x = parent[x]
        return x

    adj = [[] for _ in range(N_NODES)]
    for uu, vv in zip(u.tolist(), v.tolist()):
        if deg[uu] >= 2 or deg[vv] >= 2:
            continue
        ru, rv = find(uu), find(vv)
        if ru == rv:
            continue
        parent[ru] = rv
        deg[uu] += 1
        deg[vv] += 1
        adj[uu].append(vv)
        adj[vv].append(uu)
    visited = np.zeros(N_NODES, bool)
    pi = []
    for s in range(N_NODES):
        if visited[s] or len(adj[s]) == 2:
            continue
        cur, prev = s, -1
        while True:
            pi.append(cur)
            visited[cur] = True
            nxt = [x for x in adj[cur] if x != prev and not visited[x]]
            if not nxt:
                break
            prev, cur = cur, nxt[0]
    for s in range(N_NODES):
        if not visited[s]:
            pi.append(s)
    pi = np.asarray(pi, np.int32)
    assert len(pi) == N_NODES
    return pi


def _gpair_phase1(sets):
    """Per-core multi-round pairing.

    Returns dict with:
      pis[r]: permutation per round
      pos0: row -> position in pi_0
      pair_pos[r]: per node, array of pi_r start positions of its pairs
      pair_rows[r]: per node, [p, 2] rows of those pairs
      rows_left: per node, rows not covered by any round
      pn: [M, R] per-node pair counts
    """
    m = len(sets)
    rows_left = [sets[i].astype(np.int32) for i in range(m)]
    pis, pair_pos, pair_rows = [], [], []
    pn = np.zeros((m, GPR_ROUNDS), np.int32)
    for rnd in range(GPR_ROUNDS):
        pi = _gpair_path_forest(rows_left, seed=rnd)
        pos = np.empty(N_NODES, np.int64)
        pos[pi] = np.arange(N_NODES)
        pp_r, prow_r = [], []
        new_left = []
        for i in range(m):
            r = rows_left[i]
            if len(r) < 2:
                pp_r.append(np.empty(0, np.int32))
                prow_r.append(np.empty((0, 2), np.int32))
                new_left.append(r)
                continue
            pr = np.sort(pos[r]).astype(np.int64)
            starts = []
            j = 0
            taken = np.zeros(len(r), bool)
            while j < len(r) - 1 and len(starts) < GPR_CAPS[rnd]:
                if pr[j + 1] == pr[j] + 1:
                    starts.append(pr[j])
                    taken[j] = taken[j + 1] = True
                    j += 2
                else:
                    j += 1
            starts = np.asarray(starts, np.int64)
            pp_r.append(starts.astype(np.int32))
            prow_r.append(
                np.stack([pi[starts], pi[starts + 1]], axis=1).astype(np.int32)
                if len(starts)
                else np.empty((0, 2), np.int32)
            )
            pn[i, rnd] = len(starts)
            new_left.append(pi[pr[~taken]].astype(np.int32))
        rows_left = new_left
        pis.append(pi)
        pair_pos.append(pp_r)
        pair_rows.append(prow_r)
    pos0 = np.empty(N_NODES, np.int64)
    pos0[pis[0]] = np.arange(N_NODES)
    return {
        "pis": pis,
        "pos0": pos0,
        "pair_pos": pair_pos,
        "pair_rows": pair_rows,
        "rows_left": rows_left,
        "pn": pn,
    }


# ------------------------------------------------------------- call plan ---
def _gpair_groups(P_scheds):
    """Per-chunk group id, balancing total pair instances per group (each
    group's distinct pairs must fit the 32768-row compact table)."""
    per_chunk = [
        P * sum(P_scheds[r][c] for r in range(GPR_ROUNDS))
        for c in range(CHUNKS)
    ]
    total = sum(per_chunk)
    groups = []
    acc = 0
    for c in range(CHUNKS):
        g = min(int(acc * GPR_GROUPS / max(total, 1)), GPR_GROUPS - 1)
        groups.append(g)
        acc += per_chunk[c]
    return groups


def _gpair_call_plan(P_scheds):
    """Merged cross-chunk call plan, a pure function of the schedule.

    P_scheds: tuple of GPR_ROUNDS tuples of per-chunk pair counts.
    Streams: one pair stream per chunk GROUP (a chunk's pairs from all
    rounds are contiguous in its group's compact table; uniform 8-block
    calls, elem 256), then singles (uniform 16-block calls, elem 128).
    Uniform phases keep the Q7 emission at ~2.1 ns/position.

    Returns list of calls with keys stream (group index, or -1 for
    singles), blocks, segs=[(chunk, off_in_chunk, nblocks), ...]."""
    groups = _gpair_groups(P_scheds)
    plan = []
    for g in range(GPR_GROUPS):
        blocks = []
        for c in range(CHUNKS):
            if groups[c] != g:
                continue
            n = sum(P_scheds[r][c] for r in range(GPR_ROUNDS))
            blocks += [(c, o) for o in range(n)]
        for i in range(0, len(blocks), GPR_PAIR_CALL_BLOCKS):
            chunkb = blocks[i : i + GPR_PAIR_CALL_BLOCKS]
            segs = []
            for ch, off in chunkb:
                if segs and segs[-1][0] == ch:
                    segs[-1] = (ch, segs[-1][1], segs[-1][2] + 1)
                else:
                    segs.append((ch, off, 1))
            plan.append({"stream": g, "blocks": len(chunkb), "segs": segs})
    # the last chunk (pads + low-pair nodes) has the most single blocks:
    # emit it FIRST so the kernel tail isn't gated on its big reduce
    blocks = []
    for c in [CHUNKS - 1] + list(range(CHUNKS - 1)):
        n = K - 2 * sum(P_scheds[r][c] for r in range(GPR_ROUNDS))
        blocks += [(c, o) for o in range(n)]
    for i in range(0, len(blocks), GPR_CALL_BLOCKS):
        chunkb = blocks[i : i + GPR_CALL_BLOCKS]
        segs = []
        for ch, off in chunkb:
            if segs and segs[-1][0] == ch:
                segs[-1] = (ch, segs[-1][1], segs[-1][2] + 1)
            else:
                segs.append((ch, off, 1))
        plan.append({"stream": -1, "blocks": len(chunkb), "segs": segs})
    return plan


def _gpair_phase2(core_data, P_scheds):
    """Per-core: order nodes, build the merged-call idx array with COMPACT
    per-stream indexing: each stream's used pair-starts (or single
    positions) get ids 0..U-1 (U < 32768, so every int16 index is
    non-negative and the trailing-negative trim can never fire).

    Returns (idx array [128, total_slots] int16, node order, used):
    used[stream] = array of pi positions in id order (pair starts for
    pair streams, pi_0 positions for singles)."""
    pn = core_data["pn"]
    m = len(pn)
    order = np.lexsort(
        tuple(-pn[:, r] for r in reversed(range(GPR_ROUNDS)))
    ).astype(np.int32)
    order_pad = np.concatenate([order, np.full(PADDED - m, -1, np.int32)])
    plan = _gpair_call_plan(P_scheds)
    idmaps = {s: {} for s in list(range(GPR_GROUPS)) + [-1]}

    def to_id(stream, key):
        d = idmaps[stream]
        i = d.get(key)
        if i is None:
            i = len(d)
            d[key] = i
        return i

    groups = _gpair_groups(P_scheds)
    lists_by_chunk = []  # per chunk: {group: pair ids [P, n], -1: single ids}
    for c in range(CHUNKS):
        g = groups[c]
        caps = [P_scheds[r][c] for r in range(GPR_ROUNDS)]
        s_c = K - 2 * sum(caps)
        nodes = order_pad[c * P : (c + 1) * P]
        lists = {
            g: np.zeros((P, sum(caps)), np.int32),
            -1: np.zeros((P, s_c), np.int32),
        }
        for sl in range(P):
            n = nodes[sl]
            if n < 0:
                continue  # pads keep id 0: harmless duplicate reads
            extra_rows = []
            ids = []
            for r in range(GPR_ROUNDS):
                pp = core_data["pair_pos"][r][n]
                ids += [to_id(g, (r, int(p))) for p in pp[: caps[r]]]
                if len(pp) > caps[r]:
                    extra_rows.append(
                        core_data["pair_rows"][r][n][caps[r] :].reshape(-1)
                    )
            lists[g][sl] = ids
            sing_rows = np.concatenate(
                [core_data["rows_left"][n]] + extra_rows
            ) if extra_rows else core_data["rows_left"][n]
            assert len(sing_rows) == s_c, (c, sl, len(sing_rows), s_c)
            lists[-1][sl] = [
                to_id(-1, int(p)) for p in core_data["pos0"][sing_rows]
            ]
        lists_by_chunk.append(lists)
    used = {}
    for s, d in idmaps.items():
        cap = GPR_SING_CAP if s == -1 else GPR_PAIR_CAP
        assert len(d) <= cap, (s, len(d))
        u = [0] * max(len(d), 1)
        for key, i in d.items():
            u[i] = key
        used[s] = u
    all_vals = []
    for call in plan:
        s = call["stream"]
        for ch, off, nb in call["segs"]:
            all_vals.append(
                lists_by_chunk[ch][s][:, off : off + nb]
                .T.astype(np.int16)
                .reshape(-1)
            )
    flat = np.concatenate(all_vals)
    lanes = flat.reshape(-1, 16).T
    full = np.tile(np.ascontiguousarray(lanes), (8, 1))
    return full, order_pad, used


def _prep_gpair(s_feats, neighbor_indices):
    import ml_dtypes

    s = np.ascontiguousarray(np.asarray(s_feats), dtype=np.float32).astype(
        ml_dtypes.bfloat16
    )
    nb = np.asarray(neighbor_indices)
    cores = []
    for core in range(N_CORES):
        sets = nb[core * NODES_PER_CORE : (core + 1) * NODES_PER_CORE].astype(
            np.int32
        )
        cores.append(_gpair_phase1(sets))
    # shared schedule: per chunk, per round, min pair count across cores
    # after the lexicographic node sort; pad chunks get 0
    sorted_pn = []
    for cdat in cores:
        pn = cdat["pn"]
        o = np.lexsort(tuple(-pn[:, r] for r in reversed(range(GPR_ROUNDS))))
        sorted_pn.append(pn[o])
    P_scheds = []
    for r in range(GPR_ROUNDS):
        ps = []
        for c in range(CHUNKS):
            if (c + 1) * P > NODES_PER_CORE:
                ps.append(0)
                continue
            lo, hi = c * P, (c + 1) * P
            ps.append(min(int(sp[lo:hi, r].min()) for sp in sorted_pn))
        P_scheds.append(tuple(ps))
    P_scheds = tuple(P_scheds)
    in_maps = []
    orders = []
    for core in range(N_CORES):
        idx_full, order_pad, used = _gpair_phase2(cores[core], P_scheds)
        tabs = {"idx": idx_full}
        sing = np.asarray(used[-1], np.int64)
        t = np.zeros((GPR_SING_CAP, D), s.dtype)
        t[: len(sing)] = s[cores[core]["pis"][0][sing]]
        tabs["table"] = t
        pis = cores[core]["pis"]
        for g in range(GPR_GROUPS):
            keys = used[g]
            pt = np.zeros((GPR_PAIR_CAP, 2 * D), s.dtype)
            if keys and isinstance(keys[0], tuple):
                rr_ = np.asarray([k[0] for k in keys])
                st_ = np.asarray([k[1] for k in keys], np.int64)
                for r in range(GPR_ROUNDS):
                    sel = rr_ == r
                    if not sel.any():
                        continue
                    rows = np.nonzero(sel)[0]
                    pt[rows, :D] = s[pis[r][st_[sel]]]
                    pt[rows, D:] = s[pis[r][st_[sel] + 1]]
            tabs[f"gtable{g}"] = pt
        in_maps.append(tabs)
        orders.append(order_pad)
    return in_maps, P_scheds, orders


# ---------------------------------------------------------------- kernel ---
def _build_nc_gpair(P_scheds):
    import concourse.bacc as bacc
    import concourse.mybir as mybir
    import concourse.tile as tile
    from concourse import library_config

    nc = bacc.Bacc(
        "TRN2", target_bir_lowering=False, debug=False,
        dynamic_dma_scratch_size=49152, num_swdge_queues=4,
    )
    table = nc.dram_tensor(
        "table", [GPR_SING_CAP, D], mybir.dt.bfloat16, kind="ExternalInput"
    ).ap()
    ptables = [
        nc.dram_tensor(
            f"gtable{g}", [GPR_PAIR_CAP, 2 * D], mybir.dt.bfloat16,
            kind="ExternalInput",
        ).ap()
        for g in range(GPR_GROUPS)
    ]
    plan = _gpair_call_plan(P_scheds)
    total_slots = sum(call["blocks"] * P // 16 for call in plan)
    idx = nc.dram_tensor(
        "idx", [P, total_slots], mybir.dt.int16, kind="ExternalInput"
    ).ap()
    out = nc.dram_tensor(
        "out", [PADDED, D], mybir.dt.bfloat16, kind="ExternalOutput"
    ).ap()

    with tile.TileContext(nc) as tc:
        with (
            tc.tile_pool(name="pool", bufs=1) as pool,
            tc.tile_pool(name="stage", bufs=9) as stage_pool,
            tc.tile_pool(name="tmp", bufs=8) as tmp_pool,
            tc.tile_pool(name="parts", bufs=64) as part_pool,
        ):
            # preload the Q7 ucode library so its IRAM load overlaps the
            # idx DMA instead of delaying the first gather
            nc.gpsimd.load_library(library_config.mlp)
            idx_sb = pool.tile([P, total_slots], mybir.dt.int16, name="idx_sb")
            head_cols = min(total_slots, 256)
            nc.sync.dma_start(out=idx_sb[:, :head_cols], in_=idx[:, :head_cols])
            if head_cols < total_slots:
                nc.sync.dma_start(
                    out=idx_sb[:, head_cols:], in_=idx[:, head_cols:]
                )

            res = pool.tile([P, CHUNKS * D], mybir.dt.bfloat16, name="res")
            out_view = out.rearrange("(c p) d -> p c d", p=P)
            res_view = res[:, :].rearrange("p (c d) -> p c d", d=D)

            TMP_ELEMS = GPR_CALL_BLOCKS * D  # pair trees reach 32 width-D blocks

            def tree_reduce(st, start_elems, wblocks):
                """Max-reduce wblocks width-D blocks at st[:, start_elems:]
                to one [P, D] block. Returns (tile, offset)."""
                stragglers = []
                cur, cur_off, n = st, start_elems, wblocks
                while n > 1:
                    h = n // 2
                    if n % 2:
                        stragglers.append((cur, cur_off + (n - 1) * D))
                    if h == 1:
                        dst = part_pool.tile(
                            [P, D], mybir.dt.bfloat16, tag="pt", name="tr1"
                        )
                    else:
                        dst = tmp_pool.tile(
                            [P, TMP_ELEMS], mybir.dt.bfloat16, tag="tmp",
                            name="tr",
                        )
                    nc.vector.tensor_max(
                        out=dst[:, : h * D],
                        in0=cur[:, cur_off : cur_off + h * D],
                        in1=cur[:, cur_off + h * D : cur_off + 2 * h * D],
                    )
                    cur, cur_off, n = dst, 0, h
                for sg, off in stragglers:
                    dst = part_pool.tile(
                        [P, D], mybir.dt.bfloat16, tag="pt", name="sg"
                    )
                    nc.vector.tensor_max(
                        out=dst[:, :],
                        in0=cur[:, cur_off : cur_off + D],
                        in1=sg[:, off : off + D],
                    )
                    cur, cur_off = dst, 0
                return cur, cur_off

            # per chunk, per stream: expected segment count
            exp_s = {}
            for call in plan:
                for ch, _o, _nb in call["segs"]:
                    exp_s[(ch, call["stream"])] = (
                        exp_s.get((ch, call["stream"]), 0) + 1
                    )
            got_s = {k: 0 for k in exp_s}
            n_streams_left = [0] * CHUNKS
            for (ch, _s), _v in exp_s.items():
                n_streams_left[ch] += 1
            chunk_partials = [[] for _ in range(CHUNKS)]
            done = [False] * CHUNKS
            stored_to = 0

            def collapse(ch, sink=None):
                ps_ = chunk_partials[ch]
                if sink is None and len(ps_) <= 1:
                    return
                while len(ps_) > 2:
                    (t0, o0), (t1, o1) = ps_[0], ps_[1]
                    pt = part_pool.tile(
                        [P, D], mybir.dt.bfloat16, tag="pt", name="cl"
                    )
                    nc.vector.tensor_max(
                        out=pt[:, :],
                        in0=t0[:, o0 : o0 + D],
                        in1=t1[:, o1 : o1 + D],
                    )
                    ps_ = [(pt, 0)] + ps_[2:]
                if sink is not None:
                    if len(ps_) == 1:
                        (t0, o0) = ps_[0]
                        nc.vector.tensor_max(
                            out=sink,
                            in0=t0[:, o0 : o0 + D],
                            in1=t0[:, o0 : o0 + D],
                        )
                    else:
                        (t0, o0), (t1, o1) = ps_[0], ps_[1]
                        nc.vector.tensor_max(
                            out=sink,
                            in0=t0[:, o0 : o0 + D],
                            in1=t1[:, o1 : o1 + D],
                        )
                    chunk_partials[ch] = []
                    return
                if len(ps_) == 2:
                    (t0, o0), (t1, o1) = ps_[0], ps_[1]
                    pt = part_pool.tile(
                        [P, D], mybir.dt.bfloat16, tag="pt", name="cl2"
                    )
                    nc.vector.tensor_max(
                        out=pt[:, :],
                        in0=t0[:, o0 : o0 + D],
                        in1=t1[:, o1 : o1 + D],
                    )
                    ps_ = [(pt, 0)]
                chunk_partials[ch] = ps_

            def flush_stores():
                nonlocal stored_to
                while stored_to < CHUNKS:
                    hi = min(stored_to + GPR_STORE_GROUP, CHUNKS)
                    if not all(done[stored_to:hi]):
                        return
                    nc.sync.dma_start(
                        out=out_view[:, stored_to:hi, :],
                        in_=res_view[:, stored_to:hi, :],
                    )
                    stored_to = hi

            rr = 0
            col = 0
            for call in plan:
                stream = call["stream"]
                ispair = stream >= 0
                b = call["blocks"]
                elem = 2 * D if ispair else D
                nidx = b * P
                slots = nidx // 16
                st = stage_pool.tile(
                    [P, GPR_PAIR_CALL_BLOCKS * 2 * D], mybir.dt.bfloat16,
                    tag="sst", name="st",
                )
                nc.gpsimd.dma_gather(
                    out_ap=st[:, : b * elem].rearrange("p (b d) -> p b d", d=elem),
                    in_ap=(ptables[stream] if ispair else table)[:, :],
                    idxs_ap=idx_sb[:, col : col + slots],
                    num_idxs=nidx,
                    num_idxs_reg=nidx,
                    elem_size=elem,
                    single_packet=False,
                    queue_num=rr % 4,
                )
                rr += 1
                col += slots
                boff = 0
                for ch, _off, nb in call["segs"]:
                    w = 2 * nb if ispair else nb
                    chunk_partials[ch].append(tree_reduce(st, boff * elem, w))
                    boff += nb
                    key = (ch, stream)
                    got_s[key] += 1
                    if got_s[key] == exp_s[key]:
                        n_streams_left[ch] -= 1
                        if n_streams_left[ch] == 0:
                            collapse(ch, sink=res[:, ch * D : (ch + 1) * D])
                            done[ch] = True
                        else:
                            # stream finished with this chunk: shrink held
                            # partials to one tile
                            collapse(ch)
                flush_stores()
            flush_stores()

    nc.compile()
    return nc


# -------------------------------------------------------------------- api ---
def run_variant(np_inputs, **run_kwargs):
    """Run the kernel; returns (full f32 output, BassKernelResults)."""
    from concourse.bass_utils import run_bass_kernel_spmd

    in_maps, P_scheds, orders = _prep_gpair(**np_inputs)
    key = ("gpair", P_scheds)
    if key not in _nc_cache:
        _nc_cache[key] = _build_nc_gpair(P_scheds)
    res = run_bass_kernel_spmd(
        _nc_cache[key], in_maps, core_ids=list(range(N_CORES)), **run_kwargs
    )
    out = np.empty((N_NODES, D), np.float32)
    for core in range(N_CORES):
        r = np.asarray(res.results[core]["out"]).astype(np.float32)
        order = orders[core]
        valid = order >= 0
        out[core * NODES_PER_CORE + order[valid]] = r[valid]
    return out, res


def kernel(s_feats, neighbor_indices):
    out, _ = run_variant(
        {"s_feats": s_feats, "neighbor_indices": neighbor_indices}
    )
    return out
